# revision 1
# baseline (speedup 1.0000x reference)
"""Trainium2 Bass kernel for a 2-layer cross-encoder (CrossEncoder).

Model: B=2, NQ=NKV=2048, E=512, H=8 (d_head=64), MLP=2048, depth=2, fp32 I/O.

Sharding (8 cores, no collectives): core c handles batch b=c//4 and query
rows [qc*512, (qc+1)*512) with qc=c%4.  Each core computes the full KV
projections for its batch so every core produces its output slice
independently.

Numerics: heavy matmuls (Q/K/V/O projections, FFN, attn@V) run in fp8e4m3
with DoubleRow perf mode (2 k-chunks contracted per pass).  Weights are
pre-scaled x64 on the host so their mass sits in fp8's normal range; the
inverse scale is folded into the PSUM consumers.  q/k are stored x8 in
bf16 (scores matmul is fp8-rate anyway, bf16 costs the same and is more
accurate).  exp() of the scores is split between the Activation engine
(true Exp, fp8 out) and DVE (Schraudolph bit-trick exp directly into fp8
bits via an int8 round).  The softmax denominator comes free from 64
constant columns appended per head to V (rows 64..127 of the attn@V psum
all hold the per-query sum), so the normalizer is one reciprocal + one
multiply, no replicate matmul.  LayerNorm statistics, softmax
normalization and the residual stream stay fp32.  LN affine params and
all biases are folded into weights / matmul bias-rows on the host; the
k-bias is dropped entirely (softmax is invariant to it).
"""

import numpy as np
import ml_dtypes

import concourse.bass as bass
import concourse.bacc as bacc
import concourse.mybir as mybir
import concourse.tile as tile
from concourse import bass_utils, masks
from contextlib import ExitStack

P = 128
E = 512
EC = E // P        # 4 chunks of the embedding dim
SE = EC // 2       # 2 DoubleRow super-chunks
NQ = 512           # query rows per core
QC = NQ // P       # 4 query chunks
NKV = 2048
KC = NKV // P      # 16 key chunks of 128
KN = NKV // 512    # 4 key chunks of 512
GK = KC // 2       # 8 key pair-groups
H = 8
DH = 64
MLP = 2048
MC = MLP // P      # 16 mlp chunks of 128
SM = MC // 2       # 8 DoubleRow super-chunks
L = 2
LN_EPS = 1e-5
F32 = mybir.dt.float32
BF16 = mybir.dt.bfloat16
FP8 = mybir.dt.float8e4
I8 = mybir.dt.int8
AF = mybir.ActivationFunctionType
ALU = mybir.AluOpType
DRM = mybir.MatmulPerfMode.DoubleRow

WS = 64.0                       # fp8 weight pre-scale (host side)
QKS = 8.0                       # q/k storage scale
SCALE = DH ** -0.5
EXPS = SCALE / (QKS * QKS)      # exp scale applied to scores psum (=1/512)
AOS = 64.0                      # attnout storage scale (fp8 subnormal guard)
SCH_A = (8.0 / np.log(2.0)) * EXPS   # Schraudolph slope for fp8e4 bits
SCH_B = 56.0 - 0.47                  # fp8e4 exponent bias term - rms shift
EXP_PAT = "ADADADAA"            # per-head exp engine assignment (Act/DVE interleaved)

_CACHE = {}


def _build():
    """Build the per-core Bass program (identical on all 8 cores)."""
    nc = bacc.Bacc("TRN2", target_bir_lowering=False, debug=False, num_devices=8)

    xq_d = nc.dram_tensor("xq", [NQ, E], F32, kind="ExternalInput").ap()
    xkv_d = nc.dram_tensor("xkv", [NKV, E], F32, kind="ExternalInput").ap()
    wd = []
    for l in range(L):
        wd.append({
            "wq8": nc.dram_tensor(f"wq8_{l}", [P, SE * 2 * E], FP8, kind="ExternalInput").ap(),
            "wk8": nc.dram_tensor(f"wk8_{l}", [P, SE * 2 * E], FP8, kind="ExternalInput").ap(),
            "wv8": nc.dram_tensor(f"wv8_{l}", [P, SE * 2 * E], FP8, kind="ExternalInput").ap(),
            "wo8": nc.dram_tensor(f"wo8_{l}", [P, SE * 2 * E], FP8, kind="ExternalInput").ap(),
            "w18": nc.dram_tensor(f"w18_{l}", [P, SE * 2 * MLP], FP8, kind="ExternalInput").ap(),
            "w28": nc.dram_tensor(f"w28_{l}", [P, SM * 2 * E], FP8, kind="ExternalInput").ap(),
            "w1r8": nc.dram_tensor(f"w1r8_{l}", [P, SE * 2 * MLP], FP8, kind="ExternalInput").ap(),
            "w2r8": nc.dram_tensor(f"w2r8_{l}", [P, SM * 2 * E], FP8, kind="ExternalInput").ap(),
            "bq": nc.dram_tensor(f"bq_{l}", [P, EC], F32, kind="ExternalInput").ap(),
            "b1": nc.dram_tensor(f"b1_{l}", [P, MC], F32, kind="ExternalInput").ap(),
            "bo_row": nc.dram_tensor(f"bo_row_{l}", [1, E], BF16, kind="ExternalInput").ap(),
            "b2_row": nc.dram_tensor(f"b2_row_{l}", [1, E], BF16, kind="ExternalInput").ap(),
        })
    y_d = nc.dram_tensor("y", [NQ, E], F32, kind="ExternalOutput").ap()

    with tile.TileContext(nc) as tc, ExitStack() as ctx:
        const_pool = ctx.enter_context(tc.tile_pool(name="const", bufs=1))
        ident = const_pool.tile([P, P], BF16)
        masks.make_identity(nc, ident)
        ones1 = const_pool.tile([1, P], BF16)
        nc.gpsimd.memset(ones1[:], 1.0)
        eps_col = const_pool.tile([P, 1], F32)
        nc.gpsimd.memset(eps_col[:], LN_EPS)

        stats_pool = ctx.enter_context(tc.tile_pool(name="stats", bufs=12))

        def ln_rstd(x_ap):
            """LayerNorm stats: returns (bnag, rstd) [P,1] tiles (fp32)."""
            bnst = stats_pool.tile([P, 6], F32, name="bnst")
            nc.vector.bn_stats(bnst[:], x_ap)
            bnag = stats_pool.tile([P, 2], F32, name="bnag")
            nc.vector.bn_aggr(bnag[:], bnst[:])
            sq = stats_pool.tile([P, 1], F32, name="sq")
            nc.scalar.activation(sq[:], bnag[:, 1:2], AF.Sqrt, bias=eps_col[:])
            rstd = stats_pool.tile([P, 1], F32, name="rstd")
            nc.vector.reciprocal(rstd[:], sq[:])
            return bnag, rstd

        # Residual stream: 4 fp32 tiles of [128, 512].
        xq_pool = ctx.enter_context(tc.tile_pool(name="xq", bufs=1))
        xq = []
        for i in range(QC):
            t = xq_pool.tile([P, E], F32, name=f"xq{i}", tag=f"xq{i}")
            nc.sync.dma_start(t[:], xq_d[i * P:(i + 1) * P, :])
            xq.append(t[:])

        # hkv^T in fp8 DoubleRow pair layout: tile s holds E-chunks (2s, 2s+1)
        # as the pair dim -> [128, 2, NKV].
        hkvT_pool = ctx.enter_context(tc.tile_pool(name="hkvT", bufs=1))
        hkvT = [
            hkvT_pool.tile([P, 2, NKV], FP8, name=f"hkvT{s}", tag=f"hkvT{s}")
            for s in range(SE)
        ]

        # PSUM pools (8 banks): pp 2 + ss 2x2 + att 2 = 8.
        pp_pool = ctx.enter_context(tc.tile_pool(name="pp", bufs=2, space="PSUM"))
        ss_pool = ctx.enter_context(tc.tile_pool(name="ss", bufs=2, space="PSUM"))
        att_pool = ctx.enter_context(tc.tile_pool(name="attp", bufs=2, space="PSUM"))

        # Weight tiles for both layers live in SBUF simultaneously.
        wpool = ctx.enter_context(tc.tile_pool(name="w", bufs=1))

        def alloc_w_crit(l):
            d = {}
            for nm, sz in (("wq8", SE * 2 * E), ("wk8", SE * 2 * E), ("wv8", SE * 2 * E)):
                d[nm] = wpool.tile([P, sz], FP8, name=f"{nm}_{l}")
                nc.sync.dma_start(d[nm][:], wd[l][nm])
            d["bq"] = wpool.tile([P, EC], F32, name=f"bq_{l}")
            nc.sync.dma_start(d["bq"][:], wd[l]["bq"])
            return d

        def alloc_w_rest(d, l):
            for nm, sz, dt in (("wo8", SE * 2 * E, FP8), ("w18", SE * 2 * MLP, FP8),
                               ("w28", SM * 2 * E, FP8), ("w1r8", SE * 2 * MLP, FP8),
                               ("w2r8", SM * 2 * E, FP8)):
                d[nm] = wpool.tile([P, sz], dt, name=f"{nm}_{l}")
                nc.sync.dma_start(d[nm][:], wd[l][nm])
            d["b1"] = wpool.tile([P, MC], F32, name=f"b1_{l}")
            nc.sync.dma_start(d["b1"][:], wd[l]["b1"])
            d["bo_row"] = wpool.tile([1, E], BF16, name=f"bo_row_{l}")
            nc.sync.dma_start(d["bo_row"][:], wd[l]["bo_row"])
            d["b2_row"] = wpool.tile([1, E], BF16, name=f"b2_row_{l}")
            nc.sync.dma_start(d["b2_row"][:], wd[l]["b2_row"])
            return d

        def w_slice(wtile, s, c0, c1, S=SE):
            """[P, S*2*cols] fp8 tile -> [128, 2, c1-c0] DR stationary slice."""
            return wtile[:].rearrange("p (s j c) -> p s j c", s=S, j=2)[:, s, :, c0:c1]

        # LN + transpose into fp8 pair tiles.  Blocks are batched 8 per psum
        # bank (2 source tiles x 4 E-chunks) and copied out 2 blocks at a
        # time per destination pair-tile.
        def ln_transpose_pair(src0, src1, dstT, tok0, copy_par, resT=None):
            pt = pp_pool.tile([P, E], F32, name="pp", tag="pp")
            ptb = pt[:].bitcast(BF16)
            for sub, src in enumerate((src0, src1)):
                bnag, rstd = ln_rstd(src)
                hq_t = work.tile([P, E], BF16, name="hq_t", bufs=4)
                nc.gpsimd.tensor_scalar(
                    hq_t[:], src, bnag[:, 0:1], rstd[:], op0=ALU.subtract, op1=ALU.mult
                )
                for e in range(EC):
                    col = (e * 2 + sub) * P
                    nc.tensor.matmul(
                        ptb[:, col:col + P], hq_t[:, e * P:(e + 1) * P], ident[:],
                        is_transpose=True,
                        start=(sub == 0 and e == 0), stop=(sub == 1 and e == EC - 1),
                        skip_group_check=True,
                    )
            for s in range(SE):
                in_ap = ptb[:, s * 512:(s + 1) * 512].rearrange(
                    "p (j u c) -> p j u c", j=2, u=2
                )
                out_ap = dstT[s][:, :, tok0:tok0 + 256].rearrange(
                    "p j (u c) -> p j u c", u=2
                )
                if (s + copy_par) % 2:
                    nc.scalar.copy(out_ap, in_ap)
                else:
                    nc.vector.tensor_copy(out_ap, in_ap)
                if resT is not None:
                    # fp8 residual of the LN activations (subnormal fixed
                    # point) for the FFN1 correction pass.
                    res_ap = resT[s][:, :, tok0:tok0 + 256].rearrange(
                        "p j (u c) -> p j u c", u=2
                    )
                    nc.vector.scalar_tensor_tensor(
                        res_ap, out_ap, -1.0, in_ap, op0=ALU.mult, op1=ALU.add
                    )

        # ---- hkv^T setup: LN1-core of x_kv (layer-independent, g/b folded)
        w0 = None
        with tc.tile_pool(name="xkv", bufs=4) as xkv_pool:
            for g4 in range(KC // 2):
                if g4 == 2:
                    w0 = alloc_w_crit(0)
                if g4 == 5:
                    w0 = alloc_w_rest(w0, 0)
                xkv_t = xkv_pool.tile([P, 2, E], F32, name="xkv_t", tag="xkv_t")
                nc.sync.dma_start(
                    xkv_t[:],
                    xkv_d[g4 * 2 * P:(g4 + 1) * 2 * P, :].rearrange(
                        "(i p) c -> p i c", p=P
                    ),
                )
                pt = pp_pool.tile([P, E], F32, name="pp", tag="pp")
                ptb = pt[:].bitcast(BF16)
                for sub in range(2):
                    bnag, rstd = ln_rstd(xkv_t[:, sub, :])
                    hkv_t = xkv_pool.tile([P, E], BF16, name="hkv_t", tag="hkv_t")
                    nc.gpsimd.tensor_scalar(
                        hkv_t[:], xkv_t[:, sub, :], bnag[:, 0:1], rstd[:],
                        op0=ALU.subtract, op1=ALU.mult,
                    )
                    for e in range(EC):
                        col = (e * 2 + sub) * P
                        nc.tensor.matmul(
                            ptb[:, col:col + P], hkv_t[:, e * P:(e + 1) * P], ident[:],
                            is_transpose=True,
                            start=(sub == 0 and e == 0), stop=(sub == 1 and e == EC - 1),
                            skip_group_check=True,
                        )
                for s in range(SE):
                    in_ap = ptb[:, s * 512:(s + 1) * 512].rearrange(
                        "p (j u c) -> p j u c", j=2, u=2
                    )
                    out_ap = hkvT[s][:, :, g4 * 256:(g4 + 1) * 256].rearrange(
                        "p j (u c) -> p j u c", u=2
                    )
                    if (s + g4) % 2:
                        nc.scalar.copy(out_ap, in_ap)
                    else:
                        nc.vector.tensor_copy(out_ap, in_ap)

        # Work pools.
        work = ctx.enter_context(tc.tile_pool(name="work", bufs=1))
        big = ctx.enter_context(tc.tile_pool(name="big", bufs=1))
        ex_pool = ctx.enter_context(tc.tile_pool(name="ex", bufs=5))

        # va pair tiles persist across layers (v columns overwritten per
        # layer, the constant denominator columns are set once here).
        va = [
            big.tile([P, 2, H, 2 * DH], FP8, name=f"va{g}", tag=f"va{g}")
            for g in range(GK)
        ]
        for g in range(GK):
            nc.gpsimd.memset(va[g][:, :, :, DH:2 * DH], 1.0 / QKS)

        w_t = [w0, None]

        for l in range(L):
            wt = w_t[l]

            # ---- LN1(x_q) -> hqT fp8 pair tiles [128, 2, NQ] ----
            hqT = [
                work.tile([P, 2, NQ], FP8, name=f"hqT{s}", tag=f"actT{s}")
                for s in range(SE)
            ]
            for qp in range(QC // 2):
                ln_transpose_pair(xq[2 * qp], xq[2 * qp + 1], hqT, qp * 256, qp)

            # ---- q^T = wq^T @ hq^T: DR fp8, out scaled x8 + bias ----
            qT = [
                work.tile([P, NQ], BF16, name=f"qT{m}", tag=f"qT{m}")
                for m in range(EC)
            ]
            for m in range(EC):
                ps = pp_pool.tile([P, E], F32, name="pp", tag="pp")
                for s in range(SE):
                    nc.tensor.matmul(
                        ps[:], w_slice(wt["wq8"], s, m * P, (m + 1) * P),
                        hqT[s][:], start=(s == 0), stop=(s == SE - 1), perf_mode=DRM,
                    )
                nc.vector.tensor_scalar(
                    qT[m][:], ps[:], 1.0 / QKS, wt["bq"][:, m:m + 1],
                    op0=ALU.mult, op1=ALU.add,
                )

            # ---- k^T: DR fp8, out scaled x8, bias dropped ----
            kT = [
                big.tile([P, NKV], BF16, name=f"kT{m}", tag=f"kT{m}", bufs=2)
                for m in range(EC)
            ]
            for m in range(EC):
                for n in range(KN):
                    ps = pp_pool.tile([P, E], F32, name="pp", tag="pp")
                    for s in range(SE):
                        nc.tensor.matmul(
                            ps[:], w_slice(wt["wk8"], s, m * P, (m + 1) * P),
                            hkvT[s][:, :, n * 512:(n + 1) * 512],
                            start=(s == 0), stop=(s == SE - 1), perf_mode=DRM,
                        )
                    if (m + n) % 2:
                        nc.scalar.mul(kT[m][:, n * 512:(n + 1) * 512], ps[:], 1.0 / QKS)
                    else:
                        nc.vector.tensor_scalar_mul(
                            kT[m][:, n * 512:(n + 1) * 512], ps[:], 1.0 / QKS
                        )

            # ---- v: DR fp8 -> va pair tiles [128, 2, H, 128] (v x8 in cols
            #      0..63, constant 1/8 in cols 64..127 for the denominator) ----
            for m in range(KC):
                ps = pp_pool.tile([P, E], F32, name="pp", tag="pp")
                for s in range(SE):
                    nc.tensor.matmul(
                        ps[:], hkvT[s][:, :, m * P:(m + 1) * P],
                        w_slice(wt["wv8"], s, 0, E),
                        start=(s == 0), stop=(s == SE - 1), perf_mode=DRM,
                    )
                dst = va[m // 2][:, m % 2, :, 0:DH]
                src = ps[:].rearrange("p (h d) -> p h d", h=H)
                if m % 2:
                    nc.scalar.mul(dst, src, 1.0 / QKS)
                else:
                    nc.vector.tensor_scalar_mul(dst, src, 1.0 / QKS)

            if l == 0:
                w1c = alloc_w_crit(1)
                w_t[1] = alloc_w_rest(w1c, 1)

            # ---- attention, head by head ----
            aoT = [
                work.tile([P, 2, NQ], FP8, name=f"aoT{s}", tag=f"aoT{s}")
                for s in range(SE)
            ]
            for h in range(H):
                fh, r0 = h // 2, (h % 2) * DH
                s_ao, j_ao = fh // 2, fh % 2
                # attn@V accumulator: rows 0..63 = unnormalized attnout x8
                # (transposed), rows 64..127 = sum(ex)/8 per query.
                ps_oT = att_pool.tile([P, E], F32, name="ps_oT", tag="att")
                exs = []
                for g in range(GK):
                    ps_s = ss_pool.tile([P, 2, NQ], F32, name="ps_s", tag="ss")
                    for sub in range(2):
                        m = 2 * g + sub
                        nc.tensor.matmul(
                            ps_s[:, sub, :],
                            kT[fh][r0:r0 + DH, m * P:(m + 1) * P],
                            qT[fh][r0:r0 + DH, :],
                            start=True, stop=True,
                        )
                    ex = ex_pool.tile([P, 2, NQ], FP8, name="ex", tag="ex")
                    if EXP_PAT[g] == "A":
                        nc.scalar.activation(ex[:], ps_s[:], AF.Exp, scale=EXPS)
                    else:
                        nc.vector.tensor_scalar(
                            ex[:].bitcast(I8), ps_s[:], SCH_A, SCH_B,
                            op0=ALU.mult, op1=ALU.add,
                        )
                    exs.append(ex)
                    if g >= 1:
                        nc.tensor.matmul(
                            ps_oT[:], va[g - 1][:, :, h, :], exs[g - 1][:],
                            start=(g == 1), stop=False, perf_mode=DRM,
                        )
                nc.tensor.matmul(
                    ps_oT[:], va[GK - 1][:, :, h, :], exs[GK - 1][:],
                    start=False, stop=True, perf_mode=DRM,
                )
                # normalize: aoT rows = (x8 unnorm) * rcp(sum/8) = x64 attnout
                rcp = stats_pool.tile([DH, NQ], BF16, name="rcp", bufs=2)
                with nc.allow_low_precision(reason="bf16 softmax denominator"):
                    nc.vector.reciprocal(rcp[:], ps_oT[DH:P, :])
                nc.vector.tensor_tensor(
                    aoT[s_ao][r0:r0 + DH, j_ao, :], ps_oT[0:DH, :], rcp[:],
                    op=ALU.mult,
                )

            # ---- out-proj (DR fp8) + bo row + residual ----
            for qc in range(QC):
                ps = pp_pool.tile([P, E], F32, name="pp", tag="pp")
                for s in range(SE):
                    nc.tensor.matmul(
                        ps[:], aoT[s][:, :, qc * P:(qc + 1) * P],
                        w_slice(wt["wo8"], s, 0, E),
                        start=(s == 0), stop=False, perf_mode=DRM,
                    )
                nc.tensor.matmul(
                    ps[:], ones1[:], wt["bo_row"][:], start=False, stop=True,
                    skip_group_check=True,
                )
                nc.vector.scalar_tensor_tensor(
                    xq[qc], ps[:], 1.0 / (AOS * WS), xq[qc],
                    op0=ALU.mult, op1=ALU.add,
                )

            # ---- LN2 -> h2T ----
            h2T = [
                work.tile([P, 2, NQ], FP8, name=f"h2T{s}", tag=f"actT{s}")
                for s in range(SE)
            ]
            h2rT = [
                work.tile([P, 2, NQ], FP8, name=f"h2rT{s}", tag=f"h2rT{s}")
                for s in range(SE)
            ]
            for qp in range(QC // 2):
                ln_transpose_pair(
                    xq[2 * qp], xq[2 * qp + 1], h2T, qp * 256, qp + 1, resT=h2rT
                )

            # ---- FFN1 (DR fp8): g^T = gelu(w1^T @ h2^T / 64 + b1) ----
            gT = [
                big.tile([P, 2, NQ], FP8, name=f"gT{s}", tag=f"gT{s}")
                for s in range(SM)
            ]
            for m in range(MC):
                ps = pp_pool.tile([P, E], F32, name="pp", tag="pp")
                for s in range(SE):
                    nc.tensor.matmul(
                        ps[:], w_slice(wt["w18"], s, m * P, (m + 1) * P),
                        h2T[s][:], start=(s == 0), stop=False, perf_mode=DRM,
                    )
                for s in range(SE):
                    nc.tensor.matmul(
                        ps[:], w_slice(wt["w1r8"], s, m * P, (m + 1) * P),
                        h2T[s][:], start=False, stop=False, perf_mode=DRM,
                    )
                for s in range(SE):
                    nc.tensor.matmul(
                        ps[:], w_slice(wt["w18"], s, m * P, (m + 1) * P),
                        h2rT[s][:], start=False, stop=(s == SE - 1), perf_mode=DRM,
                    )
                nc.scalar.activation(
                    gT[m // 2][:, m % 2, :], ps[:], AF.Gelu,
                    bias=wt["b1"][:, m:m + 1], scale=1.0 / WS,
                )

            # ---- FFN2 (DR fp8) + b2 row + residual ----
            for qc in range(QC):
                ps = pp_pool.tile([P, E], F32, name="pp", tag="pp")
                for s in range(SM):
                    nc.tensor.matmul(
                        ps[:], gT[s][:, :, qc * P:(qc + 1) * P],
                        w_slice(wt["w28"], s, 0, E, SM),
                        start=(s == 0), stop=False, perf_mode=DRM,
                    )
                for s in range(SM):
                    nc.tensor.matmul(
                        ps[:], gT[s][:, :, qc * P:(qc + 1) * P],
                        w_slice(wt["w2r8"], s, 0, E, SM),
                        start=False, stop=False, perf_mode=DRM,
                    )
                nc.tensor.matmul(
                    ps[:], ones1[:], wt["b2_row"][:], start=False, stop=True,
                    skip_group_check=True,
                )
                nc.vector.scalar_tensor_tensor(
                    xq[qc], ps[:], 1.0 / WS, xq[qc], op0=ALU.mult, op1=ALU.add,
                )

        for qc in range(QC):
            nc.sync.dma_start(y_d[qc * P:(qc + 1) * P, :], xq[qc])

    nc.compile()
    return nc


def get_nc():
    if "nc" not in _CACHE:
        _CACHE["nc"] = _build()
    return _CACHE["nc"]


def _fp8(a):
    return np.clip(np.asarray(a, np.float32), -240.0, 240.0).astype(
        ml_dtypes.float8_e4m3
    )


def _bf16(a):
    return np.asarray(a, np.float32).astype(ml_dtypes.bfloat16)


def _rearr_dr(w8, S):
    """[S*2*128, C] (row-major contraction) -> [128, S*2*C] DR pair layout."""
    C = w8.shape[1]
    return np.ascontiguousarray(
        w8.reshape(S, 2, P, C).transpose(2, 0, 1, 3).reshape(P, S * 2 * C)
    )


def _cols(v):
    """[k*128] -> [128, k]: column m holds v[m*128:(m+1)*128]."""
    k = v.shape[0] // P
    return np.ascontiguousarray(np.asarray(v, np.float32).reshape(k, P).T)


def kernel(**inputs) -> np.ndarray:
    x_q = np.asarray(inputs["x_q"], np.float32)
    x_kv = np.asarray(inputs["x_kv"], np.float32)
    wq = np.asarray(inputs["wq"], np.float32)
    wkv = np.asarray(inputs["wkv"], np.float32)
    wo = np.asarray(inputs["wo"], np.float32)
    bo = np.asarray(inputs["bo"], np.float32)
    w1 = np.asarray(inputs["w1"], np.float32)
    b1 = np.asarray(inputs["b1"], np.float32)
    w2 = np.asarray(inputs["w2"], np.float32)
    b2 = np.asarray(inputs["b2"], np.float32)
    ln1_g = np.asarray(inputs["ln1_g"], np.float32)
    ln1_b = np.asarray(inputs["ln1_b"], np.float32)
    ln2_g = np.asarray(inputs["ln2_g"], np.float32)
    ln2_b = np.asarray(inputs["ln2_b"], np.float32)

    shared = {}
    for l in range(L):
        wk_f = wkv[l][:, :E]
        wv_f = wkv[l][:, E:]
        wq_eff = ln1_g[l][:, None] * wq[l]
        wk_eff = ln1_g[l][:, None] * wk_f
        wv_eff = ln1_g[l][:, None] * wv_f
        bq_eff = ln1_b[l] @ wq[l]
        bv_eff = ln1_b[l] @ wv_f
        bo_eff = bo[l] + bv_eff @ wo[l]
        w1_eff = ln2_g[l][:, None] * w1[l]
        b1_eff = ln2_b[l] @ w1[l] + b1[l]
        shared.update({
            f"wq8_{l}": _rearr_dr(_fp8(WS * wq_eff), SE),
            f"wk8_{l}": _rearr_dr(_fp8(WS * wk_eff), SE),
            f"wv8_{l}": _rearr_dr(_fp8(WS * wv_eff), SE),
            f"wo8_{l}": _rearr_dr(_fp8(WS * wo[l]), SE),
            f"w18_{l}": _rearr_dr(_fp8(WS * w1_eff), SE),
            f"w28_{l}": _rearr_dr(_fp8(WS * w2[l]), SM),
            f"w1r8_{l}": _rearr_dr(_fp8(
                WS * w1_eff - _fp8(WS * w1_eff).astype(np.float32)), SE),
            f"w2r8_{l}": _rearr_dr(_fp8(
                WS * w2[l] - _fp8(WS * w2[l]).astype(np.float32)), SM),
            f"bq_{l}": _cols(QKS * bq_eff),
            f"b1_{l}": _cols(b1_eff),
            f"bo_row_{l}": _bf16(AOS * WS * bo_eff)[None, :],
            f"b2_row_{l}": _bf16(WS * b2[l])[None, :],
        })

    in_maps = []
    for c in range(8):
        b, qc = c // 4, c % 4
        m = dict(shared)
        m["xq"] = np.ascontiguousarray(x_q[b, qc * NQ:(qc + 1) * NQ, :])
        m["xkv"] = np.ascontiguousarray(x_kv[b])
        in_maps.append(m)

    nc = get_nc()
    res = bass_utils.run_bass_kernel_spmd(nc, in_maps, core_ids=list(range(8)))

    out = np.empty((2, 2048, E), np.float32)
    for c in range(8):
        b, qc = c // 4, c % 4
        out[b, qc * NQ:(qc + 1) * NQ, :] = res.results[c]["y"]
    return out



# revision 62
# speedup vs baseline: 1.0864x; 1.0864x over previous
"""Trainium2 Bass kernel for a 2-layer cross-encoder (CrossEncoder).

Model: B=2, NQ=NKV=2048, E=512, H=8 (d_head=64), MLP=2048, depth=2, fp32 I/O.

Sharding (8 cores, no collectives): core c handles batch b=c//4 and query
rows [qc*512, (qc+1)*512) with qc=c%4.  Each core computes the full KV
projections for its batch so every core produces its output slice
independently.

Key structure (v4):
 - All heavy matmuls fp8e4m3 + DoubleRow (0.5 cyc/row), including the
   scores matmul (q/k stored as fp8 DR pair tiles, produced by a cheap
   SBUF->SBUF DMA partition-rearrange; head h sits at base partition
   0/64 of its E-chunk pair tile so the PE base-partition rule holds).
 - Attention runs head-PAIRS interleaved: the two heads' exp chains use
   opposite engines per group, so Act and DVE both stay fed and the
   scores->exp->attnV sem latency is hidden.
 - LayerNorm activations transposed by the DMA XBAR (dma_start_transpose)
   instead of PE identity matmuls + PSUM round trips; fp8 quantize runs
   from SBUF (2x/4x DVE modes).  rstd = 1/sqrt(var+eps) via bit trick +
   1 Newton step on Pool (batched for the kv setup).
 - exp() is Schraudolph-only (int8 round writes fp8e4m3 bits directly),
   split Act (Identity activation) / DVE; Act's only table is Gelu.
 - Softmax denominator comes free from 64 constant columns per head in V.
 - x_kv is uploaded bf16; all weights of a layer arrive as 3 packed DMAs
   staged so bulk transfers never sit in front of latency-critical XBAR
   or pair-rearrange DMAs in the shared DMA queue.
 - K/V of layer l+1 (which depend only on hkvT) are hoisted between
   FFN1(l) and FFN2(l) to fill idle engines there; LN2 interleaves with
   the O-proj residuals, next-layer LN1 with the FFN2 residuals.
"""

import numpy as np
import ml_dtypes

import concourse.bass as bass
import concourse.bacc as bacc
import concourse.mybir as mybir
import concourse.tile as tile
from concourse import bass_utils
from contextlib import ExitStack

P = 128
E = 512
EC = E // P        # 4 chunks of the embedding dim
SE = EC // 2       # 2 DoubleRow super-chunks
NQ = 512           # query rows per core
QC = NQ // P       # 4 query chunks
NKV = 2048
KC = NKV // P      # 16 key chunks of 128
KN = NKV // 512    # 4 key chunks of 512
GK = KC // 2       # 8 key pair-groups
H = 8
DH = 64
MLP = 2048
MC = MLP // P      # 16 mlp chunks of 128
SM = MC // 2       # 8 DoubleRow super-chunks
L = 2
LN_EPS = 1e-5
F32 = mybir.dt.float32
I32 = mybir.dt.int32
BF16 = mybir.dt.bfloat16
FP8 = mybir.dt.float8e4
I8 = mybir.dt.int8
AF = mybir.ActivationFunctionType
ALU = mybir.AluOpType
DRM = mybir.MatmulPerfMode.DoubleRow

WS = 64.0                       # fp8 weight pre-scale (host side)
QKS = 8.0                       # q/k storage scale
SCALE = DH ** -0.5
EXPS = SCALE / (QKS * QKS)      # exp scale applied to scores psum (=1/512)
AOS = 64.0                      # attnout storage scale (fp8 subnormal guard)
SCH_A = (8.0 / np.log(2.0)) * EXPS   # Schraudolph slope for fp8e4 bits
SCH_B = 56.0 - 0.47                  # fp8e4 exponent bias term - rms shift
RSQRT_MAGIC = 0x5F3759DF
# per-(head-in-pair, group) exp engine: A=Act(Identity act) D=DVE.
# Anti-aligned so the two heads of a pair use opposite engines; 9A/7D
# because DVE also owns the softmax divide.
EXP_PAT = ["AADADADA", "DDADADAA"]

# packed-weight byte offsets (per partition)
C_WQ, C_WK, C_WV, C_BQ, C_END = 0, 2048, 4096, 6144, 6160
A_WO, A_W1, A_W1R, A_END = 0, 2048, 10240, 18432
B_W2, B_W2R, B_B1, B_END = 0, 8192, 16384, 16448

_CACHE = {}


def _build():
    """Build the per-core Bass program (identical on all 8 cores)."""
    nc = bacc.Bacc("TRN2", target_bir_lowering=False, debug=False, num_devices=8)

    xq_d = nc.dram_tensor("xq", [NQ, E], F32, kind="ExternalInput").ap()
    xkv_d = nc.dram_tensor("xkv", [NKV, E], BF16, kind="ExternalInput").ap()
    wd = []
    for l in range(L):
        wd.append({
            "crit": nc.dram_tensor(f"crit_{l}", [P, C_END], FP8, kind="ExternalInput").ap(),
            "restA": nc.dram_tensor(f"restA_{l}", [P, A_END], FP8, kind="ExternalInput").ap(),
            "restB": nc.dram_tensor(f"restB_{l}", [P, B_END], FP8, kind="ExternalInput").ap(),
            "rows": nc.dram_tensor(f"rows_{l}", [1, 2, E], BF16, kind="ExternalInput").ap(),
        })
    y_d = nc.dram_tensor("y", [NQ, E], F32, kind="ExternalOutput").ap()

    with tile.TileContext(nc) as tc, ExitStack() as ctx:
        const_pool = ctx.enter_context(tc.tile_pool(name="const", bufs=1))
        ones1 = const_pool.tile([1, P], BF16)
        nc.gpsimd.memset(ones1[:], 1.0)
        schb_col = const_pool.tile([P, 1], F32)
        nc.gpsimd.memset(schb_col[:], SCH_B)

        stats_pool = ctx.enter_context(tc.tile_pool(name="stats", bufs=12))

        def rsqrt_chain(var_ap, rstd_ap, w):
            """rstd = 1/sqrt(var+eps) on Pool via bit trick + 1 Newton step."""
            ve = stats_pool.tile([P, 4], F32, name="ve")[:, :w]
            nc.gpsimd.tensor_scalar_add(ve, var_ap, LN_EPS)
            y0 = stats_pool.tile([P, 4], F32, name="y0")[:, :w]
            # int bit-trick ops run on DVE (Pool's Q7 rejects int shifts)
            nc.vector.tensor_scalar(
                y0.bitcast(I32), ve.bitcast(I32), 1, 0,
                op0=ALU.logical_shift_right, op1=ALU.bypass,
            )
            nc.vector.tensor_scalar(
                y0.bitcast(I32), y0.bitcast(I32), -1, RSQRT_MAGIC,
                op0=ALU.mult, op1=ALU.add,
            )
            t = stats_pool.tile([P, 4], F32, name="t")[:, :w]
            nc.gpsimd.tensor_tensor(t, y0, y0, op=ALU.mult)
            nc.gpsimd.tensor_tensor(t, t, ve, op=ALU.mult)
            nc.gpsimd.tensor_scalar(t, t, -0.5, 1.5, op0=ALU.mult, op1=ALU.add)
            nc.gpsimd.tensor_tensor(rstd_ap, y0, t, op=ALU.mult)

        def ln_stats(x_ap, mv_ap):
            bnst = stats_pool.tile([P, 6], F32, name="bnst")
            nc.vector.bn_stats(bnst[:], x_ap)
            nc.vector.bn_aggr(mv_ap, bnst[:])

        # norm for one [128, E] tile into slot u of a 4-wide hn4 buffer.
        def ln_norm(src_ap, mean_ap, rstd_ap, hn4, u, eng="D"):
            if eng == "P":
                nc.gpsimd.tensor_scalar(
                    hn4[:, u, :], src_ap, mean_ap, rstd_ap,
                    op0=ALU.subtract, op1=ALU.mult,
                )
            else:
                nc.vector.tensor_scalar(
                    hn4[:, u, :], src_ap, mean_ap, rstd_ap,
                    op0=ALU.subtract, op1=ALU.mult,
                )

        # one XBAR transpose + one quantize for an nu-tile hn batch
        # (nu*128 tokens).  dst slice [:, :, tok0:tok0+nu*128].
        def ln_txn(hn, t_pool, dstT8, tok0, resT8=None, eng="D", nu=4):
            ht = t_pool.tile([P, 4 * EC, P], BF16, name="ht", tag="ht",
                             bufs=2)[:, :nu * EC, :]
            nc.sync.dma_start_transpose(
                ht, hn[:].rearrange("p u c -> p (u c)")
            )
            dst = dstT8[:, :, tok0:tok0 + nu * P].rearrange(
                "p c (u t) -> p u c t", u=nu
            )
            src = ht.rearrange("p (u c) t -> p u c t", u=nu)
            if eng == "P":
                nc.gpsimd.tensor_copy(dst, src)
            elif eng == "D":
                nc.vector.tensor_copy(dst, src)
            else:
                nc.scalar.copy(dst, src)
            if resT8 is not None:
                # STT requires <=3D APs: one op per 128-token sub-block.
                for u in range(nu):
                    t0 = tok0 + u * P
                    nc.vector.scalar_tensor_tensor(
                        resT8[:, :, t0:t0 + P], dstT8[:, :, t0:t0 + P], -1.0,
                        ht[:, u * EC:(u + 1) * EC, :], op0=ALU.mult, op1=ALU.add,
                    )

        # Residual stream: one [128, 4, 512] fp32 tile (qc-major subtiles).
        xq_pool = ctx.enter_context(tc.tile_pool(name="xq", bufs=1))
        xqb = xq_pool.tile([P, QC, E], F32, name="xqb", tag="xqb")
        nc.sync.dma_start(xqb[:], xq_d.rearrange("(a p) c -> p a c", p=P))
        xq = [xqb[:, qc, :] for qc in range(QC)]

        # hkv^T fp8 DR tile [128, 4, NKV]: (c, p) holds E-row c*128+p.
        hkvT_pool = ctx.enter_context(tc.tile_pool(name="hkvT", bufs=1))
        hkvT8 = hkvT_pool.tile([P, EC, NKV], FP8, name="hkvT8", tag="hkvT8")

        # PSUM pools (8 banks): shared 1-bank ring 6 + att 2 = 8.
        ss_pool = ctx.enter_context(tc.tile_pool(name="ss", bufs=6, space="PSUM"))
        att_pool = ctx.enter_context(tc.tile_pool(name="attp", bufs=2, space="PSUM"))

        wpool = ctx.enter_context(tc.tile_pool(name="w", bufs=1))

        def load_pack(l, which, sz):
            t = wpool.tile([P, sz], FP8, name=f"{which}_{l}")
            nc.sync.dma_start(t[:], wd[l][which])
            return t

        def load_rows(l):
            t = wpool.tile([1, 2, E], BF16, name=f"rows_{l}")
            nc.sync.dma_start(t[:], wd[l]["rows"])
            return t

        def dr(ap, S):
            return ap.rearrange("p (s j c) -> p s j c", s=S, j=2)

        work = ctx.enter_context(tc.tile_pool(name="work", bufs=1))
        big = ctx.enter_context(tc.tile_pool(name="big", bufs=1))
        ex_pool = ctx.enter_context(tc.tile_pool(name="ex", bufs=6))
        lnp = ctx.enter_context(tc.tile_pool(name="lnp", bufs=4))

        def ln_hn2():
            return lnp.tile([P, 2, E], BF16, name="hn2", tag="hn2", bufs=4)

        # q-side LN: per-tile stats+norm into hn2 slot qc%2; a following
        # ln_txn(nu=2) finishes each half.
        def ln_q_tile(qc, hn2, eng="D"):
            mv = stats_pool.tile([P, 2], F32, name="mv")
            ln_stats(xq[qc], mv[:])
            rstd = stats_pool.tile([P, 1], F32, name="rstd")
            rsqrt_chain(mv[:, 1:2], rstd[:], 1)
            ln_norm(xq[qc], mv[:, 0:1], rstd[:], hn2, qc % 2, eng=eng)

        crit = [None, None]
        with tc.tile_pool(name="kvln", bufs=4) as kvln_pool:
            # x_kv bf16, 4 chunks of [128, 4, 512] (token-block subtiles),
            # double-buffered: chunk a+2 reuses chunk a's space.
            xkvb = []
            for a in range(KN):
                t = kvln_pool.tile([P, 4, E], BF16, name=f"xkvb{a}",
                                   tag="xkvb", bufs=2)
                nc.sync.dma_start(
                    t[:], xkv_d[a * 512:(a + 1) * 512, :].rearrange(
                        "(u p) c -> p u c", p=P)
                )
                xkvb.append(t)

            # ---- LN1(x_q, layer 0), two halves ----
            hqT8_l0 = work.tile([P, EC, NQ], FP8, name="hqT8_l0", tag="actT")
            for half in range(2):
                hn2 = ln_hn2()
                for qc in (2 * half, 2 * half + 1):
                    ln_q_tile(qc, hn2, eng="D")
                ln_txn(hn2, lnp, hqT8_l0, half * 256, eng="D", nu=2)

            crit[0] = load_pack(0, "crit", C_END)

            # ---- startup: x_kv LN + XBAR transpose + fp8 quantize.
            #      Stats run one batch ahead of the norm/xbar chains so
            #      DVE's queue never blocks the next batch's stats. ----
            mv4s, rstd4s = {}, {}

            def kv_stats(b):
                mv4 = stats_pool.tile([P, 2, 4], F32, name="mv4", bufs=4)
                for u in range(4):
                    ln_stats(xkvb[b][:, u, :], mv4[:, :, u])
                rstd4 = stats_pool.tile([P, 4], F32, name="rstd4", bufs=4)
                rsqrt_chain(mv4[:, 1, :], rstd4[:], 4)
                mv4s[b], rstd4s[b] = mv4, rstd4

            def kv_finish(b):
                khn4 = kvln_pool.tile([P, 4, E], BF16, name="khn4",
                                      tag="khn4", bufs=2)
                for u in range(4):
                    ln_norm(xkvb[b][:, u, :], mv4s[b][:, 0, u:u + 1],
                            rstd4s[b][:, u:u + 1], khn4, u, eng="DDAD"[u])
                ln_txn(khn4, lnp, hkvT8, b * 512, eng="DPPP"[b], nu=4)

            kv_stats(0)
            kv_stats(1)
            kv_finish(0)
            kv_stats(2)
            kv_finish(1)
            kv_stats(3)
            kv_finish(2)
            kv_finish(3)

        restA = [None, None]
        restB = [None, None]
        rows = [None, None]

        # va pair tiles persist across layers (v columns overwritten per
        # layer, the constant denominator columns are set once here).
        va = [
            big.tile([P, 2, H, 2 * DH], FP8, name=f"va{g}", tag=f"va{g}")
            for g in range(GK)
        ]
        for g in range(GK):
            nc.gpsimd.memset(va[g][:, :, :, DH:2 * DH], 1.0 / QKS)

        def proj_q(l, hqT8_ap):
            """Q projection + pair-rearrange for layer l."""
            wq_s = dr(crit[l][:, C_WQ:C_WK], SE)
            bq = crit[l][:, C_BQ:C_END].bitcast(F32)
            q8f = work.tile([P, EC, NQ], FP8, name="q8f", tag="q8f")
            q8p = work.tile([P, EC, 2, NQ], FP8, name="q8p", tag="q8p")
            for m in range(EC):
                ps = ss_pool.tile([P, E], F32, name="pp", tag="ss")
                for half in range(2):
                    c0, c1 = half * 256, half * 256 + 256
                    for s in range(SE):
                        nc.tensor.matmul(
                            ps[:, c0:c1], wq_s[:, s, :, m * P:(m + 1) * P],
                            hqT8_ap[:, 2 * s:2 * s + 2, c0:c1],
                            start=(s == 0), stop=(s == SE - 1), perf_mode=DRM,
                            skip_group_check=True,
                        )
                if m % 2:
                    nc.scalar.activation(
                        q8f[:, m, :], ps[:], AF.Identity,
                        bias=bq[:, m:m + 1], scale=1.0 / QKS,
                    )
                else:
                    nc.vector.tensor_scalar(
                        q8f[:, m, :], ps[:], 1.0 / QKS, bq[:, m:m + 1],
                        op0=ALU.mult, op1=ALU.add,
                    )
            # pair layout: head h=2m+half at partitions [64*half, 64*half+32),
            # (p, j) <-> d-row 32*j + p.  4 plain partition-slice DMAs.
            for half in range(2):
                for jj in range(2):
                    r0 = 64 * half + 32 * jj
                    nc.sync.dma_start(
                        q8p[64 * half:64 * half + 32, :, jj, :],
                        q8f[r0:r0 + 32, :, :],
                    )
            return q8p

        def proj_kv(l):
            """K (with pair-rearrange) and V, interleaved per E-chunk."""
            wk_s = dr(crit[l][:, C_WK:C_WV], SE)
            wv_s = dr(crit[l][:, C_WV:C_BQ], SE)
            k8f = big.tile([P, EC, NKV], FP8, name="k8f", tag="k8f")
            k8p = big.tile([P, EC, 2, NKV], FP8, name="k8p", tag="k8p")
            for m in range(EC):
                for n in range(KN):
                    ps = ss_pool.tile([P, E], F32, name="pp", tag="ss")
                    for s in range(SE):
                        nc.tensor.matmul(
                            ps[:], wk_s[:, s, :, m * P:(m + 1) * P],
                            hkvT8[:, 2 * s:2 * s + 2, n * 512:(n + 1) * 512],
                            start=(s == 0), stop=(s == SE - 1), perf_mode=DRM,
                        )
                    if n % 2:
                        nc.scalar.mul(k8f[:, m, n * 512:(n + 1) * 512], ps[:], 1.0 / QKS)
                    else:
                        nc.vector.tensor_scalar_mul(
                            k8f[:, m, n * 512:(n + 1) * 512], ps[:], 1.0 / QKS
                        )
                for half in range(2):
                    for jj in range(2):
                        r0 = 64 * half + 32 * jj
                        nc.sync.dma_start(
                            k8p[64 * half:64 * half + 32, m, jj, :],
                            k8f[r0:r0 + 32, m, :],
                        )
                for mv in range(4 * m, 4 * m + 4):
                    ps = ss_pool.tile([P, E], F32, name="pp", tag="ss")
                    for s in range(SE):
                        nc.tensor.matmul(
                            ps[:], hkvT8[:, 2 * s:2 * s + 2, mv * P:(mv + 1) * P],
                            wv_s[:, s, :, :],
                            start=(s == 0), stop=(s == SE - 1), perf_mode=DRM,
                        )
                    dst = va[mv // 2][:, mv % 2, :, 0:DH]
                    src = ps[:].rearrange("p (h d) -> p h d", h=H)
                    if mv % 4 == 3:
                        nc.vector.tensor_scalar_mul(dst, src, 1.0 / QKS)
                    else:
                        nc.scalar.mul(dst, src, 1.0 / QKS)
            return k8p

        # ---- layer 0 Q/K/V ----
        q8p = proj_q(0, hqT8_l0[:])
        k8p = proj_kv(0)

        for l in range(L):
            # ---- attention, head-pairs (fp8 DR scores) ----
            aoT = work.tile([P, EC, NQ], FP8, name="aoT", tag="aoT")
            for m in range(EC):          # pair (h0, h1) = (2m, 2m+1)
                # bulk weight loads, issued mid-attention so their
                # transfers never block latency-critical DMAs.
                if l == 0 and m == 1:
                    restA[0] = load_pack(0, "restA", A_END)
                    rows[0] = load_rows(0)
                if l == 0 and m == 2:
                    crit[1] = load_pack(1, "crit", C_END)
                if l == 0 and m == 3:
                    restB[0] = load_pack(0, "restB", B_END)
                kst = [k8p[0:32, m, :, :], k8p[64:96, m, :, :]]
                qmv = [q8p[0:32, m, :, :], q8p[64:96, m, :, :]]
                pso = [
                    att_pool.tile([P, E], F32, name="ps_oT", tag="att")
                    for _ in range(2)
                ]
                exs = [[], []]
                for g in range(GK):
                    for j in range(2):
                        ex = ex_pool.tile([P, 2, NQ], FP8, name="ex", tag="ex")
                        for sub in range(2):
                            ps_s = ss_pool.tile([P, NQ], F32, name="ps_s", tag="ss")
                            c0 = (2 * g + sub) * P
                            nc.tensor.matmul(
                                ps_s[:], kst[j][:, :, c0:c0 + P], qmv[j],
                                start=True, stop=True, perf_mode=DRM,
                            )
                            if EXP_PAT[j][g] == "A":
                                nc.scalar.activation(
                                    ex[:, sub, :].bitcast(I8), ps_s[:],
                                    AF.Identity, bias=schb_col[:], scale=SCH_A,
                                )
                            else:
                                nc.vector.tensor_scalar(
                                    ex[:, sub, :].bitcast(I8), ps_s[:],
                                    SCH_A, SCH_B, op0=ALU.mult, op1=ALU.add,
                                )
                        exs[j].append(ex)
                    if g >= 1:
                        for j in range(2):
                            nc.tensor.matmul(
                                pso[j][:], va[g - 1][:, :, 2 * m + j, :],
                                exs[j][g - 1][:],
                                start=(g == 1), stop=False, perf_mode=DRM,
                            )
                for j in range(2):
                    nc.tensor.matmul(
                        pso[j][:], va[GK - 1][:, :, 2 * m + j, :],
                        exs[j][GK - 1][:],
                        start=False, stop=True, perf_mode=DRM,
                    )
                    # normalize: aoT = (x8 unnorm) * rcp(sum/8) = x64 attnout.
                    # Denominator staged to SBUF by Act; reciprocal runs in
                    # DVE's fast all-SBUF mode.
                    den = stats_pool.tile([DH, NQ], BF16, name="den", bufs=2)
                    nc.scalar.copy(den[:], pso[j][DH:P, :])
                    rcp = stats_pool.tile([DH, NQ], BF16, name="rcp", bufs=2)
                    with nc.allow_low_precision(reason="bf16 softmax denom"):
                        nc.vector.reciprocal(rcp[:], den[:])
                    nc.vector.tensor_tensor(
                        aoT[64 * j:64 * j + 64, m, :], pso[j][0:DH, :],
                        rcp[:], op=ALU.mult,
                    )

            # ---- out-proj (DR fp8) + bo row + residual; LN2 interleaved ----
            wo_s = dr(restA[l][:, A_WO:A_W1], SE)
            h2T8 = work.tile([P, EC, NQ], FP8, name="h2T8", tag="actT")
            h2r8 = work.tile([P, EC, NQ], FP8, name="h2r8", tag="h2r8")
            hn2 = ln_hn2()
            for qc in range(QC):
                ps = ss_pool.tile([P, E], F32, name="pp", tag="ss")
                nc.tensor.matmul(
                    ps[:], ones1[:], rows[l][:, 0, :], start=True, stop=False,
                    skip_group_check=True,
                )
                for s in range(SE):
                    nc.tensor.matmul(
                        ps[:], aoT[:, 2 * s:2 * s + 2, qc * P:(qc + 1) * P],
                        wo_s[:, s, :, :],
                        start=False, stop=(s == SE - 1), perf_mode=DRM,
                        skip_group_check=True,
                    )
                nc.vector.scalar_tensor_tensor(
                    xq[qc], ps[:], 1.0 / (AOS * WS), xq[qc],
                    op0=ALU.mult, op1=ALU.add,
                )
                ln_q_tile(qc, hn2, eng="D")
                if qc % 2 == 1:
                    ln_txn(hn2, lnp, h2T8, (qc - 1) * P, resT8=h2r8,
                           eng="D", nu=2)
                    if qc == 1:
                        hn2 = ln_hn2()
            if l + 1 < L:
                restA[1] = load_pack(1, "restA", A_END)
                rows[1] = load_rows(1)

            # ---- FFN1 (DR fp8): g^T = gelu(w1^T @ h2^T / 64 + b1) ----
            w1_s = dr(restA[l][:, A_W1:A_W1R], SE)
            w1r_s = dr(restA[l][:, A_W1R:A_END], SE)
            b1 = restB[l][:, B_B1:B_END].bitcast(F32)
            gT8 = big.tile([P, MC, NQ], FP8, name="gT8", tag="gT8")
            for m in range(MC):
                ps = ss_pool.tile([P, E], F32, name="pp", tag="ss")
                for half in range(2):
                    c0, c1 = half * 256, half * 256 + 256
                    for s in range(SE):
                        nc.tensor.matmul(
                            ps[:, c0:c1], w1_s[:, s, :, m * P:(m + 1) * P],
                            h2T8[:, 2 * s:2 * s + 2, c0:c1],
                            start=(s == 0), stop=False, perf_mode=DRM,
                            skip_group_check=True,
                        )
                    for s in range(SE):
                        nc.tensor.matmul(
                            ps[:, c0:c1], w1r_s[:, s, :, m * P:(m + 1) * P],
                            h2T8[:, 2 * s:2 * s + 2, c0:c1],
                            start=False, stop=False, perf_mode=DRM,
                            skip_group_check=True,
                        )
                    for s in range(SE):
                        nc.tensor.matmul(
                            ps[:, c0:c1], w1_s[:, s, :, m * P:(m + 1) * P],
                            h2r8[:, 2 * s:2 * s + 2, c0:c1],
                            start=False, stop=(s == SE - 1), perf_mode=DRM,
                            skip_group_check=True,
                        )
                nc.scalar.activation(
                    gT8[:, m, :], ps[:], AF.Gelu,
                    bias=b1[:, m:m + 1], scale=1.0 / WS,
                )

            # ---- hoisted K/V of layer l+1 (depend only on hkvT) ----
            if l + 1 < L:
                k8p = proj_kv(l + 1)

            # ---- FFN2 (DR fp8) + b2 row + residual; next LN1 interleaved ----
            w2_s = dr(restB[l][:, B_W2:B_W2R], SM)
            w2r_s = dr(restB[l][:, B_W2R:B_B1], SM)
            if l + 1 < L:
                hqT8 = work.tile([P, EC, NQ], FP8, name="hqT8", tag="actT2")
                hn2 = ln_hn2()
            for qc in range(QC):
                ps = ss_pool.tile([P, E], F32, name="pp", tag="ss")
                nc.tensor.matmul(
                    ps[:], ones1[:], rows[l][:, 1, :], start=True, stop=False,
                    skip_group_check=True,
                )
                for s in range(SM):
                    nc.tensor.matmul(
                        ps[:], gT8[:, 2 * s:2 * s + 2, qc * P:(qc + 1) * P],
                        w2_s[:, s, :, :],
                        start=False, stop=False, perf_mode=DRM,
                        skip_group_check=True,
                    )
                for s in range(SM):
                    nc.tensor.matmul(
                        ps[:], gT8[:, 2 * s:2 * s + 2, qc * P:(qc + 1) * P],
                        w2r_s[:, s, :, :],
                        start=False, stop=(s == SM - 1), perf_mode=DRM,
                        skip_group_check=True,
                    )
                nc.vector.scalar_tensor_tensor(
                    xq[qc], ps[:], 1.0 / WS, xq[qc], op0=ALU.mult, op1=ALU.add,
                )
                if l + 1 < L:
                    ln_q_tile(qc, hn2, eng="D")
                    if qc % 2 == 1:
                        ln_txn(hn2, lnp, hqT8, (qc - 1) * P, eng="D", nu=2)
                        if qc == 1:
                            hn2 = ln_hn2()
                else:
                    nc.sync.dma_start(y_d[qc * P:(qc + 1) * P, :], xq[qc])
            if l + 1 < L:
                restB[1] = load_pack(1, "restB", B_END)
                q8p = proj_q(l + 1, hqT8[:])

    nc.compile()
    return nc


def get_nc():
    if "nc" not in _CACHE:
        _CACHE["nc"] = _build()
    return _CACHE["nc"]


def _fp8(a):
    return np.clip(np.asarray(a, np.float32), -240.0, 240.0).astype(
        ml_dtypes.float8_e4m3
    )


def _bf16(a):
    return np.asarray(a, np.float32).astype(ml_dtypes.bfloat16)


def _rearr_dr(w8, S):
    """[S*2*128, C] (row-major contraction) -> [128, S*2*C] DR pair layout."""
    C = w8.shape[1]
    return np.ascontiguousarray(
        w8.reshape(S, 2, P, C).transpose(2, 0, 1, 3).reshape(P, S * 2 * C)
    )


def _cols(v):
    """[k*128] -> [128, k]: column m holds v[m*128:(m+1)*128]."""
    k = v.shape[0] // P
    return np.ascontiguousarray(np.asarray(v, np.float32).reshape(k, P).T)


def _u8(a):
    return np.ascontiguousarray(a).view(np.uint8)


def kernel(**inputs) -> np.ndarray:
    x_q = np.asarray(inputs["x_q"], np.float32)
    x_kv = np.asarray(inputs["x_kv"], np.float32)
    wq = np.asarray(inputs["wq"], np.float32)
    wkv = np.asarray(inputs["wkv"], np.float32)
    wo = np.asarray(inputs["wo"], np.float32)
    bo = np.asarray(inputs["bo"], np.float32)
    w1 = np.asarray(inputs["w1"], np.float32)
    b1 = np.asarray(inputs["b1"], np.float32)
    w2 = np.asarray(inputs["w2"], np.float32)
    b2 = np.asarray(inputs["b2"], np.float32)
    ln1_g = np.asarray(inputs["ln1_g"], np.float32)
    ln1_b = np.asarray(inputs["ln1_b"], np.float32)
    ln2_g = np.asarray(inputs["ln2_g"], np.float32)
    ln2_b = np.asarray(inputs["ln2_b"], np.float32)

    shared = {}
    for l in range(L):
        wk_f = wkv[l][:, :E]
        wv_f = wkv[l][:, E:]
        wq_eff = ln1_g[l][:, None] * wq[l]
        wk_eff = ln1_g[l][:, None] * wk_f
        wv_eff = ln1_g[l][:, None] * wv_f
        bq_eff = ln1_b[l] @ wq[l]
        bv_eff = ln1_b[l] @ wv_f
        bo_eff = bo[l] + bv_eff @ wo[l]
        w1_eff = ln2_g[l][:, None] * w1[l]
        b1_eff = ln2_b[l] @ w1[l] + b1[l]

        wq8 = _rearr_dr(_fp8(WS * wq_eff), SE)
        wk8 = _rearr_dr(_fp8(WS * wk_eff), SE)
        wv8 = _rearr_dr(_fp8(WS * wv_eff), SE)
        wo8 = _rearr_dr(_fp8(WS * wo[l]), SE)
        w18 = _rearr_dr(_fp8(WS * w1_eff), SE)
        w28 = _rearr_dr(_fp8(WS * w2[l]), SM)
        w1r8 = _rearr_dr(_fp8(WS * w1_eff - _fp8(WS * w1_eff).astype(np.float32)), SE)
        w2r8 = _rearr_dr(_fp8(WS * w2[l] - _fp8(WS * w2[l]).astype(np.float32)), SM)
        bq_c = _cols(QKS * bq_eff)      # [128, 4] f32
        b1_c = _cols(b1_eff)            # [128, 16] f32

        crit = np.concatenate(
            [_u8(wq8), _u8(wk8), _u8(wv8), _u8(bq_c)], axis=1)
        restA = np.concatenate([_u8(wo8), _u8(w18), _u8(w1r8)], axis=1)
        restB = np.concatenate([_u8(w28), _u8(w2r8), _u8(b1_c)], axis=1)
        rows2 = np.stack(
            [_bf16(AOS * WS * bo_eff), _bf16(WS * b2[l])], axis=0)[None]
        shared.update({
            f"crit_{l}": crit.view(ml_dtypes.float8_e4m3),
            f"restA_{l}": restA.view(ml_dtypes.float8_e4m3),
            f"restB_{l}": restB.view(ml_dtypes.float8_e4m3),
            f"rows_{l}": rows2,
        })

    in_maps = []
    for c in range(8):
        b, qc = c // 4, c % 4
        m = dict(shared)
        m["xq"] = np.ascontiguousarray(x_q[b, qc * NQ:(qc + 1) * NQ, :])
        m["xkv"] = np.ascontiguousarray(_bf16(x_kv[b]))
        in_maps.append(m)

    nc = get_nc()
    res = bass_utils.run_bass_kernel_spmd(nc, in_maps, core_ids=list(range(8)))

    out = np.empty((2, 2048, E), np.float32)
    for c in range(8):
        b, qc = c // 4, c % 4
        out[b, qc * NQ:(qc + 1) * NQ, :] = res.results[c]["y"]
    return out


# revision 68
# speedup vs baseline: 1.1675x; 1.0746x over previous
"""Trainium2 Bass kernel for a 2-layer cross-encoder (CrossEncoder).

Model: B=2, NQ=NKV=2048, E=512, H=8 (d_head=64), MLP=2048, depth=2, fp32 I/O.

Sharding (8 cores, no collectives): core c handles batch b=c//4 and query
rows [qc*512, (qc+1)*512) with qc=c%4.  Each core computes the full KV
projections for its batch so every core produces its output slice
independently.

Key structure (v4):
 - All heavy matmuls fp8e4m3 + DoubleRow (0.5 cyc/row), including the
   scores matmul (q/k stored as fp8 DR pair tiles, produced by a cheap
   SBUF->SBUF DMA partition-rearrange; head h sits at base partition
   0/64 of its E-chunk pair tile so the PE base-partition rule holds).
 - Attention runs head-PAIRS interleaved: the two heads' exp chains use
   opposite engines per group, so Act and DVE both stay fed and the
   scores->exp->attnV sem latency is hidden.
 - LayerNorm activations transposed by the DMA XBAR (dma_start_transpose)
   instead of PE identity matmuls + PSUM round trips; fp8 quantize runs
   from SBUF (2x/4x DVE modes).  rstd = 1/sqrt(var+eps) via bit trick +
   1 Newton step on Pool (batched for the kv setup).
 - exp() is Schraudolph-only (int8 round writes fp8e4m3 bits directly),
   split Act (Identity activation) / DVE; Act's only table is Gelu.
 - Softmax denominator comes free from 64 constant columns per head in V.
 - x_kv is uploaded bf16; all weights of a layer arrive as 3 packed DMAs
   staged so bulk transfers never sit in front of latency-critical XBAR
   or pair-rearrange DMAs in the shared DMA queue.
 - K/V of layer l+1 (which depend only on hkvT) are hoisted between
   FFN1(l) and FFN2(l) to fill idle engines there; LN2 interleaves with
   the O-proj residuals, next-layer LN1 with the FFN2 residuals.
"""

import numpy as np
import ml_dtypes

import concourse.bass as bass
import concourse.bacc as bacc
import concourse.mybir as mybir
import concourse.tile as tile
from concourse import bass_utils
from contextlib import ExitStack

P = 128
E = 512
EC = E // P        # 4 chunks of the embedding dim
SE = EC // 2       # 2 DoubleRow super-chunks
NQ = 512           # query rows per core
QC = NQ // P       # 4 query chunks
NKV = 2048
KC = NKV // P      # 16 key chunks of 128
KN = NKV // 512    # 4 key chunks of 512
GK = KC // 2       # 8 key pair-groups
H = 8
DH = 64
MLP = 2048
MC = MLP // P      # 16 mlp chunks of 128
SM = MC // 2       # 8 DoubleRow super-chunks
L = 2
LN_EPS = 1e-5
F32 = mybir.dt.float32
I32 = mybir.dt.int32
BF16 = mybir.dt.bfloat16
FP8 = mybir.dt.float8e4
I8 = mybir.dt.int8
AF = mybir.ActivationFunctionType
ALU = mybir.AluOpType
DRM = mybir.MatmulPerfMode.DoubleRow

WS = 64.0                       # fp8 weight pre-scale (host side)
QKS = 8.0                       # q/k storage scale
SCALE = DH ** -0.5
EXPS = SCALE / (QKS * QKS)      # exp scale applied to scores psum (=1/512)
AOS = 64.0                      # attnout storage scale (fp8 subnormal guard)
SCH_A = (8.0 / np.log(2.0)) * EXPS   # Schraudolph slope for fp8e4 bits
SCH_B = 56.0 - 0.47                  # fp8e4 exponent bias term - rms shift
RSQRT_MAGIC = 0x5F3759DF
# per-(head-in-pair, group) exp engine: A=Act(Identity act) D=DVE.
# Anti-aligned so the two heads of a pair use opposite engines; 9A/7D
# because DVE also owns the softmax divide.
EXP_PAT = ["AADADADA", "ADADADAA"]
EXP_PAT_ODD = ["AADADADA", "DDADADAA"]   # 9A/7D for odd pairs (balance)
H2R = True                               # FFN1 activation-residual pass

# packed-weight byte offsets (per partition)
C_WQ, C_WK, C_WV, C_BQ, C_END = 0, 2048, 4096, 6144, 6160
A_WO, A_W1, A_W1R, A_END = 0, 2048, 10240, 18432
B_W2, B_W2R, B_B1, B_END = 0, 8192, 16384, 16448

_CACHE = {}


def _build():
    """Build the per-core Bass program (identical on all 8 cores)."""
    nc = bacc.Bacc("TRN2", target_bir_lowering=False, debug=False, num_devices=8)

    xq_d = nc.dram_tensor("xq", [NQ, E], F32, kind="ExternalInput").ap()
    xkv_d = nc.dram_tensor("xkv", [NKV, E], BF16, kind="ExternalInput").ap()
    wd = []
    for l in range(L):
        wd.append({
            "crit": nc.dram_tensor(f"crit_{l}", [P, C_END], FP8, kind="ExternalInput").ap(),
            "restA": nc.dram_tensor(f"restA_{l}", [P, A_END], FP8, kind="ExternalInput").ap(),
            "restB": nc.dram_tensor(f"restB_{l}", [P, B_END], FP8, kind="ExternalInput").ap(),
            "rows": nc.dram_tensor(f"rows_{l}", [1, 2, E], BF16, kind="ExternalInput").ap(),
        })
    y_d = nc.dram_tensor("y", [NQ, E], F32, kind="ExternalOutput").ap()

    with tile.TileContext(nc) as tc, ExitStack() as ctx:
        const_pool = ctx.enter_context(tc.tile_pool(name="const", bufs=1))
        ones1 = const_pool.tile([1, P], BF16)
        nc.gpsimd.memset(ones1[:], 1.0)
        schb_col = const_pool.tile([P, 1], F32)
        nc.gpsimd.memset(schb_col[:], SCH_B)

        stats_pool = ctx.enter_context(tc.tile_pool(name="stats", bufs=12))

        def rsqrt_chain(var_ap, rstd_ap, w):
            """rstd = 1/sqrt(var+eps) on Pool via bit trick + 1 Newton step."""
            ve = stats_pool.tile([P, 4], F32, name="ve")[:, :w]
            nc.gpsimd.tensor_scalar_add(ve, var_ap, LN_EPS)
            y0 = stats_pool.tile([P, 4], F32, name="y0")[:, :w]
            # int bit-trick ops run on DVE (Pool's Q7 rejects int shifts)
            nc.vector.tensor_scalar(
                y0.bitcast(I32), ve.bitcast(I32), 1, 0,
                op0=ALU.logical_shift_right, op1=ALU.bypass,
            )
            nc.vector.tensor_scalar(
                y0.bitcast(I32), y0.bitcast(I32), -1, RSQRT_MAGIC,
                op0=ALU.mult, op1=ALU.add,
            )
            t = stats_pool.tile([P, 4], F32, name="t")[:, :w]
            nc.gpsimd.tensor_tensor(t, y0, y0, op=ALU.mult)
            nc.gpsimd.tensor_tensor(t, t, ve, op=ALU.mult)
            nc.gpsimd.tensor_scalar(t, t, -0.5, 1.5, op0=ALU.mult, op1=ALU.add)
            nc.gpsimd.tensor_tensor(rstd_ap, y0, t, op=ALU.mult)

        def ln_stats(x_ap, mv_ap):
            bnst = stats_pool.tile([P, 6], F32, name="bnst")
            nc.vector.bn_stats(bnst[:], x_ap)
            nc.vector.bn_aggr(mv_ap, bnst[:])

        # norm for one [128, E] tile into slot u of a 4-wide hn4 buffer.
        def ln_norm(src_ap, mean_ap, rstd_ap, hn4, u, eng="D"):
            if eng == "P":
                nc.gpsimd.tensor_scalar(
                    hn4[:, u, :], src_ap, mean_ap, rstd_ap,
                    op0=ALU.subtract, op1=ALU.mult,
                )
            else:
                nc.vector.tensor_scalar(
                    hn4[:, u, :], src_ap, mean_ap, rstd_ap,
                    op0=ALU.subtract, op1=ALU.mult,
                )

        # one XBAR transpose + one quantize for an nu-tile hn batch
        # (nu*128 tokens).  dst slice [:, :, tok0:tok0+nu*128].
        def ln_txn(hn, t_pool, dstT8, tok0, resT8=None, eng="D", nu=4):
            ht = t_pool.tile([P, 4 * EC, P], BF16, name="ht", tag="ht",
                             bufs=2)[:, :nu * EC, :]
            nc.sync.dma_start_transpose(
                ht, hn[:].rearrange("p u c -> p (u c)")
            )
            dst = dstT8[:, :, tok0:tok0 + nu * P].rearrange(
                "p c (u t) -> p u c t", u=nu
            )
            src = ht.rearrange("p (u c) t -> p u c t", u=nu)
            if eng == "P":
                nc.gpsimd.tensor_copy(dst, src)
            elif eng == "D":
                nc.vector.tensor_copy(dst, src)
            else:
                nc.scalar.copy(dst, src)
            if resT8 is not None:
                # STT requires <=3D APs: one op per 128-token sub-block.
                for u in range(nu):
                    t0 = tok0 + u * P
                    nc.vector.scalar_tensor_tensor(
                        resT8[:, :, t0:t0 + P], dstT8[:, :, t0:t0 + P], -1.0,
                        ht[:, u * EC:(u + 1) * EC, :], op0=ALU.mult, op1=ALU.add,
                    )

        # Residual stream: one [128, 4, 512] fp32 tile (qc-major subtiles).
        xq_pool = ctx.enter_context(tc.tile_pool(name="xq", bufs=1))
        xqb = xq_pool.tile([P, QC, E], F32, name="xqb", tag="xqb")
        nc.sync.dma_start(xqb[:], xq_d.rearrange("(a p) c -> p a c", p=P))
        xq = [xqb[:, qc, :] for qc in range(QC)]

        # hkv^T fp8 DR tile [128, 4, NKV]: (c, p) holds E-row c*128+p.
        hkvT_pool = ctx.enter_context(tc.tile_pool(name="hkvT", bufs=1))
        hkvT8 = hkvT_pool.tile([P, EC, NKV], FP8, name="hkvT8", tag="hkvT8")

        # PSUM pools (8 banks): shared 1-bank ring 6 + att 2 = 8.
        ss_pool = ctx.enter_context(tc.tile_pool(name="ss", bufs=6, space="PSUM"))
        att_pool = ctx.enter_context(tc.tile_pool(name="attp", bufs=2, space="PSUM"))

        wpool = ctx.enter_context(tc.tile_pool(name="w", bufs=1))

        def load_pack(l, which, sz):
            t = wpool.tile([P, sz], FP8, name=f"{which}_{l}")
            nc.sync.dma_start(t[:], wd[l][which])
            return t

        def load_rows(l):
            t = wpool.tile([1, 2, E], BF16, name=f"rows_{l}")
            nc.sync.dma_start(t[:], wd[l]["rows"])
            return t

        def dr(ap, S):
            return ap.rearrange("p (s j c) -> p s j c", s=S, j=2)

        work = ctx.enter_context(tc.tile_pool(name="work", bufs=1))
        big = ctx.enter_context(tc.tile_pool(name="big", bufs=1))
        ex_pool = ctx.enter_context(tc.tile_pool(name="ex", bufs=5))
        lnp = ctx.enter_context(tc.tile_pool(name="lnp", bufs=4))

        def ln_hn2():
            return lnp.tile([P, 2, E], BF16, name="hn2", tag="hn2", bufs=3)

        # q-side LN: per-tile stats+norm into hn2 slot qc%2; a following
        # ln_txn(nu=2) finishes each half.
        def ln_q_tile(qc, hn2, eng="D"):
            mv = stats_pool.tile([P, 2], F32, name="mv")
            ln_stats(xq[qc], mv[:])
            rstd = stats_pool.tile([P, 1], F32, name="rstd")
            rsqrt_chain(mv[:, 1:2], rstd[:], 1)
            ln_norm(xq[qc], mv[:, 0:1], rstd[:], hn2, qc % 2, eng=eng)

        crit = [None, None]
        with tc.tile_pool(name="kvln", bufs=4) as kvln_pool:
            # x_kv bf16, 4 chunks of [128, 4, 512] (token-block subtiles),
            # ring of 3: chunk a+3 reuses chunk a's space.
            xkvb = []

            def load_xkvb(a):
                t = kvln_pool.tile([P, 4, E], BF16, name=f"xkvb{a}",
                                   tag="xkvb", bufs=3)
                nc.sync.dma_start(
                    t[:], xkv_d[a * 512:(a + 1) * 512, :].rearrange(
                        "(u p) c -> p u c", p=P)
                )
                xkvb.append(t)

            load_xkvb(0)
            load_xkvb(1)
            crit[0] = load_pack(0, "crit", C_END)
            load_xkvb(2)
            load_xkvb(3)

            # ---- LN1(x_q, layer 0), two halves ----
            hqT8_l0 = work.tile([P, EC, NQ], FP8, name="hqT8_l0", tag="actT")
            for half in range(2):
                hn2 = ln_hn2()
                for qc in (2 * half, 2 * half + 1):
                    ln_q_tile(qc, hn2, eng="D")
                ln_txn(hn2, lnp, hqT8_l0, half * 256, eng="D", nu=2)

            # ---- startup: x_kv LN + XBAR transpose + fp8 quantize.
            #      Stats run one batch ahead of the norm/xbar chains so
            #      DVE's queue never blocks the next batch's stats. ----
            mv4s, rstd4s = {}, {}

            def kv_stats(b):
                mv4 = stats_pool.tile([P, 2, 4], F32, name="mv4", bufs=4)
                for u in range(4):
                    ln_stats(xkvb[b][:, u, :], mv4[:, :, u])
                rstd4 = stats_pool.tile([P, 4], F32, name="rstd4", bufs=4)
                rsqrt_chain(mv4[:, 1, :], rstd4[:], 4)
                mv4s[b], rstd4s[b] = mv4, rstd4

            def kv_finish(b):
                khn4 = kvln_pool.tile([P, 4, E], BF16, name="khn4",
                                      tag="khn4", bufs=2)
                for u in range(4):
                    ln_norm(xkvb[b][:, u, :], mv4s[b][:, 0, u:u + 1],
                            rstd4s[b][:, u:u + 1], khn4, u, eng="DDAD"[u])
                ln_txn(khn4, lnp, hkvT8, b * 512, eng="DADA"[b], nu=4)

            kv_stats(0)
            kv_stats(1)
            kv_finish(0)
            kv_stats(2)
            kv_finish(1)
            kv_stats(3)
            kv_finish(2)
            kv_finish(3)

        restA = [None, None]
        restB = [None, None]
        rows = [None, None]

        # va pair tiles persist across layers (v columns overwritten per
        # layer, the constant denominator columns are set once here).
        va = [
            big.tile([P, 2, H, 2 * DH], FP8, name=f"va{g}", tag=f"va{g}")
            for g in range(GK)
        ]
        for g in range(GK):
            nc.gpsimd.memset(va[g][:, :, :, DH:2 * DH], 1.0 / QKS)

        def proj_q(l, hqT8_ap):
            """Q projection + pair-rearrange for layer l."""
            wq_s = dr(crit[l][:, C_WQ:C_WK], SE)
            bq = crit[l][:, C_BQ:C_END].bitcast(F32)
            q8f = work.tile([P, EC, NQ], FP8, name="q8f", tag="q8f")
            q8p = work.tile([P, EC, 2, NQ], FP8, name="q8p", tag="q8p")
            for m in range(EC):
                ps = ss_pool.tile([P, E], F32, name="pp", tag="ss")
                for half in range(2):
                    c0, c1 = half * 256, half * 256 + 256
                    for s in range(SE):
                        nc.tensor.matmul(
                            ps[:, c0:c1], wq_s[:, s, :, m * P:(m + 1) * P],
                            hqT8_ap[:, 2 * s:2 * s + 2, c0:c1],
                            start=(s == 0), stop=(s == SE - 1), perf_mode=DRM,
                            skip_group_check=True,
                        )
                if m % 2:
                    nc.scalar.activation(
                        q8f[:, m, :], ps[:], AF.Identity,
                        bias=bq[:, m:m + 1], scale=1.0 / QKS,
                    )
                else:
                    nc.vector.tensor_scalar(
                        q8f[:, m, :], ps[:], 1.0 / QKS, bq[:, m:m + 1],
                        op0=ALU.mult, op1=ALU.add,
                    )
            # pair layout: head h=2m+half at partitions [64*half, 64*half+32),
            # (p, j) <-> d-row 32*j + p.  4 plain partition-slice DMAs.
            for half in range(2):
                for jj in range(2):
                    r0 = 64 * half + 32 * jj
                    nc.sync.dma_start(
                        q8p[64 * half:64 * half + 32, :, jj, :],
                        q8f[r0:r0 + 32, :, :],
                    )
            return q8p

        def proj_kv(l):
            """K (with pair-rearrange) and V, interleaved per E-chunk."""
            wk_s = dr(crit[l][:, C_WK:C_WV], SE)
            wv_s = dr(crit[l][:, C_WV:C_BQ], SE)
            k8f = big.tile([P, EC, NKV], FP8, name="k8f", tag="k8f")
            k8p = big.tile([P, EC, 2, NKV], FP8, name="k8p", tag="k8p")
            for m in range(EC):
                for n in range(KN):
                    ps = ss_pool.tile([P, E], F32, name="pp", tag="ss")
                    for s in range(SE):
                        nc.tensor.matmul(
                            ps[:], wk_s[:, s, :, m * P:(m + 1) * P],
                            hkvT8[:, 2 * s:2 * s + 2, n * 512:(n + 1) * 512],
                            start=(s == 0), stop=(s == SE - 1), perf_mode=DRM,
                        )
                    if n % 2:
                        nc.scalar.mul(k8f[:, m, n * 512:(n + 1) * 512], ps[:], 1.0 / QKS)
                    else:
                        nc.vector.tensor_scalar_mul(
                            k8f[:, m, n * 512:(n + 1) * 512], ps[:], 1.0 / QKS
                        )
                for half in range(2):
                    for jj in range(2):
                        r0 = 64 * half + 32 * jj
                        nc.sync.dma_start(
                            k8p[64 * half:64 * half + 32, m, jj, :],
                            k8f[r0:r0 + 32, m, :],
                        )
                for mv in range(4 * m, 4 * m + 4):
                    ps = ss_pool.tile([P, E], F32, name="pp", tag="ss")
                    for s in range(SE):
                        nc.tensor.matmul(
                            ps[:], hkvT8[:, 2 * s:2 * s + 2, mv * P:(mv + 1) * P],
                            wv_s[:, s, :, :],
                            start=(s == 0), stop=(s == SE - 1), perf_mode=DRM,
                        )
                    dst = va[mv // 2][:, mv % 2, :, 0:DH]
                    src = ps[:].rearrange("p (h d) -> p h d", h=H)
                    if mv % 4 == 3:
                        nc.vector.tensor_scalar_mul(dst, src, 1.0 / QKS)
                    else:
                        nc.scalar.mul(dst, src, 1.0 / QKS)
            return k8p

        # ---- layer 0 Q/K/V ----
        q8p = proj_q(0, hqT8_l0[:])
        k8p = proj_kv(0)

        for l in range(L):
            # ---- attention, head-pairs (fp8 DR scores) ----
            aoT = work.tile([P, EC, NQ], FP8, name="aoT", tag="aoT")
            for m in range(EC):          # pair (h0, h1) = (2m, 2m+1)
                # bulk weight loads, issued mid-attention so their
                # transfers never block latency-critical DMAs.
                if l == 0 and m == 1:
                    restA[0] = load_pack(0, "restA", A_END)
                    rows[0] = load_rows(0)
                if l == 0 and m == 2:
                    crit[1] = load_pack(1, "crit", C_END)
                if l == 0 and m == 3:
                    restB[0] = load_pack(0, "restB", B_END)
                kst = [k8p[0:32, m, :, :], k8p[64:96, m, :, :]]
                qmv = [q8p[0:32, m, :, :], q8p[64:96, m, :, :]]
                pso = [
                    att_pool.tile([P, E], F32, name="ps_oT", tag="att")
                    for _ in range(2)
                ]
                exs = [[], []]
                for g in range(GK):
                    for j in range(2):
                        ex = ex_pool.tile([P, 2, NQ], FP8, name="ex", tag="ex")
                        for sub in range(2):
                            ps_s = ss_pool.tile([P, NQ], F32, name="ps_s", tag="ss")
                            c0 = (2 * g + sub) * P
                            nc.tensor.matmul(
                                ps_s[:], kst[j][:, :, c0:c0 + P], qmv[j],
                                start=True, stop=True, perf_mode=DRM,
                            )
                            if EXP_PAT[j][g] == "A":
                                nc.scalar.activation(
                                    ex[:, sub, :].bitcast(I8), ps_s[:],
                                    AF.Identity, bias=schb_col[:], scale=SCH_A,
                                )
                            else:
                                nc.vector.tensor_scalar(
                                    ex[:, sub, :].bitcast(I8), ps_s[:],
                                    SCH_A, SCH_B, op0=ALU.mult, op1=ALU.add,
                                )
                        exs[j].append(ex)
                    if g >= 1:
                        for j in range(2):
                            nc.tensor.matmul(
                                pso[j][:], va[g - 1][:, :, 2 * m + j, :],
                                exs[j][g - 1][:],
                                start=(g == 1), stop=False, perf_mode=DRM,
                            )
                for j in range(2):
                    nc.tensor.matmul(
                        pso[j][:], va[GK - 1][:, :, 2 * m + j, :],
                        exs[j][GK - 1][:],
                        start=False, stop=True, perf_mode=DRM,
                    )
                    # normalize: aoT = (x8 unnorm) * rcp(sum/8) = x64 attnout
                    rcp = stats_pool.tile([DH, NQ], BF16, name="rcp", bufs=2)
                    with nc.allow_low_precision(reason="bf16 softmax denom"):
                        nc.vector.reciprocal(rcp[:], pso[j][DH:P, :])
                    nc.vector.tensor_tensor(
                        aoT[64 * j:64 * j + 64, m, :], pso[j][0:DH, :],
                        rcp[:], op=ALU.mult,
                    )

            # ---- out-proj (DR fp8) + bo row + residual; LN2 interleaved ----
            wo_s = dr(restA[l][:, A_WO:A_W1], SE)
            h2T8 = work.tile([P, EC, NQ], FP8, name="h2T8", tag="actT")
            h2r8 = work.tile([P, EC, NQ], FP8, name="h2r8", tag="h2r8")
            hn2 = ln_hn2()
            for qc in range(QC):
                ps = ss_pool.tile([P, E], F32, name="pp", tag="ss")
                nc.tensor.matmul(
                    ps[:], ones1[:], rows[l][:, 0, :], start=True, stop=False,
                    skip_group_check=True,
                )
                for s in range(SE):
                    nc.tensor.matmul(
                        ps[:], aoT[:, 2 * s:2 * s + 2, qc * P:(qc + 1) * P],
                        wo_s[:, s, :, :],
                        start=False, stop=(s == SE - 1), perf_mode=DRM,
                        skip_group_check=True,
                    )
                nc.vector.scalar_tensor_tensor(
                    xq[qc], ps[:], 1.0 / (AOS * WS), xq[qc],
                    op0=ALU.mult, op1=ALU.add,
                )
                ln_q_tile(qc, hn2, eng="D")
                if qc % 2 == 1:
                    ln_txn(hn2, lnp, h2T8, (qc - 1) * P, resT8=h2r8,
                           eng="D", nu=2)
                    if qc == 1:
                        hn2 = ln_hn2()
            if l + 1 < L:
                restA[1] = load_pack(1, "restA", A_END)
                rows[1] = load_rows(1)

            # ---- FFN1 (DR fp8): g^T = gelu(w1^T @ h2^T / 64 + b1) ----
            w1_s = dr(restA[l][:, A_W1:A_W1R], SE)
            w1r_s = dr(restA[l][:, A_W1R:A_END], SE)
            b1 = restB[l][:, B_B1:B_END].bitcast(F32)
            gT8 = big.tile([P, MC, NQ], FP8, name="gT8", tag="gT8")
            for m in range(MC):
                ps = ss_pool.tile([P, E], F32, name="pp", tag="ss")
                for half in range(2):
                    c0, c1 = half * 256, half * 256 + 256
                    for s in range(SE):
                        nc.tensor.matmul(
                            ps[:, c0:c1], w1_s[:, s, :, m * P:(m + 1) * P],
                            h2T8[:, 2 * s:2 * s + 2, c0:c1],
                            start=(s == 0), stop=False, perf_mode=DRM,
                            skip_group_check=True,
                        )
                    for s in range(SE):
                        nc.tensor.matmul(
                            ps[:, c0:c1], w1r_s[:, s, :, m * P:(m + 1) * P],
                            h2T8[:, 2 * s:2 * s + 2, c0:c1],
                            start=False, stop=False, perf_mode=DRM,
                            skip_group_check=True,
                        )
                    for s in range(SE):
                        nc.tensor.matmul(
                            ps[:, c0:c1], w1_s[:, s, :, m * P:(m + 1) * P],
                            h2r8[:, 2 * s:2 * s + 2, c0:c1],
                            start=False, stop=(s == SE - 1), perf_mode=DRM,
                            skip_group_check=True,
                        )
                nc.scalar.activation(
                    gT8[:, m, :], ps[:], AF.Gelu,
                    bias=b1[:, m:m + 1], scale=1.0 / WS,
                )

            # ---- hoisted K/V of layer l+1 (depend only on hkvT) ----
            if l + 1 < L:
                k8p = proj_kv(l + 1)

            # ---- FFN2 (DR fp8) + b2 row + residual; next LN1 interleaved ----
            w2_s = dr(restB[l][:, B_W2:B_W2R], SM)
            w2r_s = dr(restB[l][:, B_W2R:B_B1], SM)
            if l + 1 < L:
                hqT8 = work.tile([P, EC, NQ], FP8, name="hqT8", tag="actT2")
                hn2 = ln_hn2()
            for qc in range(QC):
                ps = ss_pool.tile([P, E], F32, name="pp", tag="ss")
                nc.tensor.matmul(
                    ps[:], ones1[:], rows[l][:, 1, :], start=True, stop=False,
                    skip_group_check=True,
                )
                for s in range(SM):
                    nc.tensor.matmul(
                        ps[:], gT8[:, 2 * s:2 * s + 2, qc * P:(qc + 1) * P],
                        w2_s[:, s, :, :],
                        start=False, stop=False, perf_mode=DRM,
                        skip_group_check=True,
                    )
                for s in range(SM):
                    nc.tensor.matmul(
                        ps[:], gT8[:, 2 * s:2 * s + 2, qc * P:(qc + 1) * P],
                        w2r_s[:, s, :, :],
                        start=False, stop=(s == SM - 1), perf_mode=DRM,
                        skip_group_check=True,
                    )
                nc.vector.scalar_tensor_tensor(
                    xq[qc], ps[:], 1.0 / WS, xq[qc], op0=ALU.mult, op1=ALU.add,
                )
                if l + 1 < L:
                    ln_q_tile(qc, hn2, eng="D")
                    if qc % 2 == 1:
                        ln_txn(hn2, lnp, hqT8, (qc - 1) * P, eng="D", nu=2)
                        if qc == 1:
                            hn2 = ln_hn2()
                else:
                    nc.sync.dma_start(y_d[qc * P:(qc + 1) * P, :], xq[qc])
            if l + 1 < L:
                restB[1] = load_pack(1, "restB", B_END)
                q8p = proj_q(l + 1, hqT8[:])

    nc.compile()
    return nc


def get_nc():
    if "nc" not in _CACHE:
        _CACHE["nc"] = _build()
    return _CACHE["nc"]


def _fp8(a):
    return np.clip(np.asarray(a, np.float32), -240.0, 240.0).astype(
        ml_dtypes.float8_e4m3
    )


def _bf16(a):
    return np.asarray(a, np.float32).astype(ml_dtypes.bfloat16)


def _rearr_dr(w8, S):
    """[S*2*128, C] (row-major contraction) -> [128, S*2*C] DR pair layout."""
    C = w8.shape[1]
    return np.ascontiguousarray(
        w8.reshape(S, 2, P, C).transpose(2, 0, 1, 3).reshape(P, S * 2 * C)
    )


def _cols(v):
    """[k*128] -> [128, k]: column m holds v[m*128:(m+1)*128]."""
    k = v.shape[0] // P
    return np.ascontiguousarray(np.asarray(v, np.float32).reshape(k, P).T)


def _u8(a):
    return np.ascontiguousarray(a).view(np.uint8)


def kernel(**inputs) -> np.ndarray:
    x_q = np.asarray(inputs["x_q"], np.float32)
    x_kv = np.asarray(inputs["x_kv"], np.float32)
    wq = np.asarray(inputs["wq"], np.float32)
    wkv = np.asarray(inputs["wkv"], np.float32)
    wo = np.asarray(inputs["wo"], np.float32)
    bo = np.asarray(inputs["bo"], np.float32)
    w1 = np.asarray(inputs["w1"], np.float32)
    b1 = np.asarray(inputs["b1"], np.float32)
    w2 = np.asarray(inputs["w2"], np.float32)
    b2 = np.asarray(inputs["b2"], np.float32)
    ln1_g = np.asarray(inputs["ln1_g"], np.float32)
    ln1_b = np.asarray(inputs["ln1_b"], np.float32)
    ln2_g = np.asarray(inputs["ln2_g"], np.float32)
    ln2_b = np.asarray(inputs["ln2_b"], np.float32)

    shared = {}
    for l in range(L):
        wk_f = wkv[l][:, :E]
        wv_f = wkv[l][:, E:]
        wq_eff = ln1_g[l][:, None] * wq[l]
        wk_eff = ln1_g[l][:, None] * wk_f
        wv_eff = ln1_g[l][:, None] * wv_f
        bq_eff = ln1_b[l] @ wq[l]
        bv_eff = ln1_b[l] @ wv_f
        bo_eff = bo[l] + bv_eff @ wo[l]
        w1_eff = ln2_g[l][:, None] * w1[l]
        b1_eff = ln2_b[l] @ w1[l] + b1[l]

        wq8 = _rearr_dr(_fp8(WS * wq_eff), SE)
        wk8 = _rearr_dr(_fp8(WS * wk_eff), SE)
        wv8 = _rearr_dr(_fp8(WS * wv_eff), SE)
        wo8 = _rearr_dr(_fp8(WS * wo[l]), SE)
        w18 = _rearr_dr(_fp8(WS * w1_eff), SE)
        w28 = _rearr_dr(_fp8(WS * w2[l]), SM)
        w1r8 = _rearr_dr(_fp8(WS * w1_eff - _fp8(WS * w1_eff).astype(np.float32)), SE)
        w2r8 = _rearr_dr(_fp8(WS * w2[l] - _fp8(WS * w2[l]).astype(np.float32)), SM)
        bq_c = _cols(QKS * bq_eff)      # [128, 4] f32
        b1_c = _cols(b1_eff)            # [128, 16] f32

        crit = np.concatenate(
            [_u8(wq8), _u8(wk8), _u8(wv8), _u8(bq_c)], axis=1)
        restA = np.concatenate([_u8(wo8), _u8(w18), _u8(w1r8)], axis=1)
        restB = np.concatenate([_u8(w28), _u8(w2r8), _u8(b1_c)], axis=1)
        rows2 = np.stack(
            [_bf16(AOS * WS * bo_eff), _bf16(WS * b2[l])], axis=0)[None]
        shared.update({
            f"crit_{l}": crit.view(ml_dtypes.float8_e4m3),
            f"restA_{l}": restA.view(ml_dtypes.float8_e4m3),
            f"restB_{l}": restB.view(ml_dtypes.float8_e4m3),
            f"rows_{l}": rows2,
        })

    in_maps = []
    for c in range(8):
        b, qc = c // 4, c % 4
        m = dict(shared)
        m["xq"] = np.ascontiguousarray(x_q[b, qc * NQ:(qc + 1) * NQ, :])
        m["xkv"] = np.ascontiguousarray(_bf16(x_kv[b]))
        in_maps.append(m)

    nc = get_nc()
    res = bass_utils.run_bass_kernel_spmd(nc, in_maps, core_ids=list(range(8)))

    out = np.empty((2, 2048, E), np.float32)
    for c in range(8):
        b, qc = c // 4, c % 4
        out[b, qc * NQ:(qc + 1) * NQ, :] = res.results[c]["y"]
    return out


# revision 72
# speedup vs baseline: 1.1807x; 1.0113x over previous
"""Trainium2 Bass kernel for a 2-layer cross-encoder (CrossEncoder).

Model: B=2, NQ=NKV=2048, E=512, H=8 (d_head=64), MLP=2048, depth=2, fp32 I/O.

Sharding (8 cores, no collectives): core c handles batch b=c//4 and query
rows [qc*512, (qc+1)*512) with qc=c%4.  Each core computes the full KV
projections for its batch so every core produces its output slice
independently.

Key structure (v4):
 - All heavy matmuls fp8e4m3 + DoubleRow (0.5 cyc/row), including the
   scores matmul (q/k stored as fp8 DR pair tiles, produced by a cheap
   SBUF->SBUF DMA partition-rearrange; head h sits at base partition
   0/64 of its E-chunk pair tile so the PE base-partition rule holds).
 - Attention runs head-PAIRS interleaved: the two heads' exp chains use
   opposite engines per group, so Act and DVE both stay fed and the
   scores->exp->attnV sem latency is hidden.
 - LayerNorm activations transposed by the DMA XBAR (dma_start_transpose)
   instead of PE identity matmuls + PSUM round trips; fp8 quantize runs
   from SBUF (2x/4x DVE modes).  rstd = 1/sqrt(var+eps) via bit trick +
   1 Newton step on Pool (batched for the kv setup).
 - exp() is Schraudolph-only (int8 round writes fp8e4m3 bits directly),
   split Act (Identity activation) / DVE; Act's only table is Gelu.
 - Softmax denominator comes free from 64 constant columns per head in V.
 - x_kv is uploaded bf16; all weights of a layer arrive as 3 packed DMAs
   staged so bulk transfers never sit in front of latency-critical XBAR
   or pair-rearrange DMAs in the shared DMA queue.
 - K/V of layer l+1 (which depend only on hkvT) are hoisted between
   FFN1(l) and FFN2(l) to fill idle engines there; LN2 interleaves with
   the O-proj residuals, next-layer LN1 with the FFN2 residuals.
"""

import numpy as np
import ml_dtypes

import concourse.bass as bass
import concourse.bacc as bacc
import concourse.mybir as mybir
import concourse.tile as tile
from concourse import bass_utils
from contextlib import ExitStack

P = 128
E = 512
EC = E // P        # 4 chunks of the embedding dim
SE = EC // 2       # 2 DoubleRow super-chunks
NQ = 512           # query rows per core
QC = NQ // P       # 4 query chunks
NKV = 2048
KC = NKV // P      # 16 key chunks of 128
KN = NKV // 512    # 4 key chunks of 512
GK = KC // 2       # 8 key pair-groups
H = 8
DH = 64
MLP = 2048
MC = MLP // P      # 16 mlp chunks of 128
SM = MC // 2       # 8 DoubleRow super-chunks
L = 2
LN_EPS = 1e-5
F32 = mybir.dt.float32
I32 = mybir.dt.int32
BF16 = mybir.dt.bfloat16
FP8 = mybir.dt.float8e4
I8 = mybir.dt.int8
AF = mybir.ActivationFunctionType
ALU = mybir.AluOpType
DRM = mybir.MatmulPerfMode.DoubleRow

WS = 64.0                       # fp8 weight pre-scale (host side)
QKS = 8.0                       # q/k storage scale
SCALE = DH ** -0.5
EXPS = SCALE / (QKS * QKS)      # exp scale applied to scores psum (=1/512)
AOS = 64.0                      # attnout storage scale (fp8 subnormal guard)
SCH_A = (8.0 / np.log(2.0)) * EXPS   # Schraudolph slope for fp8e4 bits
SCH_B = 56.0 - 0.47                  # fp8e4 exponent bias term - rms shift
RSQRT_MAGIC = 0x5F3759DF
# per-(head-in-pair, group) exp engine: A=Act(Identity act) D=DVE.
# Anti-aligned so the two heads of a pair use opposite engines; 9A/7D
# because DVE also owns the softmax divide.
EXP_PAT = ["AADADADA", "ADADADAA"]
EXP_PAT_ODD = ["AADADADA", "AADADAAA"]   # 11A/5D for odd pairs (balance)
H2R = True                               # FFN1 activation-residual pass

# packed-weight byte offsets (per partition)
C_WQ, C_WK, C_WV, C_BQ, C_END = 0, 2048, 4096, 6144, 6160
A_WO, A_W1, A_W1R, A_END = 0, 2048, 10240, 18432
B_W2, B_W2R, B_B1, B_END = 0, 8192, 16384, 16448

_CACHE = {}


def _build():
    """Build the per-core Bass program (identical on all 8 cores)."""
    nc = bacc.Bacc("TRN2", target_bir_lowering=False, debug=False, num_devices=8)

    xq_d = nc.dram_tensor("xq", [NQ, E], F32, kind="ExternalInput").ap()
    xkv_d = nc.dram_tensor("xkv", [NKV, E], BF16, kind="ExternalInput").ap()
    wd = []
    for l in range(L):
        wd.append({
            "crit": nc.dram_tensor(f"crit_{l}", [P, C_END], FP8, kind="ExternalInput").ap(),
            "restA": nc.dram_tensor(f"restA_{l}", [P, A_END], FP8, kind="ExternalInput").ap(),
            "restB": nc.dram_tensor(f"restB_{l}", [P, B_END], FP8, kind="ExternalInput").ap(),
            "rows": nc.dram_tensor(f"rows_{l}", [1, 2, E], BF16, kind="ExternalInput").ap(),
        })
    y_d = nc.dram_tensor("y", [NQ, E], F32, kind="ExternalOutput").ap()

    with tile.TileContext(nc) as tc, ExitStack() as ctx:
        const_pool = ctx.enter_context(tc.tile_pool(name="const", bufs=1))
        ones1 = const_pool.tile([1, P], BF16)
        nc.gpsimd.memset(ones1[:], 1.0)
        schb_col = const_pool.tile([P, 1], F32)
        nc.gpsimd.memset(schb_col[:], SCH_B)

        stats_pool = ctx.enter_context(tc.tile_pool(name="stats", bufs=12))

        def rsqrt_chain(var_ap, rstd_ap, w):
            """rstd = 1/sqrt(var+eps) on Pool via bit trick + 1 Newton step."""
            ve = stats_pool.tile([P, 4], F32, name="ve")[:, :w]
            nc.gpsimd.tensor_scalar_add(ve, var_ap, LN_EPS)
            y0 = stats_pool.tile([P, 4], F32, name="y0")[:, :w]
            # int bit-trick ops run on DVE (Pool's Q7 rejects int shifts)
            nc.vector.tensor_scalar(
                y0.bitcast(I32), ve.bitcast(I32), 1, 0,
                op0=ALU.logical_shift_right, op1=ALU.bypass,
            )
            nc.vector.tensor_scalar(
                y0.bitcast(I32), y0.bitcast(I32), -1, RSQRT_MAGIC,
                op0=ALU.mult, op1=ALU.add,
            )
            t = stats_pool.tile([P, 4], F32, name="t")[:, :w]
            nc.gpsimd.tensor_tensor(t, y0, y0, op=ALU.mult)
            nc.gpsimd.tensor_tensor(t, t, ve, op=ALU.mult)
            nc.gpsimd.tensor_scalar(t, t, -0.5, 1.5, op0=ALU.mult, op1=ALU.add)
            nc.gpsimd.tensor_tensor(rstd_ap, y0, t, op=ALU.mult)

        def ln_stats(x_ap, mv_ap):
            bnst = stats_pool.tile([P, 6], F32, name="bnst")
            nc.vector.bn_stats(bnst[:], x_ap)
            nc.vector.bn_aggr(mv_ap, bnst[:])

        # norm for one [128, E] tile into slot u of a 4-wide hn4 buffer.
        def ln_norm(src_ap, mean_ap, rstd_ap, hn4, u, eng="D"):
            if eng == "P":
                nc.gpsimd.tensor_scalar(
                    hn4[:, u, :], src_ap, mean_ap, rstd_ap,
                    op0=ALU.subtract, op1=ALU.mult,
                )
            else:
                nc.vector.tensor_scalar(
                    hn4[:, u, :], src_ap, mean_ap, rstd_ap,
                    op0=ALU.subtract, op1=ALU.mult,
                )

        # one XBAR transpose + one quantize for an nu-tile hn batch
        # (nu*128 tokens).  dst slice [:, :, tok0:tok0+nu*128].
        def ln_txn(hn, t_pool, dstT8, tok0, resT8=None, eng="D", nu=4):
            ht = t_pool.tile([P, 4 * EC, P], BF16, name="ht", tag="ht",
                             bufs=2)[:, :nu * EC, :]
            nc.sync.dma_start_transpose(
                ht, hn[:].rearrange("p u c -> p (u c)")
            )
            dst = dstT8[:, :, tok0:tok0 + nu * P].rearrange(
                "p c (u t) -> p u c t", u=nu
            )
            src = ht.rearrange("p (u c) t -> p u c t", u=nu)
            if eng == "P":
                nc.gpsimd.tensor_copy(dst, src)
            elif eng == "D":
                nc.vector.tensor_copy(dst, src)
            else:
                nc.scalar.copy(dst, src)
            if resT8 is not None:
                # STT requires <=3D APs: one op per 128-token sub-block.
                for u in range(nu):
                    t0 = tok0 + u * P
                    nc.vector.scalar_tensor_tensor(
                        resT8[:, :, t0:t0 + P], dstT8[:, :, t0:t0 + P], -1.0,
                        ht[:, u * EC:(u + 1) * EC, :], op0=ALU.mult, op1=ALU.add,
                    )

        # Residual stream: one [128, 4, 512] fp32 tile (qc-major subtiles).
        xq_pool = ctx.enter_context(tc.tile_pool(name="xq", bufs=1))
        xqb = xq_pool.tile([P, QC, E], F32, name="xqb", tag="xqb")
        nc.sync.dma_start(xqb[:], xq_d.rearrange("(a p) c -> p a c", p=P))
        xq = [xqb[:, qc, :] for qc in range(QC)]

        # hkv^T fp8 DR tile [128, 4, NKV]: (c, p) holds E-row c*128+p.
        hkvT_pool = ctx.enter_context(tc.tile_pool(name="hkvT", bufs=1))
        hkvT8 = hkvT_pool.tile([P, EC, NKV], FP8, name="hkvT8", tag="hkvT8")

        # PSUM pools (8 banks): shared 1-bank ring 6 + att 2 = 8.
        ss_pool = ctx.enter_context(tc.tile_pool(name="ss", bufs=6, space="PSUM"))
        att_pool = ctx.enter_context(tc.tile_pool(name="attp", bufs=2, space="PSUM"))

        wpool = ctx.enter_context(tc.tile_pool(name="w", bufs=1))

        def load_pack(l, which, sz):
            t = wpool.tile([P, sz], FP8, name=f"{which}_{l}")
            nc.sync.dma_start(t[:], wd[l][which])
            return t

        def load_rows(l):
            t = wpool.tile([1, 2, E], BF16, name=f"rows_{l}")
            nc.sync.dma_start(t[:], wd[l]["rows"])
            return t

        def dr(ap, S):
            return ap.rearrange("p (s j c) -> p s j c", s=S, j=2)

        work = ctx.enter_context(tc.tile_pool(name="work", bufs=1))
        big = ctx.enter_context(tc.tile_pool(name="big", bufs=1))
        ex_pool = ctx.enter_context(tc.tile_pool(name="ex", bufs=5))
        lnp = ctx.enter_context(tc.tile_pool(name="lnp", bufs=4))

        def ln_hn2():
            return lnp.tile([P, 2, E], BF16, name="hn2", tag="hn2", bufs=3)

        # q-side LN: per-tile stats+norm into hn2 slot qc%2; a following
        # ln_txn(nu=2) finishes each half.
        def ln_q_tile(qc, hn2, eng="D"):
            mv = stats_pool.tile([P, 2], F32, name="mv")
            ln_stats(xq[qc], mv[:])
            rstd = stats_pool.tile([P, 1], F32, name="rstd")
            rsqrt_chain(mv[:, 1:2], rstd[:], 1)
            ln_norm(xq[qc], mv[:, 0:1], rstd[:], hn2, qc % 2, eng=eng)

        crit = [None, None]
        with tc.tile_pool(name="kvln", bufs=4) as kvln_pool:
            # x_kv bf16, 4 chunks of [128, 4, 512] (token-block subtiles),
            # ring of 3: chunk a+3 reuses chunk a's space.
            xkvb = []

            def load_xkvb(a):
                t = kvln_pool.tile([P, 4, E], BF16, name=f"xkvb{a}",
                                   tag="xkvb", bufs=3)
                nc.sync.dma_start(
                    t[:], xkv_d[a * 512:(a + 1) * 512, :].rearrange(
                        "(u p) c -> p u c", p=P)
                )
                xkvb.append(t)

            load_xkvb(0)
            load_xkvb(1)
            crit[0] = load_pack(0, "crit", C_END)
            load_xkvb(2)
            load_xkvb(3)

            # ---- LN1(x_q, layer 0), two halves ----
            hqT8_l0 = work.tile([P, EC, NQ], FP8, name="hqT8_l0", tag="actT")
            for half in range(2):
                hn2 = ln_hn2()
                for qc in (2 * half, 2 * half + 1):
                    ln_q_tile(qc, hn2, eng="D")
                ln_txn(hn2, lnp, hqT8_l0, half * 256, eng="D", nu=2)

            # ---- startup: x_kv LN + XBAR transpose + fp8 quantize.
            #      Stats run one batch ahead of the norm/xbar chains so
            #      DVE's queue never blocks the next batch's stats. ----
            mv4s, rstd4s = {}, {}

            def kv_stats(b):
                mv4 = stats_pool.tile([P, 2, 4], F32, name="mv4", bufs=4)
                for u in range(4):
                    ln_stats(xkvb[b][:, u, :], mv4[:, :, u])
                rstd4 = stats_pool.tile([P, 4], F32, name="rstd4", bufs=4)
                rsqrt_chain(mv4[:, 1, :], rstd4[:], 4)
                mv4s[b], rstd4s[b] = mv4, rstd4

            def kv_finish(b):
                khn4 = kvln_pool.tile([P, 4, E], BF16, name="khn4",
                                      tag="khn4", bufs=2)
                for u in range(4):
                    ln_norm(xkvb[b][:, u, :], mv4s[b][:, 0, u:u + 1],
                            rstd4s[b][:, u:u + 1], khn4, u, eng="DDAD"[u])
                ln_txn(khn4, lnp, hkvT8, b * 512, eng="DADA"[b], nu=4)

            kv_stats(0)
            kv_stats(1)
            kv_finish(0)
            kv_stats(2)
            kv_finish(1)
            kv_stats(3)
            kv_finish(2)
            kv_finish(3)

        restA = [None, None]
        restB = [None, None]
        rows = [None, None]

        # va pair tiles persist across layers (v columns overwritten per
        # layer, the constant denominator columns are set once here).
        va = [
            big.tile([P, 2, H, 2 * DH], FP8, name=f"va{g}", tag=f"va{g}")
            for g in range(GK)
        ]
        for g in range(GK):
            nc.gpsimd.memset(va[g][:, :, :, DH:2 * DH], 1.0 / QKS)

        def proj_q(l, hqT8_ap):
            """Q projection + pair-rearrange for layer l."""
            wq_s = dr(crit[l][:, C_WQ:C_WK], SE)
            bq = crit[l][:, C_BQ:C_END].bitcast(F32)
            q8f = work.tile([P, EC, NQ], FP8, name="q8f", tag="q8f")
            q8p = work.tile([P, EC, 2, NQ], FP8, name="q8p", tag="q8p")
            for m in range(EC):
                ps = ss_pool.tile([P, E], F32, name="pp", tag="ss")
                for half in range(2):
                    c0, c1 = half * 256, half * 256 + 256
                    for s in range(SE):
                        nc.tensor.matmul(
                            ps[:, c0:c1], wq_s[:, s, :, m * P:(m + 1) * P],
                            hqT8_ap[:, 2 * s:2 * s + 2, c0:c1],
                            start=(s == 0), stop=(s == SE - 1), perf_mode=DRM,
                            skip_group_check=True,
                        )
                if m % 2:
                    nc.scalar.activation(
                        q8f[:, m, :], ps[:], AF.Identity,
                        bias=bq[:, m:m + 1], scale=1.0 / QKS,
                    )
                else:
                    nc.vector.tensor_scalar(
                        q8f[:, m, :], ps[:], 1.0 / QKS, bq[:, m:m + 1],
                        op0=ALU.mult, op1=ALU.add,
                    )
            # pair layout: head h=2m+half at partitions [64*half, 64*half+32),
            # (p, j) <-> d-row 32*j + p.  4 plain partition-slice DMAs.
            for half in range(2):
                for jj in range(2):
                    r0 = 64 * half + 32 * jj
                    nc.sync.dma_start(
                        q8p[64 * half:64 * half + 32, :, jj, :],
                        q8f[r0:r0 + 32, :, :],
                    )
            return q8p

        def proj_kv(l):
            """K (with pair-rearrange) and V, interleaved per E-chunk."""
            wk_s = dr(crit[l][:, C_WK:C_WV], SE)
            wv_s = dr(crit[l][:, C_WV:C_BQ], SE)
            k8f = big.tile([P, EC, NKV], FP8, name="k8f", tag="k8f")
            k8p = big.tile([P, EC, 2, NKV], FP8, name="k8p", tag="k8p")
            for m in range(EC):
                for n in range(KN):
                    ps = ss_pool.tile([P, E], F32, name="pp", tag="ss")
                    for s in range(SE):
                        nc.tensor.matmul(
                            ps[:], wk_s[:, s, :, m * P:(m + 1) * P],
                            hkvT8[:, 2 * s:2 * s + 2, n * 512:(n + 1) * 512],
                            start=(s == 0), stop=(s == SE - 1), perf_mode=DRM,
                        )
                    if n % 2:
                        nc.scalar.mul(k8f[:, m, n * 512:(n + 1) * 512], ps[:], 1.0 / QKS)
                    else:
                        nc.vector.tensor_scalar_mul(
                            k8f[:, m, n * 512:(n + 1) * 512], ps[:], 1.0 / QKS
                        )
                for half in range(2):
                    for jj in range(2):
                        r0 = 64 * half + 32 * jj
                        nc.sync.dma_start(
                            k8p[64 * half:64 * half + 32, m, jj, :],
                            k8f[r0:r0 + 32, m, :],
                        )
                for mv in range(4 * m, 4 * m + 4):
                    ps = ss_pool.tile([P, E], F32, name="pp", tag="ss")
                    for s in range(SE):
                        nc.tensor.matmul(
                            ps[:], hkvT8[:, 2 * s:2 * s + 2, mv * P:(mv + 1) * P],
                            wv_s[:, s, :, :],
                            start=(s == 0), stop=(s == SE - 1), perf_mode=DRM,
                        )
                    dst = va[mv // 2][:, mv % 2, :, 0:DH]
                    src = ps[:].rearrange("p (h d) -> p h d", h=H)
                    if mv % 4 == 3:
                        nc.vector.tensor_scalar_mul(dst, src, 1.0 / QKS)
                    else:
                        nc.scalar.mul(dst, src, 1.0 / QKS)
            return k8p

        # ---- layer 0 Q/K/V ----
        q8p = proj_q(0, hqT8_l0[:])
        k8p = proj_kv(0)

        for l in range(L):
            # ---- attention, head-pairs (fp8 DR scores) ----
            aoT = work.tile([P, EC, NQ], FP8, name="aoT", tag="aoT")
            for m in range(EC):          # pair (h0, h1) = (2m, 2m+1)
                # bulk weight loads, issued mid-attention so their
                # transfers never block latency-critical DMAs.
                if l == 0 and m == 1:
                    restA[0] = load_pack(0, "restA", A_END)
                    rows[0] = load_rows(0)
                if l == 0 and m == 2:
                    crit[1] = load_pack(1, "crit", C_END)
                if l == 0 and m == 3:
                    restB[0] = load_pack(0, "restB", B_END)
                kst = [k8p[0:32, m, :, :], k8p[64:96, m, :, :]]
                qmv = [q8p[0:32, m, :, :], q8p[64:96, m, :, :]]
                pso = [
                    att_pool.tile([P, E], F32, name="ps_oT", tag="att")
                    for _ in range(2)
                ]
                exs = [[], []]
                for g in range(GK):
                    for j in range(2):
                        ex = ex_pool.tile([P, 2, NQ], FP8, name="ex", tag="ex")
                        for sub in range(2):
                            ps_s = ss_pool.tile([P, NQ], F32, name="ps_s", tag="ss")
                            c0 = (2 * g + sub) * P
                            nc.tensor.matmul(
                                ps_s[:], kst[j][:, :, c0:c0 + P], qmv[j],
                                start=True, stop=True, perf_mode=DRM,
                            )
                            pat = EXP_PAT if m % 2 == 0 else EXP_PAT_ODD
                            if pat[j][g] == "A":
                                nc.scalar.activation(
                                    ex[:, sub, :].bitcast(I8), ps_s[:],
                                    AF.Identity, bias=schb_col[:], scale=SCH_A,
                                )
                            else:
                                nc.vector.tensor_scalar(
                                    ex[:, sub, :].bitcast(I8), ps_s[:],
                                    SCH_A, SCH_B, op0=ALU.mult, op1=ALU.add,
                                )
                        exs[j].append(ex)
                    if g >= 1:
                        for j in range(2):
                            nc.tensor.matmul(
                                pso[j][:], va[g - 1][:, :, 2 * m + j, :],
                                exs[j][g - 1][:],
                                start=(g == 1), stop=False, perf_mode=DRM,
                            )
                for j in range(2):
                    nc.tensor.matmul(
                        pso[j][:], va[GK - 1][:, :, 2 * m + j, :],
                        exs[j][GK - 1][:],
                        start=False, stop=True, perf_mode=DRM,
                    )
                    # normalize: aoT = (x8 unnorm) * rcp(sum/8) = x64 attnout
                    rcp = stats_pool.tile([DH, NQ], BF16, name="rcp", bufs=2)
                    with nc.allow_low_precision(reason="bf16 softmax denom"):
                        nc.vector.reciprocal(rcp[:], pso[j][DH:P, :])
                    nc.vector.tensor_tensor(
                        aoT[64 * j:64 * j + 64, m, :], pso[j][0:DH, :],
                        rcp[:], op=ALU.mult,
                    )

            # ---- out-proj (DR fp8) + bo row + residual; LN2 interleaved ----
            wo_s = dr(restA[l][:, A_WO:A_W1], SE)
            h2T8 = work.tile([P, EC, NQ], FP8, name="h2T8", tag="actT")
            h2r8 = (work.tile([P, EC, NQ], FP8, name="h2r8", tag="h2r8")
                    if H2R else None)
            hn2 = ln_hn2()
            for qc in range(QC):
                ps = ss_pool.tile([P, E], F32, name="pp", tag="ss")
                nc.tensor.matmul(
                    ps[:], ones1[:], rows[l][:, 0, :], start=True, stop=False,
                    skip_group_check=True,
                )
                for s in range(SE):
                    nc.tensor.matmul(
                        ps[:], aoT[:, 2 * s:2 * s + 2, qc * P:(qc + 1) * P],
                        wo_s[:, s, :, :],
                        start=False, stop=(s == SE - 1), perf_mode=DRM,
                        skip_group_check=True,
                    )
                nc.vector.scalar_tensor_tensor(
                    xq[qc], ps[:], 1.0 / (AOS * WS), xq[qc],
                    op0=ALU.mult, op1=ALU.add,
                )
                ln_q_tile(qc, hn2, eng="D")
                if qc % 2 == 1:
                    ln_txn(hn2, lnp, h2T8, (qc - 1) * P, resT8=h2r8,
                           eng="D", nu=2)
                    if qc == 1:
                        hn2 = ln_hn2()
            if l + 1 < L:
                restA[1] = load_pack(1, "restA", A_END)
                rows[1] = load_rows(1)

            # ---- hoisted K/V of layer l+1 (depend only on hkvT): emitted
            #      here so the PE fills the LN2-chain gap before FFN1. ----
            if l + 1 < L:
                k8p = proj_kv(l + 1)

            # ---- FFN1 (DR fp8): g^T = gelu(w1^T @ h2^T / 64 + b1) ----
            w1_s = dr(restA[l][:, A_W1:A_W1R], SE)
            w1r_s = dr(restA[l][:, A_W1R:A_END], SE)
            b1 = restB[l][:, B_B1:B_END].bitcast(F32)
            gT8 = big.tile([P, MC, NQ], FP8, name="gT8", tag="gT8")
            for m in range(MC):
                ps = ss_pool.tile([P, E], F32, name="pp", tag="ss")
                for half in range(2):
                    c0, c1 = half * 256, half * 256 + 256
                    for s in range(SE):
                        nc.tensor.matmul(
                            ps[:, c0:c1], w1_s[:, s, :, m * P:(m + 1) * P],
                            h2T8[:, 2 * s:2 * s + 2, c0:c1],
                            start=(s == 0), stop=False, perf_mode=DRM,
                            skip_group_check=True,
                        )
                    for s in range(SE):
                        nc.tensor.matmul(
                            ps[:, c0:c1], w1r_s[:, s, :, m * P:(m + 1) * P],
                            h2T8[:, 2 * s:2 * s + 2, c0:c1],
                            start=False, stop=(not H2R and s == SE - 1),
                            perf_mode=DRM, skip_group_check=True,
                        )
                    if H2R:
                        for s in range(SE):
                            nc.tensor.matmul(
                                ps[:, c0:c1], w1_s[:, s, :, m * P:(m + 1) * P],
                                h2r8[:, 2 * s:2 * s + 2, c0:c1],
                                start=False, stop=(s == SE - 1), perf_mode=DRM,
                                skip_group_check=True,
                            )
                nc.scalar.activation(
                    gT8[:, m, :], ps[:], AF.Gelu,
                    bias=b1[:, m:m + 1], scale=1.0 / WS,
                )

            # ---- FFN2 (DR fp8) + b2 row + residual; next LN1 interleaved ----
            w2_s = dr(restB[l][:, B_W2:B_W2R], SM)
            w2r_s = dr(restB[l][:, B_W2R:B_B1], SM)
            if l + 1 < L:
                hqT8 = work.tile([P, EC, NQ], FP8, name="hqT8", tag="actT2")
                hn2 = ln_hn2()
            for qc in range(QC):
                ps = ss_pool.tile([P, E], F32, name="pp", tag="ss")
                nc.tensor.matmul(
                    ps[:], ones1[:], rows[l][:, 1, :], start=True, stop=False,
                    skip_group_check=True,
                )
                for s in range(SM):
                    nc.tensor.matmul(
                        ps[:], gT8[:, 2 * s:2 * s + 2, qc * P:(qc + 1) * P],
                        w2_s[:, s, :, :],
                        start=False, stop=False, perf_mode=DRM,
                        skip_group_check=True,
                    )
                for s in range(SM):
                    nc.tensor.matmul(
                        ps[:], gT8[:, 2 * s:2 * s + 2, qc * P:(qc + 1) * P],
                        w2r_s[:, s, :, :],
                        start=False, stop=(s == SM - 1), perf_mode=DRM,
                        skip_group_check=True,
                    )
                nc.vector.scalar_tensor_tensor(
                    xq[qc], ps[:], 1.0 / WS, xq[qc], op0=ALU.mult, op1=ALU.add,
                )
                if l + 1 < L:
                    ln_q_tile(qc, hn2, eng="D")
                    if qc % 2 == 1:
                        ln_txn(hn2, lnp, hqT8, (qc - 1) * P, eng="D", nu=2)
                        if qc == 1:
                            hn2 = ln_hn2()
                else:
                    nc.sync.dma_start(y_d[qc * P:(qc + 1) * P, :], xq[qc])
            if l + 1 < L:
                restB[1] = load_pack(1, "restB", B_END)
                q8p = proj_q(l + 1, hqT8[:])

    nc.compile()
    return nc


def get_nc():
    if "nc" not in _CACHE:
        _CACHE["nc"] = _build()
    return _CACHE["nc"]


def _fp8(a):
    return np.clip(np.asarray(a, np.float32), -240.0, 240.0).astype(
        ml_dtypes.float8_e4m3
    )


def _bf16(a):
    return np.asarray(a, np.float32).astype(ml_dtypes.bfloat16)


def _rearr_dr(w8, S):
    """[S*2*128, C] (row-major contraction) -> [128, S*2*C] DR pair layout."""
    C = w8.shape[1]
    return np.ascontiguousarray(
        w8.reshape(S, 2, P, C).transpose(2, 0, 1, 3).reshape(P, S * 2 * C)
    )


def _cols(v):
    """[k*128] -> [128, k]: column m holds v[m*128:(m+1)*128]."""
    k = v.shape[0] // P
    return np.ascontiguousarray(np.asarray(v, np.float32).reshape(k, P).T)


def _u8(a):
    return np.ascontiguousarray(a).view(np.uint8)


def kernel(**inputs) -> np.ndarray:
    x_q = np.asarray(inputs["x_q"], np.float32)
    x_kv = np.asarray(inputs["x_kv"], np.float32)
    wq = np.asarray(inputs["wq"], np.float32)
    wkv = np.asarray(inputs["wkv"], np.float32)
    wo = np.asarray(inputs["wo"], np.float32)
    bo = np.asarray(inputs["bo"], np.float32)
    w1 = np.asarray(inputs["w1"], np.float32)
    b1 = np.asarray(inputs["b1"], np.float32)
    w2 = np.asarray(inputs["w2"], np.float32)
    b2 = np.asarray(inputs["b2"], np.float32)
    ln1_g = np.asarray(inputs["ln1_g"], np.float32)
    ln1_b = np.asarray(inputs["ln1_b"], np.float32)
    ln2_g = np.asarray(inputs["ln2_g"], np.float32)
    ln2_b = np.asarray(inputs["ln2_b"], np.float32)

    shared = {}
    for l in range(L):
        wk_f = wkv[l][:, :E]
        wv_f = wkv[l][:, E:]
        wq_eff = ln1_g[l][:, None] * wq[l]
        wk_eff = ln1_g[l][:, None] * wk_f
        wv_eff = ln1_g[l][:, None] * wv_f
        bq_eff = ln1_b[l] @ wq[l]
        bv_eff = ln1_b[l] @ wv_f
        bo_eff = bo[l] + bv_eff @ wo[l]
        w1_eff = ln2_g[l][:, None] * w1[l]
        b1_eff = ln2_b[l] @ w1[l] + b1[l]

        wq8 = _rearr_dr(_fp8(WS * wq_eff), SE)
        wk8 = _rearr_dr(_fp8(WS * wk_eff), SE)
        wv8 = _rearr_dr(_fp8(WS * wv_eff), SE)
        wo8 = _rearr_dr(_fp8(WS * wo[l]), SE)
        w18 = _rearr_dr(_fp8(WS * w1_eff), SE)
        w28 = _rearr_dr(_fp8(WS * w2[l]), SM)
        w1r8 = _rearr_dr(_fp8(WS * w1_eff - _fp8(WS * w1_eff).astype(np.float32)), SE)
        w2r8 = _rearr_dr(_fp8(WS * w2[l] - _fp8(WS * w2[l]).astype(np.float32)), SM)
        bq_c = _cols(QKS * bq_eff)      # [128, 4] f32
        b1_c = _cols(b1_eff)            # [128, 16] f32

        crit = np.concatenate(
            [_u8(wq8), _u8(wk8), _u8(wv8), _u8(bq_c)], axis=1)
        restA = np.concatenate([_u8(wo8), _u8(w18), _u8(w1r8)], axis=1)
        restB = np.concatenate([_u8(w28), _u8(w2r8), _u8(b1_c)], axis=1)
        rows2 = np.stack(
            [_bf16(AOS * WS * bo_eff), _bf16(WS * b2[l])], axis=0)[None]
        shared.update({
            f"crit_{l}": crit.view(ml_dtypes.float8_e4m3),
            f"restA_{l}": restA.view(ml_dtypes.float8_e4m3),
            f"restB_{l}": restB.view(ml_dtypes.float8_e4m3),
            f"rows_{l}": rows2,
        })

    in_maps = []
    for c in range(8):
        b, qc = c // 4, c % 4
        m = dict(shared)
        m["xq"] = np.ascontiguousarray(x_q[b, qc * NQ:(qc + 1) * NQ, :])
        m["xkv"] = np.ascontiguousarray(_bf16(x_kv[b]))
        in_maps.append(m)

    nc = get_nc()
    res = bass_utils.run_bass_kernel_spmd(nc, in_maps, core_ids=list(range(8)))

    out = np.empty((2, 2048, E), np.float32)
    for c in range(8):
        b, qc = c // 4, c % 4
        out[b, qc * NQ:(qc + 1) * NQ, :] = res.results[c]["y"]
    return out


# revision 74
# speedup vs baseline: 1.2115x; 1.0261x over previous
"""Trainium2 Bass kernel for a 2-layer cross-encoder (CrossEncoder).

Model: B=2, NQ=NKV=2048, E=512, H=8 (d_head=64), MLP=2048, depth=2, fp32 I/O.

Sharding (8 cores, no collectives): core c handles batch b=c//4 and query
rows [qc*512, (qc+1)*512) with qc=c%4.  Each core computes the full KV
projections for its batch so every core produces its output slice
independently.

Key structure (v4):
 - All heavy matmuls fp8e4m3 + DoubleRow (0.5 cyc/row), including the
   scores matmul (q/k stored as fp8 DR pair tiles, produced by a cheap
   SBUF->SBUF DMA partition-rearrange; head h sits at base partition
   0/64 of its E-chunk pair tile so the PE base-partition rule holds).
 - Attention runs head-PAIRS interleaved: the two heads' exp chains use
   opposite engines per group, so Act and DVE both stay fed and the
   scores->exp->attnV sem latency is hidden.
 - LayerNorm activations transposed by the DMA XBAR (dma_start_transpose)
   instead of PE identity matmuls + PSUM round trips; fp8 quantize runs
   from SBUF (2x/4x DVE modes).  rstd = 1/sqrt(var+eps) via bit trick +
   1 Newton step on Pool (batched for the kv setup).
 - exp() is Schraudolph-only (int8 round writes fp8e4m3 bits directly),
   split Act (Identity activation) / DVE; Act's only table is Gelu.
 - Softmax denominator comes free from 64 constant columns per head in V.
 - x_kv is uploaded bf16; all weights of a layer arrive as 3 packed DMAs
   staged so bulk transfers never sit in front of latency-critical XBAR
   or pair-rearrange DMAs in the shared DMA queue.
 - K/V of layer l+1 (which depend only on hkvT) are hoisted between
   FFN1(l) and FFN2(l) to fill idle engines there; LN2 interleaves with
   the O-proj residuals, next-layer LN1 with the FFN2 residuals.
"""

import numpy as np
import ml_dtypes

import concourse.bass as bass
import concourse.bacc as bacc
import concourse.mybir as mybir
import concourse.tile as tile
from concourse import bass_utils
from contextlib import ExitStack

P = 128
E = 512
EC = E // P        # 4 chunks of the embedding dim
SE = EC // 2       # 2 DoubleRow super-chunks
NQ = 512           # query rows per core
QC = NQ // P       # 4 query chunks
NKV = 2048
KC = NKV // P      # 16 key chunks of 128
KN = NKV // 512    # 4 key chunks of 512
GK = KC // 2       # 8 key pair-groups
H = 8
DH = 64
MLP = 2048
MC = MLP // P      # 16 mlp chunks of 128
SM = MC // 2       # 8 DoubleRow super-chunks
L = 2
LN_EPS = 1e-5
F32 = mybir.dt.float32
I32 = mybir.dt.int32
BF16 = mybir.dt.bfloat16
FP8 = mybir.dt.float8e4
I8 = mybir.dt.int8
AF = mybir.ActivationFunctionType
ALU = mybir.AluOpType
DRM = mybir.MatmulPerfMode.DoubleRow

WS = 64.0                       # fp8 weight pre-scale (host side)
QKS = 8.0                       # q/k storage scale
SCALE = DH ** -0.5
EXPS = SCALE / (QKS * QKS)      # exp scale applied to scores psum (=1/512)
AOS = 64.0                      # attnout storage scale (fp8 subnormal guard)
SCH_A = (8.0 / np.log(2.0)) * EXPS   # Schraudolph slope for fp8e4 bits
SCH_B = 56.0 - 0.47                  # fp8e4 exponent bias term - rms shift
RSQRT_MAGIC = 0x5F3759DF
# per-(head-in-pair, group) exp engine: A=Act(Identity act) D=DVE.
# Anti-aligned so the two heads of a pair use opposite engines; 9A/7D
# because DVE also owns the softmax divide.
EXP_PAT = ["AADADADA", "ADADADAA"]
EXP_PAT_ODD = ["AADADADA", "DDADADAA"]   # 9A/7D for odd pairs (balance)
H2R = False                              # FFN1 activation-residual pass

# packed-weight byte offsets (per partition)
C_WQ, C_WK, C_WV, C_BQ, C_END = 0, 2048, 4096, 6144, 6160
A_WO, A_W1, A_W1R, A_END = 0, 2048, 10240, 18432
B_W2, B_W2R, B_B1, B_END = 0, 8192, 16384, 16448

_CACHE = {}


def _build():
    """Build the per-core Bass program (identical on all 8 cores)."""
    nc = bacc.Bacc("TRN2", target_bir_lowering=False, debug=False, num_devices=8)

    xq_d = nc.dram_tensor("xq", [NQ, E], F32, kind="ExternalInput").ap()
    xkv_d = nc.dram_tensor("xkv", [NKV, E], BF16, kind="ExternalInput").ap()
    wd = []
    for l in range(L):
        wd.append({
            "crit": nc.dram_tensor(f"crit_{l}", [P, C_END], FP8, kind="ExternalInput").ap(),
            "restA": nc.dram_tensor(f"restA_{l}", [P, A_END], FP8, kind="ExternalInput").ap(),
            "restB": nc.dram_tensor(f"restB_{l}", [P, B_END], FP8, kind="ExternalInput").ap(),
            "rows": nc.dram_tensor(f"rows_{l}", [1, 2, E], BF16, kind="ExternalInput").ap(),
        })
    y_d = nc.dram_tensor("y", [NQ, E], F32, kind="ExternalOutput").ap()

    with tile.TileContext(nc) as tc, ExitStack() as ctx:
        const_pool = ctx.enter_context(tc.tile_pool(name="const", bufs=1))
        ones1 = const_pool.tile([1, P], BF16)
        nc.gpsimd.memset(ones1[:], 1.0)
        schb_col = const_pool.tile([P, 1], F32)
        nc.gpsimd.memset(schb_col[:], SCH_B)

        stats_pool = ctx.enter_context(tc.tile_pool(name="stats", bufs=12))

        def rsqrt_chain(var_ap, rstd_ap, w):
            """rstd = 1/sqrt(var+eps) on Pool via bit trick + 1 Newton step."""
            ve = stats_pool.tile([P, 4], F32, name="ve")[:, :w]
            nc.gpsimd.tensor_scalar_add(ve, var_ap, LN_EPS)
            y0 = stats_pool.tile([P, 4], F32, name="y0")[:, :w]
            # int bit-trick ops run on DVE (Pool's Q7 rejects int shifts)
            nc.vector.tensor_scalar(
                y0.bitcast(I32), ve.bitcast(I32), 1, 0,
                op0=ALU.logical_shift_right, op1=ALU.bypass,
            )
            nc.vector.tensor_scalar(
                y0.bitcast(I32), y0.bitcast(I32), -1, RSQRT_MAGIC,
                op0=ALU.mult, op1=ALU.add,
            )
            t = stats_pool.tile([P, 4], F32, name="t")[:, :w]
            nc.gpsimd.tensor_tensor(t, y0, y0, op=ALU.mult)
            nc.gpsimd.tensor_tensor(t, t, ve, op=ALU.mult)
            nc.gpsimd.tensor_scalar(t, t, -0.5, 1.5, op0=ALU.mult, op1=ALU.add)
            nc.gpsimd.tensor_tensor(rstd_ap, y0, t, op=ALU.mult)

        def ln_stats(x_ap, mv_ap):
            bnst = stats_pool.tile([P, 6], F32, name="bnst")
            nc.vector.bn_stats(bnst[:], x_ap)
            nc.vector.bn_aggr(mv_ap, bnst[:])

        # norm for one [128, E] tile into slot u of a 4-wide hn4 buffer.
        def ln_norm(src_ap, mean_ap, rstd_ap, hn4, u, eng="D"):
            if eng == "P":
                nc.gpsimd.tensor_scalar(
                    hn4[:, u, :], src_ap, mean_ap, rstd_ap,
                    op0=ALU.subtract, op1=ALU.mult,
                )
            else:
                nc.vector.tensor_scalar(
                    hn4[:, u, :], src_ap, mean_ap, rstd_ap,
                    op0=ALU.subtract, op1=ALU.mult,
                )

        # one XBAR transpose + one quantize for an nu-tile hn batch
        # (nu*128 tokens).  dst slice [:, :, tok0:tok0+nu*128].
        def ln_txn(hn, t_pool, dstT8, tok0, resT8=None, eng="D", nu=4):
            ht = t_pool.tile([P, 4 * EC, P], BF16, name="ht", tag="ht",
                             bufs=2)[:, :nu * EC, :]
            nc.sync.dma_start_transpose(
                ht, hn[:].rearrange("p u c -> p (u c)")
            )
            dst = dstT8[:, :, tok0:tok0 + nu * P].rearrange(
                "p c (u t) -> p u c t", u=nu
            )
            src = ht.rearrange("p (u c) t -> p u c t", u=nu)
            if eng == "P":
                nc.gpsimd.tensor_copy(dst, src)
            elif eng == "D":
                nc.vector.tensor_copy(dst, src)
            else:
                nc.scalar.copy(dst, src)
            if resT8 is not None:
                # STT requires <=3D APs: one op per 128-token sub-block.
                for u in range(nu):
                    t0 = tok0 + u * P
                    nc.vector.scalar_tensor_tensor(
                        resT8[:, :, t0:t0 + P], dstT8[:, :, t0:t0 + P], -1.0,
                        ht[:, u * EC:(u + 1) * EC, :], op0=ALU.mult, op1=ALU.add,
                    )

        # Residual stream: one [128, 4, 512] fp32 tile (qc-major subtiles).
        xq_pool = ctx.enter_context(tc.tile_pool(name="xq", bufs=1))
        xqb = xq_pool.tile([P, QC, E], F32, name="xqb", tag="xqb")
        nc.sync.dma_start(xqb[:], xq_d.rearrange("(a p) c -> p a c", p=P))
        xq = [xqb[:, qc, :] for qc in range(QC)]

        # hkv^T fp8 DR tile [128, 4, NKV]: (c, p) holds E-row c*128+p.
        hkvT_pool = ctx.enter_context(tc.tile_pool(name="hkvT", bufs=1))
        hkvT8 = hkvT_pool.tile([P, EC, NKV], FP8, name="hkvT8", tag="hkvT8")

        # PSUM pools (8 banks): shared 1-bank ring 6 + att 2 = 8.
        ss_pool = ctx.enter_context(tc.tile_pool(name="ss", bufs=6, space="PSUM"))
        att_pool = ctx.enter_context(tc.tile_pool(name="attp", bufs=2, space="PSUM"))

        wpool = ctx.enter_context(tc.tile_pool(name="w", bufs=1))

        def load_pack(l, which, sz):
            t = wpool.tile([P, sz], FP8, name=f"{which}_{l}")
            nc.sync.dma_start(t[:], wd[l][which])
            return t

        def load_rows(l):
            t = wpool.tile([1, 2, E], BF16, name=f"rows_{l}")
            nc.sync.dma_start(t[:], wd[l]["rows"])
            return t

        def dr(ap, S):
            return ap.rearrange("p (s j c) -> p s j c", s=S, j=2)

        work = ctx.enter_context(tc.tile_pool(name="work", bufs=1))
        big = ctx.enter_context(tc.tile_pool(name="big", bufs=1))
        ex_pool = ctx.enter_context(tc.tile_pool(name="ex", bufs=5))
        lnp = ctx.enter_context(tc.tile_pool(name="lnp", bufs=4))

        def ln_hn2():
            return lnp.tile([P, 2, E], BF16, name="hn2", tag="hn2", bufs=3)

        # q-side LN: per-tile stats+norm into hn2 slot qc%2; a following
        # ln_txn(nu=2) finishes each half.
        def ln_q_tile(qc, hn2, eng="D"):
            mv = stats_pool.tile([P, 2], F32, name="mv")
            ln_stats(xq[qc], mv[:])
            rstd = stats_pool.tile([P, 1], F32, name="rstd")
            rsqrt_chain(mv[:, 1:2], rstd[:], 1)
            ln_norm(xq[qc], mv[:, 0:1], rstd[:], hn2, qc % 2, eng=eng)

        crit = [None, None]
        with tc.tile_pool(name="kvln", bufs=4) as kvln_pool:
            # x_kv bf16, 4 chunks of [128, 4, 512] (token-block subtiles),
            # ring of 3: chunk a+3 reuses chunk a's space.
            xkvb = []

            def load_xkvb(a):
                t = kvln_pool.tile([P, 4, E], BF16, name=f"xkvb{a}",
                                   tag="xkvb", bufs=3)
                nc.sync.dma_start(
                    t[:], xkv_d[a * 512:(a + 1) * 512, :].rearrange(
                        "(u p) c -> p u c", p=P)
                )
                xkvb.append(t)

            load_xkvb(0)
            load_xkvb(1)
            crit[0] = load_pack(0, "crit", C_END)
            load_xkvb(2)
            load_xkvb(3)

            # ---- startup: x_kv LN + XBAR transpose + fp8 quantize.
            #      Stats run one batch ahead of the norm/xbar chains so
            #      DVE's queue never blocks the next batch's stats. ----
            mv4s, rstd4s = {}, {}

            def kv_stats(b):
                mv4 = stats_pool.tile([P, 2, 4], F32, name="mv4", bufs=4)
                for u in range(4):
                    ln_stats(xkvb[b][:, u, :], mv4[:, :, u])
                rstd4 = stats_pool.tile([P, 4], F32, name="rstd4", bufs=4)
                rsqrt_chain(mv4[:, 1, :], rstd4[:], 4)
                mv4s[b], rstd4s[b] = mv4, rstd4

            def kv_finish(b):
                khn4 = kvln_pool.tile([P, 4, E], BF16, name="khn4",
                                      tag="khn4", bufs=2)
                for u in range(4):
                    ln_norm(xkvb[b][:, u, :], mv4s[b][:, 0, u:u + 1],
                            rstd4s[b][:, u:u + 1], khn4, u, eng="DDAD"[u])
                ln_txn(khn4, lnp, hkvT8, b * 512, eng="DADA"[b], nu=4)

            kv_stats(0)
            kv_stats(1)
            kv_finish(0)
            kv_stats(2)

            # ---- LN1(x_q, layer 0), two halves (after the kv-chain's
            #      critical stats so DVE unblocks the K gate first) ----
            hqT8_l0 = work.tile([P, EC, NQ], FP8, name="hqT8_l0", tag="actT")
            for half in range(2):
                hn2 = ln_hn2()
                for qc in (2 * half, 2 * half + 1):
                    ln_q_tile(qc, hn2, eng="D")
                ln_txn(hn2, lnp, hqT8_l0, half * 256, eng="D", nu=2)

            kv_finish(1)
            kv_stats(3)
            kv_finish(2)
            kv_finish(3)

        restA = [None, None]
        restB = [None, None]
        rows = [None, None]

        # va pair tiles persist across layers (v columns overwritten per
        # layer, the constant denominator columns are set once here).
        va = [
            big.tile([P, 2, H, 2 * DH], FP8, name=f"va{g}", tag=f"va{g}")
            for g in range(GK)
        ]
        for g in range(GK):
            nc.gpsimd.memset(va[g][:, :, :, DH:2 * DH], 1.0 / QKS)

        def proj_q(l, hqT8_ap):
            """Q projection + pair-rearrange for layer l."""
            wq_s = dr(crit[l][:, C_WQ:C_WK], SE)
            bq = crit[l][:, C_BQ:C_END].bitcast(F32)
            q8f = work.tile([P, EC, NQ], FP8, name="q8f", tag="q8f")
            q8p = work.tile([P, EC, 2, NQ], FP8, name="q8p", tag="q8p")
            for m in range(EC):
                ps = ss_pool.tile([P, E], F32, name="pp", tag="ss")
                for half in range(2):
                    c0, c1 = half * 256, half * 256 + 256
                    for s in range(SE):
                        nc.tensor.matmul(
                            ps[:, c0:c1], wq_s[:, s, :, m * P:(m + 1) * P],
                            hqT8_ap[:, 2 * s:2 * s + 2, c0:c1],
                            start=(s == 0), stop=(s == SE - 1), perf_mode=DRM,
                            skip_group_check=True,
                        )
                if m % 2:
                    nc.scalar.activation(
                        q8f[:, m, :], ps[:], AF.Identity,
                        bias=bq[:, m:m + 1], scale=1.0 / QKS,
                    )
                else:
                    nc.vector.tensor_scalar(
                        q8f[:, m, :], ps[:], 1.0 / QKS, bq[:, m:m + 1],
                        op0=ALU.mult, op1=ALU.add,
                    )
            # pair layout: head h=2m+half at partitions [64*half, 64*half+32),
            # (p, j) <-> d-row 32*j + p.  4 plain partition-slice DMAs.
            for half in range(2):
                for jj in range(2):
                    r0 = 64 * half + 32 * jj
                    nc.sync.dma_start(
                        q8p[64 * half:64 * half + 32, :, jj, :],
                        q8f[r0:r0 + 32, :, :],
                    )
            return q8p

        def proj_kv(l):
            """K (with pair-rearrange) and V, interleaved per E-chunk."""
            wk_s = dr(crit[l][:, C_WK:C_WV], SE)
            wv_s = dr(crit[l][:, C_WV:C_BQ], SE)
            k8f = big.tile([P, EC, NKV], FP8, name="k8f", tag="k8f")
            k8p = big.tile([P, EC, 2, NKV], FP8, name="k8p", tag="k8p")
            for m in range(EC):
                for n in range(KN):
                    ps = ss_pool.tile([P, E], F32, name="pp", tag="ss")
                    for s in range(SE):
                        nc.tensor.matmul(
                            ps[:], wk_s[:, s, :, m * P:(m + 1) * P],
                            hkvT8[:, 2 * s:2 * s + 2, n * 512:(n + 1) * 512],
                            start=(s == 0), stop=(s == SE - 1), perf_mode=DRM,
                        )
                    if n % 2:
                        nc.scalar.mul(k8f[:, m, n * 512:(n + 1) * 512], ps[:], 1.0 / QKS)
                    else:
                        nc.vector.tensor_scalar_mul(
                            k8f[:, m, n * 512:(n + 1) * 512], ps[:], 1.0 / QKS
                        )
                for half in range(2):
                    for jj in range(2):
                        r0 = 64 * half + 32 * jj
                        nc.sync.dma_start(
                            k8p[64 * half:64 * half + 32, m, jj, :],
                            k8f[r0:r0 + 32, m, :],
                        )
                for mv in range(4 * m, 4 * m + 4):
                    ps = ss_pool.tile([P, E], F32, name="pp", tag="ss")
                    for s in range(SE):
                        nc.tensor.matmul(
                            ps[:], hkvT8[:, 2 * s:2 * s + 2, mv * P:(mv + 1) * P],
                            wv_s[:, s, :, :],
                            start=(s == 0), stop=(s == SE - 1), perf_mode=DRM,
                        )
                    dst = va[mv // 2][:, mv % 2, :, 0:DH]
                    src = ps[:].rearrange("p (h d) -> p h d", h=H)
                    if mv % 4 == 3:
                        nc.vector.tensor_scalar_mul(dst, src, 1.0 / QKS)
                    else:
                        nc.scalar.mul(dst, src, 1.0 / QKS)
            return k8p

        # ---- layer 0 Q/K/V ----
        q8p = proj_q(0, hqT8_l0[:])
        k8p = proj_kv(0)

        for l in range(L):
            # ---- attention, head-pairs (fp8 DR scores) ----
            aoT = work.tile([P, EC, NQ], FP8, name="aoT", tag="aoT")
            for m in range(EC):          # pair (h0, h1) = (2m, 2m+1)
                # bulk weight loads, issued mid-attention so their
                # transfers never block latency-critical DMAs.
                if l == 0 and m == 1:
                    restA[0] = load_pack(0, "restA", A_END)
                    rows[0] = load_rows(0)
                if l == 0 and m == 2:
                    crit[1] = load_pack(1, "crit", C_END)
                if l == 0 and m == 3:
                    restB[0] = load_pack(0, "restB", B_END)
                kst = [k8p[0:32, m, :, :], k8p[64:96, m, :, :]]
                qmv = [q8p[0:32, m, :, :], q8p[64:96, m, :, :]]
                pso = [
                    att_pool.tile([P, E], F32, name="ps_oT", tag="att")
                    for _ in range(2)
                ]
                exs = [[], []]
                for g in range(GK):
                    for j in range(2):
                        ex = ex_pool.tile([P, 2, NQ], FP8, name="ex", tag="ex")
                        for sub in range(2):
                            ps_s = ss_pool.tile([P, NQ], F32, name="ps_s", tag="ss")
                            c0 = (2 * g + sub) * P
                            nc.tensor.matmul(
                                ps_s[:], kst[j][:, :, c0:c0 + P], qmv[j],
                                start=True, stop=True, perf_mode=DRM,
                            )
                            pat = EXP_PAT if m % 2 == 0 else EXP_PAT_ODD
                            if pat[j][g] == "A":
                                nc.scalar.activation(
                                    ex[:, sub, :].bitcast(I8), ps_s[:],
                                    AF.Identity, bias=schb_col[:], scale=SCH_A,
                                )
                            else:
                                nc.vector.tensor_scalar(
                                    ex[:, sub, :].bitcast(I8), ps_s[:],
                                    SCH_A, SCH_B, op0=ALU.mult, op1=ALU.add,
                                )
                        exs[j].append(ex)
                    if g >= 1:
                        for j in range(2):
                            nc.tensor.matmul(
                                pso[j][:], va[g - 1][:, :, 2 * m + j, :],
                                exs[j][g - 1][:],
                                start=(g == 1), stop=False, perf_mode=DRM,
                            )
                for j in range(2):
                    nc.tensor.matmul(
                        pso[j][:], va[GK - 1][:, :, 2 * m + j, :],
                        exs[j][GK - 1][:],
                        start=False, stop=True, perf_mode=DRM,
                    )
                    # normalize: aoT = (x8 unnorm) * rcp(sum/8) = x64 attnout
                    rcp = stats_pool.tile([DH, NQ], BF16, name="rcp", bufs=2)
                    with nc.allow_low_precision(reason="bf16 softmax denom"):
                        nc.vector.reciprocal(rcp[:], pso[j][DH:P, :])
                    nc.vector.tensor_tensor(
                        aoT[64 * j:64 * j + 64, m, :], pso[j][0:DH, :],
                        rcp[:], op=ALU.mult,
                    )

            # ---- out-proj (DR fp8) + bo row + residual; LN2 interleaved ----
            wo_s = dr(restA[l][:, A_WO:A_W1], SE)
            h2T8 = work.tile([P, EC, NQ], FP8, name="h2T8", tag="actT")
            h2r8 = (work.tile([P, EC, NQ], FP8, name="h2r8", tag="h2r8")
                    if H2R else None)
            hn2 = ln_hn2()
            for qc in range(QC):
                ps = ss_pool.tile([P, E], F32, name="pp", tag="ss")
                nc.tensor.matmul(
                    ps[:], ones1[:], rows[l][:, 0, :], start=True, stop=False,
                    skip_group_check=True,
                )
                for s in range(SE):
                    nc.tensor.matmul(
                        ps[:], aoT[:, 2 * s:2 * s + 2, qc * P:(qc + 1) * P],
                        wo_s[:, s, :, :],
                        start=False, stop=(s == SE - 1), perf_mode=DRM,
                        skip_group_check=True,
                    )
                nc.vector.scalar_tensor_tensor(
                    xq[qc], ps[:], 1.0 / (AOS * WS), xq[qc],
                    op0=ALU.mult, op1=ALU.add,
                )
                ln_q_tile(qc, hn2, eng="D")
                if qc % 2 == 1:
                    ln_txn(hn2, lnp, h2T8, (qc - 1) * P, resT8=h2r8,
                           eng="D", nu=2)
                    if qc == 1:
                        hn2 = ln_hn2()
            if l + 1 < L:
                restA[1] = load_pack(1, "restA", A_END)
                rows[1] = load_rows(1)

            # ---- hoisted K/V of layer l+1 (depend only on hkvT): emitted
            #      here so the PE fills the LN2-chain gap before FFN1. ----
            if l + 1 < L:
                k8p = proj_kv(l + 1)

            # ---- FFN1 (DR fp8): g^T = gelu(w1^T @ h2^T / 64 + b1) ----
            w1_s = dr(restA[l][:, A_W1:A_W1R], SE)
            w1r_s = dr(restA[l][:, A_W1R:A_END], SE)
            b1 = restB[l][:, B_B1:B_END].bitcast(F32)
            gT8 = big.tile([P, MC, NQ], FP8, name="gT8", tag="gT8")
            for m in range(MC):
                ps = ss_pool.tile([P, E], F32, name="pp", tag="ss")
                for half in range(2):
                    c0, c1 = half * 256, half * 256 + 256
                    for s in range(SE):
                        nc.tensor.matmul(
                            ps[:, c0:c1], w1_s[:, s, :, m * P:(m + 1) * P],
                            h2T8[:, 2 * s:2 * s + 2, c0:c1],
                            start=(s == 0), stop=False, perf_mode=DRM,
                            skip_group_check=True,
                        )
                    for s in range(SE):
                        nc.tensor.matmul(
                            ps[:, c0:c1], w1r_s[:, s, :, m * P:(m + 1) * P],
                            h2T8[:, 2 * s:2 * s + 2, c0:c1],
                            start=False, stop=(not H2R and s == SE - 1),
                            perf_mode=DRM, skip_group_check=True,
                        )
                    if H2R:
                        for s in range(SE):
                            nc.tensor.matmul(
                                ps[:, c0:c1], w1_s[:, s, :, m * P:(m + 1) * P],
                                h2r8[:, 2 * s:2 * s + 2, c0:c1],
                                start=False, stop=(s == SE - 1), perf_mode=DRM,
                                skip_group_check=True,
                            )
                nc.scalar.activation(
                    gT8[:, m, :], ps[:], AF.Gelu,
                    bias=b1[:, m:m + 1], scale=1.0 / WS,
                )

            # ---- FFN2 (DR fp8) + b2 row + residual; next LN1 interleaved ----
            w2_s = dr(restB[l][:, B_W2:B_W2R], SM)
            w2r_s = dr(restB[l][:, B_W2R:B_B1], SM)
            if l + 1 < L:
                hqT8 = work.tile([P, EC, NQ], FP8, name="hqT8", tag="actT2")
                hn2 = ln_hn2()
            for qc in range(QC):
                ps = ss_pool.tile([P, E], F32, name="pp", tag="ss")
                nc.tensor.matmul(
                    ps[:], ones1[:], rows[l][:, 1, :], start=True, stop=False,
                    skip_group_check=True,
                )
                for s in range(SM):
                    nc.tensor.matmul(
                        ps[:], gT8[:, 2 * s:2 * s + 2, qc * P:(qc + 1) * P],
                        w2_s[:, s, :, :],
                        start=False, stop=False, perf_mode=DRM,
                        skip_group_check=True,
                    )
                for s in range(SM):
                    nc.tensor.matmul(
                        ps[:], gT8[:, 2 * s:2 * s + 2, qc * P:(qc + 1) * P],
                        w2r_s[:, s, :, :],
                        start=False, stop=(s == SM - 1), perf_mode=DRM,
                        skip_group_check=True,
                    )
                nc.vector.scalar_tensor_tensor(
                    xq[qc], ps[:], 1.0 / WS, xq[qc], op0=ALU.mult, op1=ALU.add,
                )
                if l + 1 < L:
                    ln_q_tile(qc, hn2, eng="D")
                    if qc % 2 == 1:
                        ln_txn(hn2, lnp, hqT8, (qc - 1) * P, eng="D", nu=2)
                        if qc == 1:
                            hn2 = ln_hn2()
                else:
                    nc.sync.dma_start(y_d[qc * P:(qc + 1) * P, :], xq[qc])
            if l + 1 < L:
                restB[1] = load_pack(1, "restB", B_END)
                q8p = proj_q(l + 1, hqT8[:])

    nc.compile()
    return nc


def get_nc():
    if "nc" not in _CACHE:
        _CACHE["nc"] = _build()
    return _CACHE["nc"]


def _fp8(a):
    return np.clip(np.asarray(a, np.float32), -240.0, 240.0).astype(
        ml_dtypes.float8_e4m3
    )


def _bf16(a):
    return np.asarray(a, np.float32).astype(ml_dtypes.bfloat16)


def _rearr_dr(w8, S):
    """[S*2*128, C] (row-major contraction) -> [128, S*2*C] DR pair layout."""
    C = w8.shape[1]
    return np.ascontiguousarray(
        w8.reshape(S, 2, P, C).transpose(2, 0, 1, 3).reshape(P, S * 2 * C)
    )


def _cols(v):
    """[k*128] -> [128, k]: column m holds v[m*128:(m+1)*128]."""
    k = v.shape[0] // P
    return np.ascontiguousarray(np.asarray(v, np.float32).reshape(k, P).T)


def _u8(a):
    return np.ascontiguousarray(a).view(np.uint8)


def kernel(**inputs) -> np.ndarray:
    x_q = np.asarray(inputs["x_q"], np.float32)
    x_kv = np.asarray(inputs["x_kv"], np.float32)
    wq = np.asarray(inputs["wq"], np.float32)
    wkv = np.asarray(inputs["wkv"], np.float32)
    wo = np.asarray(inputs["wo"], np.float32)
    bo = np.asarray(inputs["bo"], np.float32)
    w1 = np.asarray(inputs["w1"], np.float32)
    b1 = np.asarray(inputs["b1"], np.float32)
    w2 = np.asarray(inputs["w2"], np.float32)
    b2 = np.asarray(inputs["b2"], np.float32)
    ln1_g = np.asarray(inputs["ln1_g"], np.float32)
    ln1_b = np.asarray(inputs["ln1_b"], np.float32)
    ln2_g = np.asarray(inputs["ln2_g"], np.float32)
    ln2_b = np.asarray(inputs["ln2_b"], np.float32)

    shared = {}
    for l in range(L):
        wk_f = wkv[l][:, :E]
        wv_f = wkv[l][:, E:]
        wq_eff = ln1_g[l][:, None] * wq[l]
        wk_eff = ln1_g[l][:, None] * wk_f
        wv_eff = ln1_g[l][:, None] * wv_f
        bq_eff = ln1_b[l] @ wq[l]
        bv_eff = ln1_b[l] @ wv_f
        bo_eff = bo[l] + bv_eff @ wo[l]
        w1_eff = ln2_g[l][:, None] * w1[l]
        b1_eff = ln2_b[l] @ w1[l] + b1[l]

        wq8 = _rearr_dr(_fp8(WS * wq_eff), SE)
        wk8 = _rearr_dr(_fp8(WS * wk_eff), SE)
        wv8 = _rearr_dr(_fp8(WS * wv_eff), SE)
        wo8 = _rearr_dr(_fp8(WS * wo[l]), SE)
        w18 = _rearr_dr(_fp8(WS * w1_eff), SE)
        w28 = _rearr_dr(_fp8(WS * w2[l]), SM)
        w1r8 = _rearr_dr(_fp8(WS * w1_eff - _fp8(WS * w1_eff).astype(np.float32)), SE)
        w2r8 = _rearr_dr(_fp8(WS * w2[l] - _fp8(WS * w2[l]).astype(np.float32)), SM)
        bq_c = _cols(QKS * bq_eff)      # [128, 4] f32
        b1_c = _cols(b1_eff)            # [128, 16] f32

        crit = np.concatenate(
            [_u8(wq8), _u8(wk8), _u8(wv8), _u8(bq_c)], axis=1)
        restA = np.concatenate([_u8(wo8), _u8(w18), _u8(w1r8)], axis=1)
        restB = np.concatenate([_u8(w28), _u8(w2r8), _u8(b1_c)], axis=1)
        rows2 = np.stack(
            [_bf16(AOS * WS * bo_eff), _bf16(WS * b2[l])], axis=0)[None]
        shared.update({
            f"crit_{l}": crit.view(ml_dtypes.float8_e4m3),
            f"restA_{l}": restA.view(ml_dtypes.float8_e4m3),
            f"restB_{l}": restB.view(ml_dtypes.float8_e4m3),
            f"rows_{l}": rows2,
        })

    in_maps = []
    for c in range(8):
        b, qc = c // 4, c % 4
        m = dict(shared)
        m["xq"] = np.ascontiguousarray(x_q[b, qc * NQ:(qc + 1) * NQ, :])
        m["xkv"] = np.ascontiguousarray(_bf16(x_kv[b]))
        in_maps.append(m)

    nc = get_nc()
    res = bass_utils.run_bass_kernel_spmd(nc, in_maps, core_ids=list(range(8)))

    out = np.empty((2, 2048, E), np.float32)
    for c in range(8):
        b, qc = c // 4, c % 4
        out[b, qc * NQ:(qc + 1) * NQ, :] = res.results[c]["y"]
    return out


# revision 76
# speedup vs baseline: 1.2297x; 1.0150x over previous
"""Trainium2 Bass kernel for a 2-layer cross-encoder (CrossEncoder).

Model: B=2, NQ=NKV=2048, E=512, H=8 (d_head=64), MLP=2048, depth=2, fp32 I/O.

Sharding (8 cores, no collectives): core c handles batch b=c//4 and query
rows [qc*512, (qc+1)*512) with qc=c%4.  Each core computes the full KV
projections for its batch so every core produces its output slice
independently.

Key structure (v4):
 - All heavy matmuls fp8e4m3 + DoubleRow (0.5 cyc/row), including the
   scores matmul (q/k stored as fp8 DR pair tiles, produced by a cheap
   SBUF->SBUF DMA partition-rearrange; head h sits at base partition
   0/64 of its E-chunk pair tile so the PE base-partition rule holds).
 - Attention runs head-PAIRS interleaved: the two heads' exp chains use
   opposite engines per group, so Act and DVE both stay fed and the
   scores->exp->attnV sem latency is hidden.
 - LayerNorm activations transposed by the DMA XBAR (dma_start_transpose)
   instead of PE identity matmuls + PSUM round trips; fp8 quantize runs
   from SBUF (2x/4x DVE modes).  rstd = 1/sqrt(var+eps) via bit trick +
   1 Newton step on Pool (batched for the kv setup).
 - exp() is Schraudolph-only (int8 round writes fp8e4m3 bits directly),
   split Act (Identity activation) / DVE; Act's only table is Gelu.
 - Softmax denominator comes free from 64 constant columns per head in V.
 - x_kv is uploaded bf16; all weights of a layer arrive as 3 packed DMAs
   staged so bulk transfers never sit in front of latency-critical XBAR
   or pair-rearrange DMAs in the shared DMA queue.
 - K/V of layer l+1 (which depend only on hkvT) are hoisted between
   FFN1(l) and FFN2(l) to fill idle engines there; LN2 interleaves with
   the O-proj residuals, next-layer LN1 with the FFN2 residuals.
"""

import numpy as np
import ml_dtypes

import concourse.bass as bass
import concourse.bacc as bacc
import concourse.mybir as mybir
import concourse.tile as tile
from concourse import bass_utils
from contextlib import ExitStack

P = 128
E = 512
EC = E // P        # 4 chunks of the embedding dim
SE = EC // 2       # 2 DoubleRow super-chunks
NQ = 512           # query rows per core
QC = NQ // P       # 4 query chunks
NKV = 2048
KC = NKV // P      # 16 key chunks of 128
KN = NKV // 512    # 4 key chunks of 512
GK = KC // 2       # 8 key pair-groups
H = 8
DH = 64
MLP = 2048
MC = MLP // P      # 16 mlp chunks of 128
SM = MC // 2       # 8 DoubleRow super-chunks
L = 2
LN_EPS = 1e-5
F32 = mybir.dt.float32
I32 = mybir.dt.int32
BF16 = mybir.dt.bfloat16
FP8 = mybir.dt.float8e4
I8 = mybir.dt.int8
AF = mybir.ActivationFunctionType
ALU = mybir.AluOpType
DRM = mybir.MatmulPerfMode.DoubleRow

WS = 64.0                       # fp8 weight pre-scale (host side)
QKS = 8.0                       # q/k storage scale
SCALE = DH ** -0.5
EXPS = SCALE / (QKS * QKS)      # exp scale applied to scores psum (=1/512)
AOS = 64.0                      # attnout storage scale (fp8 subnormal guard)
SCH_A = (8.0 / np.log(2.0)) * EXPS   # Schraudolph slope for fp8e4 bits
SCH_B = 56.0 - 0.47                  # fp8e4 exponent bias term - rms shift
RSQRT_MAGIC = 0x5F3759DF
# per-(head-in-pair, group) exp engine: A=Act(Identity act) D=DVE.
# Anti-aligned so the two heads of a pair use opposite engines; 9A/7D
# because DVE also owns the softmax divide.
EXP_PAT = ["AADADADA", "ADADADAA"]
EXP_PAT_ODD = ["AADADADA", "DDADADAA"]   # 9A/7D for odd pairs (balance)
H2R = False                              # FFN1 activation-residual pass
W2R = False                              # FFN2 weight-residual pass

# packed-weight byte offsets (per partition)
C_WQ, C_WK, C_WV, C_BQ, C_END = 0, 2048, 4096, 6144, 6160
A_WO, A_W1, A_W1R, A_END = 0, 2048, 10240, 18432
B_W2, B_W2R, B_B1, B_END = 0, 8192, 16384, 16448

_CACHE = {}


def _build():
    """Build the per-core Bass program (identical on all 8 cores)."""
    nc = bacc.Bacc("TRN2", target_bir_lowering=False, debug=False, num_devices=8)

    xq_d = nc.dram_tensor("xq", [NQ, E], F32, kind="ExternalInput").ap()
    xkv_d = nc.dram_tensor("xkv", [NKV, E], BF16, kind="ExternalInput").ap()
    wd = []
    for l in range(L):
        wd.append({
            "crit": nc.dram_tensor(f"crit_{l}", [P, C_END], FP8, kind="ExternalInput").ap(),
            "restA": nc.dram_tensor(f"restA_{l}", [P, A_END], FP8, kind="ExternalInput").ap(),
            "restB": nc.dram_tensor(f"restB_{l}", [P, B_END], FP8, kind="ExternalInput").ap(),
            "rows": nc.dram_tensor(f"rows_{l}", [1, 2, E], BF16, kind="ExternalInput").ap(),
        })
    y_d = nc.dram_tensor("y", [NQ, E], F32, kind="ExternalOutput").ap()

    with tile.TileContext(nc) as tc, ExitStack() as ctx:
        const_pool = ctx.enter_context(tc.tile_pool(name="const", bufs=1))
        ones1 = const_pool.tile([1, P], BF16)
        nc.gpsimd.memset(ones1[:], 1.0)
        schb_col = const_pool.tile([P, 1], F32)
        nc.gpsimd.memset(schb_col[:], SCH_B)

        stats_pool = ctx.enter_context(tc.tile_pool(name="stats", bufs=12))

        def rsqrt_chain(var_ap, rstd_ap, w):
            """rstd = 1/sqrt(var+eps) on Pool via bit trick + 1 Newton step."""
            ve = stats_pool.tile([P, 4], F32, name="ve")[:, :w]
            nc.gpsimd.tensor_scalar_add(ve, var_ap, LN_EPS)
            y0 = stats_pool.tile([P, 4], F32, name="y0")[:, :w]
            # int bit-trick ops run on DVE (Pool's Q7 rejects int shifts)
            nc.vector.tensor_scalar(
                y0.bitcast(I32), ve.bitcast(I32), 1, 0,
                op0=ALU.logical_shift_right, op1=ALU.bypass,
            )
            nc.vector.tensor_scalar(
                y0.bitcast(I32), y0.bitcast(I32), -1, RSQRT_MAGIC,
                op0=ALU.mult, op1=ALU.add,
            )
            t = stats_pool.tile([P, 4], F32, name="t")[:, :w]
            nc.gpsimd.tensor_tensor(t, y0, y0, op=ALU.mult)
            nc.gpsimd.tensor_tensor(t, t, ve, op=ALU.mult)
            nc.gpsimd.tensor_scalar(t, t, -0.5, 1.5, op0=ALU.mult, op1=ALU.add)
            nc.gpsimd.tensor_tensor(rstd_ap, y0, t, op=ALU.mult)

        def ln_stats(x_ap, mv_ap):
            bnst = stats_pool.tile([P, 6], F32, name="bnst")
            nc.vector.bn_stats(bnst[:], x_ap)
            nc.vector.bn_aggr(mv_ap, bnst[:])

        # norm for one [128, E] tile into slot u of a 4-wide hn4 buffer.
        def ln_norm(src_ap, mean_ap, rstd_ap, hn4, u, eng="D"):
            if eng == "P":
                nc.gpsimd.tensor_scalar(
                    hn4[:, u, :], src_ap, mean_ap, rstd_ap,
                    op0=ALU.subtract, op1=ALU.mult,
                )
            else:
                nc.vector.tensor_scalar(
                    hn4[:, u, :], src_ap, mean_ap, rstd_ap,
                    op0=ALU.subtract, op1=ALU.mult,
                )

        # one XBAR transpose + one quantize for an nu-tile hn batch
        # (nu*128 tokens).  dst slice [:, :, tok0:tok0+nu*128].
        def ln_txn(hn, t_pool, dstT8, tok0, resT8=None, eng="D", nu=4):
            ht = t_pool.tile([P, 4 * EC, P], BF16, name="ht", tag="ht",
                             bufs=2)[:, :nu * EC, :]
            nc.sync.dma_start_transpose(
                ht, hn[:].rearrange("p u c -> p (u c)")
            )
            dst = dstT8[:, :, tok0:tok0 + nu * P].rearrange(
                "p c (u t) -> p u c t", u=nu
            )
            src = ht.rearrange("p (u c) t -> p u c t", u=nu)
            if eng == "P":
                nc.gpsimd.tensor_copy(dst, src)
            elif eng == "D":
                nc.vector.tensor_copy(dst, src)
            else:
                nc.scalar.copy(dst, src)
            if resT8 is not None:
                # STT requires <=3D APs: one op per 128-token sub-block.
                for u in range(nu):
                    t0 = tok0 + u * P
                    nc.vector.scalar_tensor_tensor(
                        resT8[:, :, t0:t0 + P], dstT8[:, :, t0:t0 + P], -1.0,
                        ht[:, u * EC:(u + 1) * EC, :], op0=ALU.mult, op1=ALU.add,
                    )

        # Residual stream: one [128, 4, 512] fp32 tile (qc-major subtiles).
        xq_pool = ctx.enter_context(tc.tile_pool(name="xq", bufs=1))
        xqb = xq_pool.tile([P, QC, E], F32, name="xqb", tag="xqb")
        nc.sync.dma_start(xqb[:], xq_d.rearrange("(a p) c -> p a c", p=P))
        xq = [xqb[:, qc, :] for qc in range(QC)]

        # hkv^T fp8 DR tile [128, 4, NKV]: (c, p) holds E-row c*128+p.
        hkvT_pool = ctx.enter_context(tc.tile_pool(name="hkvT", bufs=1))
        hkvT8 = hkvT_pool.tile([P, EC, NKV], FP8, name="hkvT8", tag="hkvT8")

        # PSUM pools (8 banks): shared 1-bank ring 6 + att 2 = 8.
        ss_pool = ctx.enter_context(tc.tile_pool(name="ss", bufs=6, space="PSUM"))
        att_pool = ctx.enter_context(tc.tile_pool(name="attp", bufs=2, space="PSUM"))

        wpool = ctx.enter_context(tc.tile_pool(name="w", bufs=1))

        def load_pack(l, which, sz):
            t = wpool.tile([P, sz], FP8, name=f"{which}_{l}")
            nc.sync.dma_start(t[:], wd[l][which])
            return t

        def load_rows(l):
            t = wpool.tile([1, 2, E], BF16, name=f"rows_{l}")
            nc.sync.dma_start(t[:], wd[l]["rows"])
            return t

        def dr(ap, S):
            return ap.rearrange("p (s j c) -> p s j c", s=S, j=2)

        work = ctx.enter_context(tc.tile_pool(name="work", bufs=1))
        big = ctx.enter_context(tc.tile_pool(name="big", bufs=1))
        ex_pool = ctx.enter_context(tc.tile_pool(name="ex", bufs=5))
        lnp = ctx.enter_context(tc.tile_pool(name="lnp", bufs=4))

        def ln_hn2():
            return lnp.tile([P, 2, E], BF16, name="hn2", tag="hn2", bufs=3)

        # q-side LN: per-tile stats+norm into hn2 slot qc%2; a following
        # ln_txn(nu=2) finishes each half.
        def ln_q_tile(qc, hn2, eng="D"):
            mv = stats_pool.tile([P, 2], F32, name="mv")
            ln_stats(xq[qc], mv[:])
            rstd = stats_pool.tile([P, 1], F32, name="rstd")
            rsqrt_chain(mv[:, 1:2], rstd[:], 1)
            ln_norm(xq[qc], mv[:, 0:1], rstd[:], hn2, qc % 2, eng=eng)

        crit = [None, None]
        with tc.tile_pool(name="kvln", bufs=4) as kvln_pool:
            # x_kv bf16, 4 chunks of [128, 4, 512] (token-block subtiles),
            # ring of 3: chunk a+3 reuses chunk a's space.
            xkvb = []

            def load_xkvb(a):
                t = kvln_pool.tile([P, 4, E], BF16, name=f"xkvb{a}",
                                   tag="xkvb", bufs=3)
                nc.sync.dma_start(
                    t[:], xkv_d[a * 512:(a + 1) * 512, :].rearrange(
                        "(u p) c -> p u c", p=P)
                )
                xkvb.append(t)

            load_xkvb(0)
            load_xkvb(1)
            crit[0] = load_pack(0, "crit", C_END)
            load_xkvb(2)
            load_xkvb(3)

            # ---- LN1(x_q, layer 0), two halves ----
            hqT8_l0 = work.tile([P, EC, NQ], FP8, name="hqT8_l0", tag="actT")
            for half in range(2):
                hn2 = ln_hn2()
                for qc in (2 * half, 2 * half + 1):
                    ln_q_tile(qc, hn2, eng="D")
                ln_txn(hn2, lnp, hqT8_l0, half * 256, eng="D", nu=2)

            # ---- startup: x_kv LN + XBAR transpose + fp8 quantize.
            #      Stats run one batch ahead of the norm/xbar chains so
            #      DVE's queue never blocks the next batch's stats. ----
            mv4s, rstd4s = {}, {}

            def kv_stats(b):
                mv4 = stats_pool.tile([P, 2, 4], F32, name="mv4", bufs=4)
                for u in range(4):
                    ln_stats(xkvb[b][:, u, :], mv4[:, :, u])
                rstd4 = stats_pool.tile([P, 4], F32, name="rstd4", bufs=4)
                rsqrt_chain(mv4[:, 1, :], rstd4[:], 4)
                mv4s[b], rstd4s[b] = mv4, rstd4

            def kv_finish(b):
                khn4 = kvln_pool.tile([P, 4, E], BF16, name="khn4",
                                      tag="khn4", bufs=2)
                for u in range(4):
                    ln_norm(xkvb[b][:, u, :], mv4s[b][:, 0, u:u + 1],
                            rstd4s[b][:, u:u + 1], khn4, u, eng="DDAD"[u])
                ln_txn(khn4, lnp, hkvT8, b * 512, eng="DADA"[b], nu=4)

            kv_stats(0)
            kv_stats(1)
            kv_finish(0)
            kv_stats(2)
            kv_finish(1)
            kv_stats(3)
            kv_finish(2)
            kv_finish(3)

        restA = [None, None]
        restB = [None, None]
        rows = [None, None]

        # va pair tiles persist across layers (v columns overwritten per
        # layer, the constant denominator columns are set once here).
        va = [
            big.tile([P, 2, H, 2 * DH], FP8, name=f"va{g}", tag=f"va{g}")
            for g in range(GK)
        ]
        for g in range(GK):
            nc.gpsimd.memset(va[g][:, :, :, DH:2 * DH], 1.0 / QKS)

        def proj_q(l, hqT8_ap):
            """Q projection + pair-rearrange for layer l."""
            wq_s = dr(crit[l][:, C_WQ:C_WK], SE)
            bq = crit[l][:, C_BQ:C_END].bitcast(F32)
            q8f = work.tile([P, EC, NQ], FP8, name="q8f", tag="q8f")
            q8p = work.tile([P, EC, 2, NQ], FP8, name="q8p", tag="q8p")
            for m in range(EC):
                ps = ss_pool.tile([P, E], F32, name="pp", tag="ss")
                for half in range(2):
                    c0, c1 = half * 256, half * 256 + 256
                    for s in range(SE):
                        nc.tensor.matmul(
                            ps[:, c0:c1], wq_s[:, s, :, m * P:(m + 1) * P],
                            hqT8_ap[:, 2 * s:2 * s + 2, c0:c1],
                            start=(s == 0), stop=(s == SE - 1), perf_mode=DRM,
                            skip_group_check=True,
                        )
                if m % 2:
                    nc.scalar.activation(
                        q8f[:, m, :], ps[:], AF.Identity,
                        bias=bq[:, m:m + 1], scale=1.0 / QKS,
                    )
                else:
                    nc.vector.tensor_scalar(
                        q8f[:, m, :], ps[:], 1.0 / QKS, bq[:, m:m + 1],
                        op0=ALU.mult, op1=ALU.add,
                    )
            # pair layout: head h=2m+half at partitions [64*half, 64*half+32),
            # (p, j) <-> d-row 32*j + p.  4 plain partition-slice DMAs.
            for half in range(2):
                for jj in range(2):
                    r0 = 64 * half + 32 * jj
                    nc.sync.dma_start(
                        q8p[64 * half:64 * half + 32, :, jj, :],
                        q8f[r0:r0 + 32, :, :],
                    )
            return q8p

        def proj_kv(l):
            """K (with pair-rearrange) and V, interleaved per E-chunk."""
            wk_s = dr(crit[l][:, C_WK:C_WV], SE)
            wv_s = dr(crit[l][:, C_WV:C_BQ], SE)
            k8f = big.tile([P, EC, NKV], FP8, name="k8f", tag="k8f")
            k8p = big.tile([P, EC, 2, NKV], FP8, name="k8p", tag="k8p")
            for m in range(EC):
                for n in range(KN):
                    ps = ss_pool.tile([P, E], F32, name="pp", tag="ss")
                    for s in range(SE):
                        nc.tensor.matmul(
                            ps[:], wk_s[:, s, :, m * P:(m + 1) * P],
                            hkvT8[:, 2 * s:2 * s + 2, n * 512:(n + 1) * 512],
                            start=(s == 0), stop=(s == SE - 1), perf_mode=DRM,
                        )
                    if n % 2:
                        nc.scalar.mul(k8f[:, m, n * 512:(n + 1) * 512], ps[:], 1.0 / QKS)
                    else:
                        nc.vector.tensor_scalar_mul(
                            k8f[:, m, n * 512:(n + 1) * 512], ps[:], 1.0 / QKS
                        )
                for half in range(2):
                    for jj in range(2):
                        r0 = 64 * half + 32 * jj
                        nc.sync.dma_start(
                            k8p[64 * half:64 * half + 32, m, jj, :],
                            k8f[r0:r0 + 32, m, :],
                        )
                for mv in range(4 * m, 4 * m + 4):
                    ps = ss_pool.tile([P, E], F32, name="pp", tag="ss")
                    for s in range(SE):
                        nc.tensor.matmul(
                            ps[:], hkvT8[:, 2 * s:2 * s + 2, mv * P:(mv + 1) * P],
                            wv_s[:, s, :, :],
                            start=(s == 0), stop=(s == SE - 1), perf_mode=DRM,
                        )
                    dst = va[mv // 2][:, mv % 2, :, 0:DH]
                    src = ps[:].rearrange("p (h d) -> p h d", h=H)
                    if mv % 4 == 3:
                        nc.vector.tensor_scalar_mul(dst, src, 1.0 / QKS)
                    else:
                        nc.scalar.mul(dst, src, 1.0 / QKS)
            return k8p

        # ---- layer 0 Q/K/V ----
        q8p = proj_q(0, hqT8_l0[:])
        k8p = proj_kv(0)

        for l in range(L):
            # ---- attention, head-pairs (fp8 DR scores) ----
            aoT = work.tile([P, EC, NQ], FP8, name="aoT", tag="aoT")
            for m in range(EC):          # pair (h0, h1) = (2m, 2m+1)
                # bulk weight loads, issued mid-attention so their
                # transfers never block latency-critical DMAs.
                if l == 0 and m == 1:
                    restA[0] = load_pack(0, "restA", A_END)
                    rows[0] = load_rows(0)
                if l == 0 and m == 2:
                    crit[1] = load_pack(1, "crit", C_END)
                if l == 0 and m == 3:
                    restB[0] = load_pack(0, "restB", B_END)
                kst = [k8p[0:32, m, :, :], k8p[64:96, m, :, :]]
                qmv = [q8p[0:32, m, :, :], q8p[64:96, m, :, :]]
                pso = [
                    att_pool.tile([P, E], F32, name="ps_oT", tag="att")
                    for _ in range(2)
                ]
                exs = [[], []]
                for g in range(GK):
                    for j in range(2):
                        ex = ex_pool.tile([P, 2, NQ], FP8, name="ex", tag="ex")
                        for sub in range(2):
                            ps_s = ss_pool.tile([P, NQ], F32, name="ps_s", tag="ss")
                            c0 = (2 * g + sub) * P
                            nc.tensor.matmul(
                                ps_s[:], kst[j][:, :, c0:c0 + P], qmv[j],
                                start=True, stop=True, perf_mode=DRM,
                            )
                            pat = EXP_PAT if m % 2 == 0 else EXP_PAT_ODD
                            if pat[j][g] == "A":
                                nc.scalar.activation(
                                    ex[:, sub, :].bitcast(I8), ps_s[:],
                                    AF.Identity, bias=schb_col[:], scale=SCH_A,
                                )
                            else:
                                nc.vector.tensor_scalar(
                                    ex[:, sub, :].bitcast(I8), ps_s[:],
                                    SCH_A, SCH_B, op0=ALU.mult, op1=ALU.add,
                                )
                        exs[j].append(ex)
                    if g >= 1:
                        for j in range(2):
                            nc.tensor.matmul(
                                pso[j][:], va[g - 1][:, :, 2 * m + j, :],
                                exs[j][g - 1][:],
                                start=(g == 1), stop=False, perf_mode=DRM,
                            )
                for j in range(2):
                    nc.tensor.matmul(
                        pso[j][:], va[GK - 1][:, :, 2 * m + j, :],
                        exs[j][GK - 1][:],
                        start=False, stop=True, perf_mode=DRM,
                    )
                    # normalize: aoT = (x8 unnorm) * rcp(sum/8) = x64 attnout
                    rcp = stats_pool.tile([DH, NQ], BF16, name="rcp", bufs=2)
                    with nc.allow_low_precision(reason="bf16 softmax denom"):
                        nc.vector.reciprocal(rcp[:], pso[j][DH:P, :])
                    nc.vector.tensor_tensor(
                        aoT[64 * j:64 * j + 64, m, :], pso[j][0:DH, :],
                        rcp[:], op=ALU.mult,
                    )

            # ---- out-proj (DR fp8) + bo row + residual; LN2 interleaved ----
            wo_s = dr(restA[l][:, A_WO:A_W1], SE)
            h2T8 = work.tile([P, EC, NQ], FP8, name="h2T8", tag="actT")
            h2r8 = (work.tile([P, EC, NQ], FP8, name="h2r8", tag="h2r8")
                    if H2R else None)
            hn2 = ln_hn2()
            for qc in range(QC):
                ps = ss_pool.tile([P, E], F32, name="pp", tag="ss")
                nc.tensor.matmul(
                    ps[:], ones1[:], rows[l][:, 0, :], start=True, stop=False,
                    skip_group_check=True,
                )
                for s in range(SE):
                    nc.tensor.matmul(
                        ps[:], aoT[:, 2 * s:2 * s + 2, qc * P:(qc + 1) * P],
                        wo_s[:, s, :, :],
                        start=False, stop=(s == SE - 1), perf_mode=DRM,
                        skip_group_check=True,
                    )
                nc.vector.scalar_tensor_tensor(
                    xq[qc], ps[:], 1.0 / (AOS * WS), xq[qc],
                    op0=ALU.mult, op1=ALU.add,
                )
                ln_q_tile(qc, hn2, eng="D")
                if qc % 2 == 1:
                    ln_txn(hn2, lnp, h2T8, (qc - 1) * P, resT8=h2r8,
                           eng="D", nu=2)
                    if qc == 1:
                        hn2 = ln_hn2()
            if l + 1 < L:
                restA[1] = load_pack(1, "restA", A_END)
                rows[1] = load_rows(1)

            # ---- hoisted K/V of layer l+1 (depend only on hkvT): emitted
            #      here so the PE fills the LN2-chain gap before FFN1. ----
            if l + 1 < L:
                k8p = proj_kv(l + 1)

            # ---- FFN1 (DR fp8): g^T = gelu(w1^T @ h2^T / 64 + b1) ----
            w1_s = dr(restA[l][:, A_W1:A_W1R], SE)
            w1r_s = dr(restA[l][:, A_W1R:A_END], SE)
            b1 = restB[l][:, B_B1:B_END].bitcast(F32)
            gT8 = big.tile([P, MC, NQ], FP8, name="gT8", tag="gT8")
            for m in range(MC):
                ps = ss_pool.tile([P, E], F32, name="pp", tag="ss")
                for half in range(2):
                    c0, c1 = half * 256, half * 256 + 256
                    for s in range(SE):
                        nc.tensor.matmul(
                            ps[:, c0:c1], w1_s[:, s, :, m * P:(m + 1) * P],
                            h2T8[:, 2 * s:2 * s + 2, c0:c1],
                            start=(s == 0), stop=False, perf_mode=DRM,
                            skip_group_check=True,
                        )
                    for s in range(SE):
                        nc.tensor.matmul(
                            ps[:, c0:c1], w1r_s[:, s, :, m * P:(m + 1) * P],
                            h2T8[:, 2 * s:2 * s + 2, c0:c1],
                            start=False, stop=(not H2R and s == SE - 1),
                            perf_mode=DRM, skip_group_check=True,
                        )
                    if H2R:
                        for s in range(SE):
                            nc.tensor.matmul(
                                ps[:, c0:c1], w1_s[:, s, :, m * P:(m + 1) * P],
                                h2r8[:, 2 * s:2 * s + 2, c0:c1],
                                start=False, stop=(s == SE - 1), perf_mode=DRM,
                                skip_group_check=True,
                            )
                nc.scalar.activation(
                    gT8[:, m, :], ps[:], AF.Gelu,
                    bias=b1[:, m:m + 1], scale=1.0 / WS,
                )

            # ---- FFN2 (DR fp8) + b2 row + residual; next LN1 interleaved ----
            w2_s = dr(restB[l][:, B_W2:B_W2R], SM)
            w2r_s = dr(restB[l][:, B_W2R:B_B1], SM)
            if l + 1 < L:
                hqT8 = work.tile([P, EC, NQ], FP8, name="hqT8", tag="actT2")
                hn2 = ln_hn2()
            for qc in range(QC):
                ps = ss_pool.tile([P, E], F32, name="pp", tag="ss")
                nc.tensor.matmul(
                    ps[:], ones1[:], rows[l][:, 1, :], start=True, stop=False,
                    skip_group_check=True,
                )
                for s in range(SM):
                    nc.tensor.matmul(
                        ps[:], gT8[:, 2 * s:2 * s + 2, qc * P:(qc + 1) * P],
                        w2_s[:, s, :, :],
                        start=False, stop=(not W2R and s == SM - 1),
                        perf_mode=DRM, skip_group_check=True,
                    )
                if W2R:
                    for s in range(SM):
                        nc.tensor.matmul(
                            ps[:], gT8[:, 2 * s:2 * s + 2, qc * P:(qc + 1) * P],
                            w2r_s[:, s, :, :],
                            start=False, stop=(s == SM - 1), perf_mode=DRM,
                            skip_group_check=True,
                        )
                nc.vector.scalar_tensor_tensor(
                    xq[qc], ps[:], 1.0 / WS, xq[qc], op0=ALU.mult, op1=ALU.add,
                )
                if l + 1 < L:
                    ln_q_tile(qc, hn2, eng="D")
                    if qc % 2 == 1:
                        ln_txn(hn2, lnp, hqT8, (qc - 1) * P, eng="D", nu=2)
                        if qc == 1:
                            hn2 = ln_hn2()
                else:
                    nc.sync.dma_start(y_d[qc * P:(qc + 1) * P, :], xq[qc])
            if l + 1 < L:
                restB[1] = load_pack(1, "restB", B_END)
                q8p = proj_q(l + 1, hqT8[:])

    nc.compile()
    return nc


def get_nc():
    if "nc" not in _CACHE:
        _CACHE["nc"] = _build()
    return _CACHE["nc"]


def _fp8(a):
    return np.clip(np.asarray(a, np.float32), -240.0, 240.0).astype(
        ml_dtypes.float8_e4m3
    )


def _bf16(a):
    return np.asarray(a, np.float32).astype(ml_dtypes.bfloat16)


def _rearr_dr(w8, S):
    """[S*2*128, C] (row-major contraction) -> [128, S*2*C] DR pair layout."""
    C = w8.shape[1]
    return np.ascontiguousarray(
        w8.reshape(S, 2, P, C).transpose(2, 0, 1, 3).reshape(P, S * 2 * C)
    )


def _cols(v):
    """[k*128] -> [128, k]: column m holds v[m*128:(m+1)*128]."""
    k = v.shape[0] // P
    return np.ascontiguousarray(np.asarray(v, np.float32).reshape(k, P).T)


def _u8(a):
    return np.ascontiguousarray(a).view(np.uint8)


def kernel(**inputs) -> np.ndarray:
    x_q = np.asarray(inputs["x_q"], np.float32)
    x_kv = np.asarray(inputs["x_kv"], np.float32)
    wq = np.asarray(inputs["wq"], np.float32)
    wkv = np.asarray(inputs["wkv"], np.float32)
    wo = np.asarray(inputs["wo"], np.float32)
    bo = np.asarray(inputs["bo"], np.float32)
    w1 = np.asarray(inputs["w1"], np.float32)
    b1 = np.asarray(inputs["b1"], np.float32)
    w2 = np.asarray(inputs["w2"], np.float32)
    b2 = np.asarray(inputs["b2"], np.float32)
    ln1_g = np.asarray(inputs["ln1_g"], np.float32)
    ln1_b = np.asarray(inputs["ln1_b"], np.float32)
    ln2_g = np.asarray(inputs["ln2_g"], np.float32)
    ln2_b = np.asarray(inputs["ln2_b"], np.float32)

    shared = {}
    for l in range(L):
        wk_f = wkv[l][:, :E]
        wv_f = wkv[l][:, E:]
        wq_eff = ln1_g[l][:, None] * wq[l]
        wk_eff = ln1_g[l][:, None] * wk_f
        wv_eff = ln1_g[l][:, None] * wv_f
        bq_eff = ln1_b[l] @ wq[l]
        bv_eff = ln1_b[l] @ wv_f
        bo_eff = bo[l] + bv_eff @ wo[l]
        w1_eff = ln2_g[l][:, None] * w1[l]
        b1_eff = ln2_b[l] @ w1[l] + b1[l]

        wq8 = _rearr_dr(_fp8(WS * wq_eff), SE)
        wk8 = _rearr_dr(_fp8(WS * wk_eff), SE)
        wv8 = _rearr_dr(_fp8(WS * wv_eff), SE)
        wo8 = _rearr_dr(_fp8(WS * wo[l]), SE)
        w18 = _rearr_dr(_fp8(WS * w1_eff), SE)
        w28 = _rearr_dr(_fp8(WS * w2[l]), SM)
        w1r8 = _rearr_dr(_fp8(WS * w1_eff - _fp8(WS * w1_eff).astype(np.float32)), SE)
        w2r8 = _rearr_dr(_fp8(WS * w2[l] - _fp8(WS * w2[l]).astype(np.float32)), SM)
        bq_c = _cols(QKS * bq_eff)      # [128, 4] f32
        b1_c = _cols(b1_eff)            # [128, 16] f32

        crit = np.concatenate(
            [_u8(wq8), _u8(wk8), _u8(wv8), _u8(bq_c)], axis=1)
        restA = np.concatenate([_u8(wo8), _u8(w18), _u8(w1r8)], axis=1)
        restB = np.concatenate([_u8(w28), _u8(w2r8), _u8(b1_c)], axis=1)
        rows2 = np.stack(
            [_bf16(AOS * WS * bo_eff), _bf16(WS * b2[l])], axis=0)[None]
        shared.update({
            f"crit_{l}": crit.view(ml_dtypes.float8_e4m3),
            f"restA_{l}": restA.view(ml_dtypes.float8_e4m3),
            f"restB_{l}": restB.view(ml_dtypes.float8_e4m3),
            f"rows_{l}": rows2,
        })

    in_maps = []
    for c in range(8):
        b, qc = c // 4, c % 4
        m = dict(shared)
        m["xq"] = np.ascontiguousarray(x_q[b, qc * NQ:(qc + 1) * NQ, :])
        m["xkv"] = np.ascontiguousarray(_bf16(x_kv[b]))
        in_maps.append(m)

    nc = get_nc()
    res = bass_utils.run_bass_kernel_spmd(nc, in_maps, core_ids=list(range(8)))

    out = np.empty((2, 2048, E), np.float32)
    for c in range(8):
        b, qc = c // 4, c % 4
        out[b, qc * NQ:(qc + 1) * NQ, :] = res.results[c]["y"]
    return out


# revision 79
# speedup vs baseline: 1.2330x; 1.0027x over previous
"""Trainium2 Bass kernel for a 2-layer cross-encoder (CrossEncoder).

Model: B=2, NQ=NKV=2048, E=512, H=8 (d_head=64), MLP=2048, depth=2, fp32 I/O.

Sharding (8 cores, no collectives): core c handles batch b=c//4 and query
rows [qc*512, (qc+1)*512) with qc=c%4.  Each core computes the full KV
projections for its batch so every core produces its output slice
independently.

Key structure (v4):
 - All heavy matmuls fp8e4m3 + DoubleRow (0.5 cyc/row), including the
   scores matmul (q/k stored as fp8 DR pair tiles, produced by a cheap
   SBUF->SBUF DMA partition-rearrange; head h sits at base partition
   0/64 of its E-chunk pair tile so the PE base-partition rule holds).
 - Attention runs head-PAIRS interleaved: the two heads' exp chains use
   opposite engines per group, so Act and DVE both stay fed and the
   scores->exp->attnV sem latency is hidden.
 - LayerNorm activations transposed by the DMA XBAR (dma_start_transpose)
   instead of PE identity matmuls + PSUM round trips; fp8 quantize runs
   from SBUF (2x/4x DVE modes).  rstd = 1/sqrt(var+eps) via bit trick +
   1 Newton step on Pool (batched for the kv setup).
 - exp() is Schraudolph-only (int8 round writes fp8e4m3 bits directly),
   split Act (Identity activation) / DVE; Act's only table is Gelu.
 - Softmax denominator comes free from 64 constant columns per head in V.
 - x_kv is uploaded bf16; all weights of a layer arrive as 3 packed DMAs
   staged so bulk transfers never sit in front of latency-critical XBAR
   or pair-rearrange DMAs in the shared DMA queue.
 - K/V of layer l+1 (which depend only on hkvT) are hoisted between
   FFN1(l) and FFN2(l) to fill idle engines there; LN2 interleaves with
   the O-proj residuals, next-layer LN1 with the FFN2 residuals.
"""

import numpy as np
import ml_dtypes

import concourse.bass as bass
import concourse.bacc as bacc
import concourse.mybir as mybir
import concourse.tile as tile
from concourse import bass_utils
from contextlib import ExitStack

P = 128
E = 512
EC = E // P        # 4 chunks of the embedding dim
SE = EC // 2       # 2 DoubleRow super-chunks
NQ = 512           # query rows per core
QC = NQ // P       # 4 query chunks
NKV = 2048
KC = NKV // P      # 16 key chunks of 128
KN = NKV // 512    # 4 key chunks of 512
GK = KC // 2       # 8 key pair-groups
H = 8
DH = 64
MLP = 2048
MC = MLP // P      # 16 mlp chunks of 128
SM = MC // 2       # 8 DoubleRow super-chunks
L = 2
LN_EPS = 1e-5
F32 = mybir.dt.float32
I32 = mybir.dt.int32
BF16 = mybir.dt.bfloat16
FP8 = mybir.dt.float8e4
I8 = mybir.dt.int8
AF = mybir.ActivationFunctionType
ALU = mybir.AluOpType
DRM = mybir.MatmulPerfMode.DoubleRow

WS = 64.0                       # fp8 weight pre-scale (host side)
QKS = 8.0                       # q/k storage scale
SCALE = DH ** -0.5
EXPS = SCALE / (QKS * QKS)      # exp scale applied to scores psum (=1/512)
AOS = 64.0                      # attnout storage scale (fp8 subnormal guard)
SCH_A = (8.0 / np.log(2.0)) * EXPS   # Schraudolph slope for fp8e4 bits
SCH_B = 56.0 - 0.47                  # fp8e4 exponent bias term - rms shift
RSQRT_MAGIC = 0x5F3759DF
# per-(head-in-pair, group) exp engine: A=Act(Identity act) D=DVE.
# Anti-aligned so the two heads of a pair use opposite engines; 9A/7D
# because DVE also owns the softmax divide.
EXP_PAT = ["AADADADA", "ADADADAA"]
EXP_PAT_ODD = ["AADADADA", "DDADADAA"]   # 9A/7D for odd pairs (balance)
H2R = False                              # FFN1 activation-residual pass
W2R = False                              # FFN2 weight-residual pass

# packed-weight byte offsets (per partition)
C_WQ, C_WK, C_WV, C_BQ, C_END = 0, 2048, 4096, 6144, 6160
A_WO, A_W1, A_W1R, A_END = 0, 2048, 10240, 18432
B_W2, B_W2R, B_B1, B_END = 0, 8192, 16384, 16448

_CACHE = {}


def _build():
    """Build the per-core Bass program (identical on all 8 cores)."""
    nc = bacc.Bacc("TRN2", target_bir_lowering=False, debug=False, num_devices=8)

    xq_d = nc.dram_tensor("xq", [NQ, E], F32, kind="ExternalInput").ap()
    xkv_d = nc.dram_tensor("xkv", [NKV, E], BF16, kind="ExternalInput").ap()
    wd = []
    for l in range(L):
        wd.append({
            "crit": nc.dram_tensor(f"crit_{l}", [P, C_END], FP8, kind="ExternalInput").ap(),
            "restA": nc.dram_tensor(f"restA_{l}", [P, A_END], FP8, kind="ExternalInput").ap(),
            "restB": nc.dram_tensor(f"restB_{l}", [P, B_END], FP8, kind="ExternalInput").ap(),
            "rows": nc.dram_tensor(f"rows_{l}", [1, 2, E], BF16, kind="ExternalInput").ap(),
        })
    y_d = nc.dram_tensor("y", [NQ, E], F32, kind="ExternalOutput").ap()

    with tile.TileContext(nc) as tc, ExitStack() as ctx:
        const_pool = ctx.enter_context(tc.tile_pool(name="const", bufs=1))
        ones1 = const_pool.tile([1, P], BF16)
        nc.gpsimd.memset(ones1[:], 1.0)
        schb_col = const_pool.tile([P, 1], F32)
        nc.gpsimd.memset(schb_col[:], SCH_B)

        stats_pool = ctx.enter_context(tc.tile_pool(name="stats", bufs=12))

        def rsqrt_chain(var_ap, rstd_ap, w):
            """rstd = 1/sqrt(var+eps) on Pool via bit trick + 1 Newton step."""
            ve = stats_pool.tile([P, 4], F32, name="ve")[:, :w]
            nc.gpsimd.tensor_scalar_add(ve, var_ap, LN_EPS)
            y0 = stats_pool.tile([P, 4], F32, name="y0")[:, :w]
            # int bit-trick ops run on DVE (Pool's Q7 rejects int shifts)
            nc.vector.tensor_scalar(
                y0.bitcast(I32), ve.bitcast(I32), 1, 0,
                op0=ALU.logical_shift_right, op1=ALU.bypass,
            )
            nc.vector.tensor_scalar(
                y0.bitcast(I32), y0.bitcast(I32), -1, RSQRT_MAGIC,
                op0=ALU.mult, op1=ALU.add,
            )
            t = stats_pool.tile([P, 4], F32, name="t")[:, :w]
            nc.gpsimd.tensor_tensor(t, y0, y0, op=ALU.mult)
            nc.gpsimd.tensor_tensor(t, t, ve, op=ALU.mult)
            nc.gpsimd.tensor_scalar(t, t, -0.5, 1.5, op0=ALU.mult, op1=ALU.add)
            nc.gpsimd.tensor_tensor(rstd_ap, y0, t, op=ALU.mult)

        def ln_stats(x_ap, mv_ap):
            bnst = stats_pool.tile([P, 6], F32, name="bnst")
            nc.vector.bn_stats(bnst[:], x_ap)
            nc.vector.bn_aggr(mv_ap, bnst[:])

        # norm for one [128, E] tile into slot u of a 4-wide hn4 buffer.
        def ln_norm(src_ap, mean_ap, rstd_ap, hn4, u, eng="D"):
            if eng == "P":
                nc.gpsimd.tensor_scalar(
                    hn4[:, u, :], src_ap, mean_ap, rstd_ap,
                    op0=ALU.subtract, op1=ALU.mult,
                )
            else:
                nc.vector.tensor_scalar(
                    hn4[:, u, :], src_ap, mean_ap, rstd_ap,
                    op0=ALU.subtract, op1=ALU.mult,
                )

        # one XBAR transpose + one quantize for an nu-tile hn batch
        # (nu*128 tokens).  dst slice [:, :, tok0:tok0+nu*128].
        def ln_txn(hn, t_pool, dstT8, tok0, resT8=None, eng="D", nu=4):
            ht = t_pool.tile([P, 4 * EC, P], BF16, name="ht", tag="ht",
                             bufs=2)[:, :nu * EC, :]
            nc.sync.dma_start_transpose(
                ht, hn[:].rearrange("p u c -> p (u c)")
            )
            dst = dstT8[:, :, tok0:tok0 + nu * P].rearrange(
                "p c (u t) -> p u c t", u=nu
            )
            src = ht.rearrange("p (u c) t -> p u c t", u=nu)
            if eng == "P":
                nc.gpsimd.tensor_copy(dst, src)
            elif eng == "D":
                nc.vector.tensor_copy(dst, src)
            else:
                nc.scalar.copy(dst, src)
            if resT8 is not None:
                # STT requires <=3D APs: one op per 128-token sub-block.
                for u in range(nu):
                    t0 = tok0 + u * P
                    nc.vector.scalar_tensor_tensor(
                        resT8[:, :, t0:t0 + P], dstT8[:, :, t0:t0 + P], -1.0,
                        ht[:, u * EC:(u + 1) * EC, :], op0=ALU.mult, op1=ALU.add,
                    )

        # Residual stream: one [128, 4, 512] fp32 tile (qc-major subtiles).
        xq_pool = ctx.enter_context(tc.tile_pool(name="xq", bufs=1))
        xqb = xq_pool.tile([P, QC, E], F32, name="xqb", tag="xqb")
        nc.sync.dma_start(xqb[:], xq_d.rearrange("(a p) c -> p a c", p=P))
        xq = [xqb[:, qc, :] for qc in range(QC)]

        # hkv^T fp8 DR tile [128, 4, NKV]: (c, p) holds E-row c*128+p.
        hkvT_pool = ctx.enter_context(tc.tile_pool(name="hkvT", bufs=1))
        hkvT8 = hkvT_pool.tile([P, EC, NKV], FP8, name="hkvT8", tag="hkvT8")

        # PSUM pools (8 banks): shared 1-bank ring 6 + att 2 = 8.
        ss_pool = ctx.enter_context(tc.tile_pool(name="ss", bufs=6, space="PSUM"))
        att_pool = ctx.enter_context(tc.tile_pool(name="attp", bufs=2, space="PSUM"))

        wpool = ctx.enter_context(tc.tile_pool(name="w", bufs=1))

        def load_pack(l, which, sz):
            t = wpool.tile([P, sz], FP8, name=f"{which}_{l}")
            nc.sync.dma_start(t[:], wd[l][which])
            return t

        def load_rows(l):
            t = wpool.tile([1, 2, E], BF16, name=f"rows_{l}")
            nc.sync.dma_start(t[:], wd[l]["rows"])
            return t

        def dr(ap, S):
            return ap.rearrange("p (s j c) -> p s j c", s=S, j=2)

        work = ctx.enter_context(tc.tile_pool(name="work", bufs=1))
        big = ctx.enter_context(tc.tile_pool(name="big", bufs=1))
        ex_pool = ctx.enter_context(tc.tile_pool(name="ex", bufs=5))
        lnp = ctx.enter_context(tc.tile_pool(name="lnp", bufs=4))

        def ln_hn2():
            return lnp.tile([P, 2, E], BF16, name="hn2", tag="hn2", bufs=3)

        # q-side LN: per-tile stats+norm into hn2 slot qc%2; a following
        # ln_txn(nu=2) finishes each half.
        def ln_q_tile(qc, hn2, eng="D"):
            mv = stats_pool.tile([P, 2], F32, name="mv")
            ln_stats(xq[qc], mv[:])
            rstd = stats_pool.tile([P, 1], F32, name="rstd")
            rsqrt_chain(mv[:, 1:2], rstd[:], 1)
            ln_norm(xq[qc], mv[:, 0:1], rstd[:], hn2, qc % 2, eng=eng)

        crit = [None, None]
        with tc.tile_pool(name="kvln", bufs=4) as kvln_pool:
            # x_kv bf16, 4 chunks of [128, 4, 512] (token-block subtiles),
            # ring of 3: chunk a+3 reuses chunk a's space.
            xkvb = []

            def load_xkvb(a):
                t = kvln_pool.tile([P, 4, E], BF16, name=f"xkvb{a}",
                                   tag="xkvb", bufs=3)
                nc.sync.dma_start(
                    t[:], xkv_d[a * 512:(a + 1) * 512, :].rearrange(
                        "(u p) c -> p u c", p=P)
                )
                xkvb.append(t)

            load_xkvb(0)
            load_xkvb(1)
            crit[0] = load_pack(0, "crit", C_END)
            load_xkvb(2)
            load_xkvb(3)

            # ---- LN1(x_q, layer 0), two halves ----
            hqT8_l0 = work.tile([P, EC, NQ], FP8, name="hqT8_l0", tag="actT")
            for half in range(2):
                hn2 = ln_hn2()
                for qc in (2 * half, 2 * half + 1):
                    ln_q_tile(qc, hn2, eng="D")
                ln_txn(hn2, lnp, hqT8_l0, half * 256, eng="A", nu=2)

            # ---- startup: x_kv LN + XBAR transpose + fp8 quantize.
            #      Stats run one batch ahead of the norm/xbar chains so
            #      DVE's queue never blocks the next batch's stats. ----
            mv4s, rstd4s = {}, {}

            def kv_stats(b):
                mv4 = stats_pool.tile([P, 2, 4], F32, name="mv4", bufs=4)
                for u in range(4):
                    ln_stats(xkvb[b][:, u, :], mv4[:, :, u])
                rstd4 = stats_pool.tile([P, 4], F32, name="rstd4", bufs=4)
                rsqrt_chain(mv4[:, 1, :], rstd4[:], 4)
                mv4s[b], rstd4s[b] = mv4, rstd4

            def kv_finish(b):
                khn4 = kvln_pool.tile([P, 4, E], BF16, name="khn4",
                                      tag="khn4", bufs=2)
                for u in range(4):
                    ln_norm(xkvb[b][:, u, :], mv4s[b][:, 0, u:u + 1],
                            rstd4s[b][:, u:u + 1], khn4, u, eng="DDAD"[u])
                ln_txn(khn4, lnp, hkvT8, b * 512, eng="AADA"[b], nu=4)

            kv_stats(0)
            kv_stats(1)
            kv_finish(0)
            kv_stats(2)
            kv_finish(1)
            kv_stats(3)
            kv_finish(2)
            kv_finish(3)

        restA = [None, None]
        restB = [None, None]
        rows = [None, None]

        # va pair tiles persist across layers (v columns overwritten per
        # layer, the constant denominator columns are set once here).
        va = [
            big.tile([P, 2, H, 2 * DH], FP8, name=f"va{g}", tag=f"va{g}")
            for g in range(GK)
        ]
        for g in range(GK):
            nc.gpsimd.memset(va[g][:, :, :, DH:2 * DH], 1.0 / QKS)

        def proj_q(l, hqT8_ap):
            """Q projection + pair-rearrange for layer l."""
            wq_s = dr(crit[l][:, C_WQ:C_WK], SE)
            bq = crit[l][:, C_BQ:C_END].bitcast(F32)
            q8f = work.tile([P, EC, NQ], FP8, name="q8f", tag="q8f")
            q8p = work.tile([P, EC, 2, NQ], FP8, name="q8p", tag="q8p")
            for m in range(EC):
                ps = ss_pool.tile([P, E], F32, name="pp", tag="ss")
                for half in range(2):
                    c0, c1 = half * 256, half * 256 + 256
                    for s in range(SE):
                        nc.tensor.matmul(
                            ps[:, c0:c1], wq_s[:, s, :, m * P:(m + 1) * P],
                            hqT8_ap[:, 2 * s:2 * s + 2, c0:c1],
                            start=(s == 0), stop=(s == SE - 1), perf_mode=DRM,
                            skip_group_check=True,
                        )
                if m % 2:
                    nc.scalar.activation(
                        q8f[:, m, :], ps[:], AF.Identity,
                        bias=bq[:, m:m + 1], scale=1.0 / QKS,
                    )
                else:
                    nc.vector.tensor_scalar(
                        q8f[:, m, :], ps[:], 1.0 / QKS, bq[:, m:m + 1],
                        op0=ALU.mult, op1=ALU.add,
                    )
            # pair layout: head h=2m+half at partitions [64*half, 64*half+32),
            # (p, j) <-> d-row 32*j + p.  4 plain partition-slice DMAs.
            for half in range(2):
                for jj in range(2):
                    r0 = 64 * half + 32 * jj
                    nc.sync.dma_start(
                        q8p[64 * half:64 * half + 32, :, jj, :],
                        q8f[r0:r0 + 32, :, :],
                    )
            return q8p

        def proj_kv(l):
            """K (with pair-rearrange) and V, interleaved per E-chunk."""
            wk_s = dr(crit[l][:, C_WK:C_WV], SE)
            wv_s = dr(crit[l][:, C_WV:C_BQ], SE)
            k8f = big.tile([P, EC, NKV], FP8, name="k8f", tag="k8f")
            k8p = big.tile([P, EC, 2, NKV], FP8, name="k8p", tag="k8p")
            for m in range(EC):
                for n in range(KN):
                    ps = ss_pool.tile([P, E], F32, name="pp", tag="ss")
                    for s in range(SE):
                        nc.tensor.matmul(
                            ps[:], wk_s[:, s, :, m * P:(m + 1) * P],
                            hkvT8[:, 2 * s:2 * s + 2, n * 512:(n + 1) * 512],
                            start=(s == 0), stop=(s == SE - 1), perf_mode=DRM,
                        )
                    if n % 2:
                        nc.scalar.mul(k8f[:, m, n * 512:(n + 1) * 512], ps[:], 1.0 / QKS)
                    else:
                        nc.vector.tensor_scalar_mul(
                            k8f[:, m, n * 512:(n + 1) * 512], ps[:], 1.0 / QKS
                        )
                for half in range(2):
                    for jj in range(2):
                        r0 = 64 * half + 32 * jj
                        nc.sync.dma_start(
                            k8p[64 * half:64 * half + 32, m, jj, :],
                            k8f[r0:r0 + 32, m, :],
                        )
                for mv in range(4 * m, 4 * m + 4):
                    ps = ss_pool.tile([P, E], F32, name="pp", tag="ss")
                    for s in range(SE):
                        nc.tensor.matmul(
                            ps[:], hkvT8[:, 2 * s:2 * s + 2, mv * P:(mv + 1) * P],
                            wv_s[:, s, :, :],
                            start=(s == 0), stop=(s == SE - 1), perf_mode=DRM,
                        )
                    dst = va[mv // 2][:, mv % 2, :, 0:DH]
                    src = ps[:].rearrange("p (h d) -> p h d", h=H)
                    if mv % 4 == 3:
                        nc.vector.tensor_scalar_mul(dst, src, 1.0 / QKS)
                    else:
                        nc.scalar.mul(dst, src, 1.0 / QKS)
            return k8p

        # ---- layer 0 Q/K/V ----
        q8p = proj_q(0, hqT8_l0[:])
        k8p = proj_kv(0)

        for l in range(L):
            # ---- attention, head-pairs (fp8 DR scores) ----
            aoT = work.tile([P, EC, NQ], FP8, name="aoT", tag="aoT")
            for m in range(EC):          # pair (h0, h1) = (2m, 2m+1)
                # bulk weight loads, issued mid-attention so their
                # transfers never block latency-critical DMAs.
                if l == 0 and m == 1:
                    restA[0] = load_pack(0, "restA", A_END)
                    rows[0] = load_rows(0)
                if l == 0 and m == 2:
                    crit[1] = load_pack(1, "crit", C_END)
                if l == 0 and m == 3:
                    restB[0] = load_pack(0, "restB", B_END)
                kst = [k8p[0:32, m, :, :], k8p[64:96, m, :, :]]
                qmv = [q8p[0:32, m, :, :], q8p[64:96, m, :, :]]
                pso = [
                    att_pool.tile([P, E], F32, name="ps_oT", tag="att")
                    for _ in range(2)
                ]
                exs = [[], []]
                for g in range(GK):
                    for j in range(2):
                        ex = ex_pool.tile([P, 2, NQ], FP8, name="ex", tag="ex")
                        for sub in range(2):
                            ps_s = ss_pool.tile([P, NQ], F32, name="ps_s", tag="ss")
                            c0 = (2 * g + sub) * P
                            nc.tensor.matmul(
                                ps_s[:], kst[j][:, :, c0:c0 + P], qmv[j],
                                start=True, stop=True, perf_mode=DRM,
                            )
                            pat = EXP_PAT if m % 2 == 0 else EXP_PAT_ODD
                            if pat[j][g] == "A":
                                nc.scalar.activation(
                                    ex[:, sub, :].bitcast(I8), ps_s[:],
                                    AF.Identity, bias=schb_col[:], scale=SCH_A,
                                )
                            else:
                                nc.vector.tensor_scalar(
                                    ex[:, sub, :].bitcast(I8), ps_s[:],
                                    SCH_A, SCH_B, op0=ALU.mult, op1=ALU.add,
                                )
                        exs[j].append(ex)
                    if g >= 1:
                        for j in range(2):
                            nc.tensor.matmul(
                                pso[j][:], va[g - 1][:, :, 2 * m + j, :],
                                exs[j][g - 1][:],
                                start=(g == 1), stop=False, perf_mode=DRM,
                            )
                for j in range(2):
                    nc.tensor.matmul(
                        pso[j][:], va[GK - 1][:, :, 2 * m + j, :],
                        exs[j][GK - 1][:],
                        start=False, stop=True, perf_mode=DRM,
                    )
                    # normalize: aoT = (x8 unnorm) * rcp(sum/8) = x64 attnout
                    rcp = stats_pool.tile([DH, NQ], BF16, name="rcp", bufs=2)
                    with nc.allow_low_precision(reason="bf16 softmax denom"):
                        nc.vector.reciprocal(rcp[:], pso[j][DH:P, :])
                    nc.vector.tensor_tensor(
                        aoT[64 * j:64 * j + 64, m, :], pso[j][0:DH, :],
                        rcp[:], op=ALU.mult,
                    )

            # ---- out-proj (DR fp8) + bo row + residual; LN2 interleaved ----
            wo_s = dr(restA[l][:, A_WO:A_W1], SE)
            h2T8 = work.tile([P, EC, NQ], FP8, name="h2T8", tag="actT")
            h2r8 = (work.tile([P, EC, NQ], FP8, name="h2r8", tag="h2r8")
                    if H2R else None)
            hn2 = ln_hn2()
            for qc in range(QC):
                ps = ss_pool.tile([P, E], F32, name="pp", tag="ss")
                nc.tensor.matmul(
                    ps[:], ones1[:], rows[l][:, 0, :], start=True, stop=False,
                    skip_group_check=True,
                )
                for s in range(SE):
                    nc.tensor.matmul(
                        ps[:], aoT[:, 2 * s:2 * s + 2, qc * P:(qc + 1) * P],
                        wo_s[:, s, :, :],
                        start=False, stop=(s == SE - 1), perf_mode=DRM,
                        skip_group_check=True,
                    )
                nc.vector.scalar_tensor_tensor(
                    xq[qc], ps[:], 1.0 / (AOS * WS), xq[qc],
                    op0=ALU.mult, op1=ALU.add,
                )
                ln_q_tile(qc, hn2, eng="D")
                if qc % 2 == 1:
                    ln_txn(hn2, lnp, h2T8, (qc - 1) * P, resT8=h2r8,
                           eng="D", nu=2)
                    if qc == 1:
                        hn2 = ln_hn2()
            if l + 1 < L:
                restA[1] = load_pack(1, "restA", A_END)
                rows[1] = load_rows(1)

            # ---- hoisted K/V of layer l+1 (depend only on hkvT): emitted
            #      here so the PE fills the LN2-chain gap before FFN1. ----
            if l + 1 < L:
                k8p = proj_kv(l + 1)

            # ---- FFN1 (DR fp8): g^T = gelu(w1^T @ h2^T / 64 + b1) ----
            w1_s = dr(restA[l][:, A_W1:A_W1R], SE)
            w1r_s = dr(restA[l][:, A_W1R:A_END], SE)
            b1 = restB[l][:, B_B1:B_END].bitcast(F32)
            gT8 = big.tile([P, MC, NQ], FP8, name="gT8", tag="gT8")
            for m in range(MC):
                ps = ss_pool.tile([P, E], F32, name="pp", tag="ss")
                for half in range(2):
                    c0, c1 = half * 256, half * 256 + 256
                    for s in range(SE):
                        nc.tensor.matmul(
                            ps[:, c0:c1], w1_s[:, s, :, m * P:(m + 1) * P],
                            h2T8[:, 2 * s:2 * s + 2, c0:c1],
                            start=(s == 0), stop=False, perf_mode=DRM,
                            skip_group_check=True,
                        )
                    for s in range(SE):
                        nc.tensor.matmul(
                            ps[:, c0:c1], w1r_s[:, s, :, m * P:(m + 1) * P],
                            h2T8[:, 2 * s:2 * s + 2, c0:c1],
                            start=False, stop=(not H2R and s == SE - 1),
                            perf_mode=DRM, skip_group_check=True,
                        )
                    if H2R:
                        for s in range(SE):
                            nc.tensor.matmul(
                                ps[:, c0:c1], w1_s[:, s, :, m * P:(m + 1) * P],
                                h2r8[:, 2 * s:2 * s + 2, c0:c1],
                                start=False, stop=(s == SE - 1), perf_mode=DRM,
                                skip_group_check=True,
                            )
                nc.scalar.activation(
                    gT8[:, m, :], ps[:], AF.Gelu,
                    bias=b1[:, m:m + 1], scale=1.0 / WS,
                )

            # ---- FFN2 (DR fp8) + b2 row + residual; next LN1 interleaved ----
            w2_s = dr(restB[l][:, B_W2:B_W2R], SM)
            w2r_s = dr(restB[l][:, B_W2R:B_B1], SM)
            if l + 1 < L:
                hqT8 = work.tile([P, EC, NQ], FP8, name="hqT8", tag="actT2")
                hn2 = ln_hn2()
            for qc in range(QC):
                ps = ss_pool.tile([P, E], F32, name="pp", tag="ss")
                nc.tensor.matmul(
                    ps[:], ones1[:], rows[l][:, 1, :], start=True, stop=False,
                    skip_group_check=True,
                )
                for s in range(SM):
                    nc.tensor.matmul(
                        ps[:], gT8[:, 2 * s:2 * s + 2, qc * P:(qc + 1) * P],
                        w2_s[:, s, :, :],
                        start=False, stop=(not W2R and s == SM - 1),
                        perf_mode=DRM, skip_group_check=True,
                    )
                if W2R:
                    for s in range(SM):
                        nc.tensor.matmul(
                            ps[:], gT8[:, 2 * s:2 * s + 2, qc * P:(qc + 1) * P],
                            w2r_s[:, s, :, :],
                            start=False, stop=(s == SM - 1), perf_mode=DRM,
                            skip_group_check=True,
                        )
                nc.vector.scalar_tensor_tensor(
                    xq[qc], ps[:], 1.0 / WS, xq[qc], op0=ALU.mult, op1=ALU.add,
                )
                if l + 1 < L:
                    ln_q_tile(qc, hn2, eng="D")
                    if qc % 2 == 1:
                        ln_txn(hn2, lnp, hqT8, (qc - 1) * P, eng="D", nu=2)
                        if qc == 1:
                            hn2 = ln_hn2()
                else:
                    nc.sync.dma_start(y_d[qc * P:(qc + 1) * P, :], xq[qc])
            if l + 1 < L:
                restB[1] = load_pack(1, "restB", B_END)
                q8p = proj_q(l + 1, hqT8[:])

    nc.compile()
    return nc


def get_nc():
    if "nc" not in _CACHE:
        _CACHE["nc"] = _build()
    return _CACHE["nc"]


def _fp8(a):
    return np.clip(np.asarray(a, np.float32), -240.0, 240.0).astype(
        ml_dtypes.float8_e4m3
    )


def _bf16(a):
    return np.asarray(a, np.float32).astype(ml_dtypes.bfloat16)


def _rearr_dr(w8, S):
    """[S*2*128, C] (row-major contraction) -> [128, S*2*C] DR pair layout."""
    C = w8.shape[1]
    return np.ascontiguousarray(
        w8.reshape(S, 2, P, C).transpose(2, 0, 1, 3).reshape(P, S * 2 * C)
    )


def _cols(v):
    """[k*128] -> [128, k]: column m holds v[m*128:(m+1)*128]."""
    k = v.shape[0] // P
    return np.ascontiguousarray(np.asarray(v, np.float32).reshape(k, P).T)


def _u8(a):
    return np.ascontiguousarray(a).view(np.uint8)


def kernel(**inputs) -> np.ndarray:
    x_q = np.asarray(inputs["x_q"], np.float32)
    x_kv = np.asarray(inputs["x_kv"], np.float32)
    wq = np.asarray(inputs["wq"], np.float32)
    wkv = np.asarray(inputs["wkv"], np.float32)
    wo = np.asarray(inputs["wo"], np.float32)
    bo = np.asarray(inputs["bo"], np.float32)
    w1 = np.asarray(inputs["w1"], np.float32)
    b1 = np.asarray(inputs["b1"], np.float32)
    w2 = np.asarray(inputs["w2"], np.float32)
    b2 = np.asarray(inputs["b2"], np.float32)
    ln1_g = np.asarray(inputs["ln1_g"], np.float32)
    ln1_b = np.asarray(inputs["ln1_b"], np.float32)
    ln2_g = np.asarray(inputs["ln2_g"], np.float32)
    ln2_b = np.asarray(inputs["ln2_b"], np.float32)

    shared = {}
    for l in range(L):
        wk_f = wkv[l][:, :E]
        wv_f = wkv[l][:, E:]
        wq_eff = ln1_g[l][:, None] * wq[l]
        wk_eff = ln1_g[l][:, None] * wk_f
        wv_eff = ln1_g[l][:, None] * wv_f
        bq_eff = ln1_b[l] @ wq[l]
        bv_eff = ln1_b[l] @ wv_f
        bo_eff = bo[l] + bv_eff @ wo[l]
        w1_eff = ln2_g[l][:, None] * w1[l]
        b1_eff = ln2_b[l] @ w1[l] + b1[l]

        wq8 = _rearr_dr(_fp8(WS * wq_eff), SE)
        wk8 = _rearr_dr(_fp8(WS * wk_eff), SE)
        wv8 = _rearr_dr(_fp8(WS * wv_eff), SE)
        wo8 = _rearr_dr(_fp8(WS * wo[l]), SE)
        w18 = _rearr_dr(_fp8(WS * w1_eff), SE)
        w28 = _rearr_dr(_fp8(WS * w2[l]), SM)
        w1r8 = _rearr_dr(_fp8(WS * w1_eff - _fp8(WS * w1_eff).astype(np.float32)), SE)
        w2r8 = _rearr_dr(_fp8(WS * w2[l] - _fp8(WS * w2[l]).astype(np.float32)), SM)
        bq_c = _cols(QKS * bq_eff)      # [128, 4] f32
        b1_c = _cols(b1_eff)            # [128, 16] f32

        crit = np.concatenate(
            [_u8(wq8), _u8(wk8), _u8(wv8), _u8(bq_c)], axis=1)
        restA = np.concatenate([_u8(wo8), _u8(w18), _u8(w1r8)], axis=1)
        restB = np.concatenate([_u8(w28), _u8(w2r8), _u8(b1_c)], axis=1)
        rows2 = np.stack(
            [_bf16(AOS * WS * bo_eff), _bf16(WS * b2[l])], axis=0)[None]
        shared.update({
            f"crit_{l}": crit.view(ml_dtypes.float8_e4m3),
            f"restA_{l}": restA.view(ml_dtypes.float8_e4m3),
            f"restB_{l}": restB.view(ml_dtypes.float8_e4m3),
            f"rows_{l}": rows2,
        })

    in_maps = []
    for c in range(8):
        b, qc = c // 4, c % 4
        m = dict(shared)
        m["xq"] = np.ascontiguousarray(x_q[b, qc * NQ:(qc + 1) * NQ, :])
        m["xkv"] = np.ascontiguousarray(_bf16(x_kv[b]))
        in_maps.append(m)

    nc = get_nc()
    res = bass_utils.run_bass_kernel_spmd(nc, in_maps, core_ids=list(range(8)))

    out = np.empty((2, 2048, E), np.float32)
    for c in range(8):
        b, qc = c // 4, c % 4
        out[b, qc * NQ:(qc + 1) * NQ, :] = res.results[c]["y"]
    return out


# revision 80
# speedup vs baseline: 1.2506x; 1.0143x over previous
"""Trainium2 Bass kernel for a 2-layer cross-encoder (CrossEncoder).

Model: B=2, NQ=NKV=2048, E=512, H=8 (d_head=64), MLP=2048, depth=2, fp32 I/O.

Sharding (8 cores, no collectives): core c handles batch b=c//4 and query
rows [qc*512, (qc+1)*512) with qc=c%4.  Each core computes the full KV
projections for its batch so every core produces its output slice
independently.

Key structure (v4):
 - All heavy matmuls fp8e4m3 + DoubleRow (0.5 cyc/row), including the
   scores matmul (q/k stored as fp8 DR pair tiles, produced by a cheap
   SBUF->SBUF DMA partition-rearrange; head h sits at base partition
   0/64 of its E-chunk pair tile so the PE base-partition rule holds).
 - Attention runs head-PAIRS interleaved: the two heads' exp chains use
   opposite engines per group, so Act and DVE both stay fed and the
   scores->exp->attnV sem latency is hidden.
 - LayerNorm activations transposed by the DMA XBAR (dma_start_transpose)
   instead of PE identity matmuls + PSUM round trips; fp8 quantize runs
   from SBUF (2x/4x DVE modes).  rstd = 1/sqrt(var+eps) via bit trick +
   1 Newton step on Pool (batched for the kv setup).
 - exp() is Schraudolph-only (int8 round writes fp8e4m3 bits directly),
   split Act (Identity activation) / DVE; Act's only table is Gelu.
 - Softmax denominator comes free from 64 constant columns per head in V.
 - x_kv is uploaded bf16; all weights of a layer arrive as 3 packed DMAs
   staged so bulk transfers never sit in front of latency-critical XBAR
   or pair-rearrange DMAs in the shared DMA queue.
 - K/V of layer l+1 (which depend only on hkvT) are hoisted between
   FFN1(l) and FFN2(l) to fill idle engines there; LN2 interleaves with
   the O-proj residuals, next-layer LN1 with the FFN2 residuals.
"""

import numpy as np
import ml_dtypes

import concourse.bass as bass
import concourse.bacc as bacc
import concourse.mybir as mybir
import concourse.tile as tile
from concourse import bass_utils
from contextlib import ExitStack

P = 128
E = 512
EC = E // P        # 4 chunks of the embedding dim
SE = EC // 2       # 2 DoubleRow super-chunks
NQ = 512           # query rows per core
QC = NQ // P       # 4 query chunks
NKV = 2048
KC = NKV // P      # 16 key chunks of 128
KN = NKV // 512    # 4 key chunks of 512
GK = KC // 2       # 8 key pair-groups
H = 8
DH = 64
MLP = 2048
MC = MLP // P      # 16 mlp chunks of 128
SM = MC // 2       # 8 DoubleRow super-chunks
L = 2
LN_EPS = 1e-5
F32 = mybir.dt.float32
I32 = mybir.dt.int32
BF16 = mybir.dt.bfloat16
FP8 = mybir.dt.float8e4
I8 = mybir.dt.int8
AF = mybir.ActivationFunctionType
ALU = mybir.AluOpType
DRM = mybir.MatmulPerfMode.DoubleRow

WS = 64.0                       # fp8 weight pre-scale (host side)
QKS = 8.0                       # q/k storage scale
SCALE = DH ** -0.5
EXPS = SCALE / (QKS * QKS)      # exp scale applied to scores psum (=1/512)
AOS = 64.0                      # attnout storage scale (fp8 subnormal guard)
SCH_A = (8.0 / np.log(2.0)) * EXPS   # Schraudolph slope for fp8e4 bits
SCH_B = 56.0 - 0.47                  # fp8e4 exponent bias term - rms shift
RSQRT_MAGIC = 0x5F3759DF
# per-(head-in-pair, group) exp engine: A=Act(Identity act) D=DVE.
# Anti-aligned so the two heads of a pair use opposite engines; 9A/7D
# because DVE also owns the softmax divide.
EXP_PAT = ["AADADADA", "ADADADAA"]
EXP_PAT_ODD = ["AADADADA", "DDADADAA"]   # 9A/7D for odd pairs (balance)
H2R = False                              # FFN1 activation-residual pass
W2R = False                              # FFN2 weight-residual pass

# packed-weight byte offsets (per partition)
C_WQ, C_WK, C_WV, C_BQ, C_END = 0, 2048, 4096, 6144, 6160
A_WO, A_W1, A_W1R, A_END = 0, 2048, 10240, 18432
B_W2, B_W2R, B_B1, B_END = 0, 8192, 16384, 16448

_CACHE = {}


def _build():
    """Build the per-core Bass program (identical on all 8 cores)."""
    nc = bacc.Bacc("TRN2", target_bir_lowering=False, debug=False, num_devices=8)

    xq_d = nc.dram_tensor("xq", [NQ, E], F32, kind="ExternalInput").ap()
    xkv_d = nc.dram_tensor("xkv", [NKV, E], BF16, kind="ExternalInput").ap()
    wd = []
    for l in range(L):
        wd.append({
            "crit": nc.dram_tensor(f"crit_{l}", [P, C_END], FP8, kind="ExternalInput").ap(),
            "restA": nc.dram_tensor(f"restA_{l}", [P, A_END], FP8, kind="ExternalInput").ap(),
            "restB": nc.dram_tensor(f"restB_{l}", [P, B_END], FP8, kind="ExternalInput").ap(),
            "rows": nc.dram_tensor(f"rows_{l}", [1, 2, E], BF16, kind="ExternalInput").ap(),
        })
    y_d = nc.dram_tensor("y", [NQ, E], F32, kind="ExternalOutput").ap()

    with tile.TileContext(nc) as tc, ExitStack() as ctx:
        const_pool = ctx.enter_context(tc.tile_pool(name="const", bufs=1))
        ones1 = const_pool.tile([1, P], BF16)
        nc.gpsimd.memset(ones1[:], 1.0)
        schb_col = const_pool.tile([P, 1], F32)
        nc.gpsimd.memset(schb_col[:], SCH_B)

        stats_pool = ctx.enter_context(tc.tile_pool(name="stats", bufs=12))

        def rsqrt_chain(var_ap, rstd_ap, w):
            """rstd = 1/sqrt(var+eps) on Pool via bit trick + 1 Newton step."""
            ve = stats_pool.tile([P, 4], F32, name="ve")[:, :w]
            nc.gpsimd.tensor_scalar_add(ve, var_ap, LN_EPS)
            y0 = stats_pool.tile([P, 4], F32, name="y0")[:, :w]
            # int bit-trick ops run on DVE (Pool's Q7 rejects int shifts)
            nc.vector.tensor_scalar(
                y0.bitcast(I32), ve.bitcast(I32), 1, 0,
                op0=ALU.logical_shift_right, op1=ALU.bypass,
            )
            nc.vector.tensor_scalar(
                y0.bitcast(I32), y0.bitcast(I32), -1, RSQRT_MAGIC,
                op0=ALU.mult, op1=ALU.add,
            )
            t = stats_pool.tile([P, 4], F32, name="t")[:, :w]
            nc.gpsimd.tensor_tensor(t, y0, y0, op=ALU.mult)
            nc.gpsimd.tensor_tensor(t, t, ve, op=ALU.mult)
            nc.gpsimd.tensor_scalar(t, t, -0.5, 1.5, op0=ALU.mult, op1=ALU.add)
            nc.gpsimd.tensor_tensor(rstd_ap, y0, t, op=ALU.mult)

        def ln_stats(x_ap, mv_ap):
            bnst = stats_pool.tile([P, 6], F32, name="bnst")
            nc.vector.bn_stats(bnst[:], x_ap)
            nc.vector.bn_aggr(mv_ap, bnst[:])

        # norm for one [128, E] tile into slot u of a 4-wide hn4 buffer.
        def ln_norm(src_ap, mean_ap, rstd_ap, hn4, u, eng="D"):
            if eng == "P":
                nc.gpsimd.tensor_scalar(
                    hn4[:, u, :], src_ap, mean_ap, rstd_ap,
                    op0=ALU.subtract, op1=ALU.mult,
                )
            else:
                nc.vector.tensor_scalar(
                    hn4[:, u, :], src_ap, mean_ap, rstd_ap,
                    op0=ALU.subtract, op1=ALU.mult,
                )

        # one XBAR transpose + one quantize for an nu-tile hn batch
        # (nu*128 tokens).  dst slice [:, :, tok0:tok0+nu*128].
        def ln_txn(hn, t_pool, dstT8, tok0, resT8=None, eng="D", nu=4):
            ht = t_pool.tile([P, 4 * EC, P], BF16, name="ht", tag="ht",
                             bufs=2)[:, :nu * EC, :]
            nc.sync.dma_start_transpose(
                ht, hn[:].rearrange("p u c -> p (u c)")
            )
            dst = dstT8[:, :, tok0:tok0 + nu * P].rearrange(
                "p c (u t) -> p u c t", u=nu
            )
            src = ht.rearrange("p (u c) t -> p u c t", u=nu)
            if eng == "P":
                nc.gpsimd.tensor_copy(dst, src)
            elif eng == "D":
                nc.vector.tensor_copy(dst, src)
            else:
                nc.scalar.copy(dst, src)
            if resT8 is not None:
                # STT requires <=3D APs: one op per 128-token sub-block.
                for u in range(nu):
                    t0 = tok0 + u * P
                    nc.vector.scalar_tensor_tensor(
                        resT8[:, :, t0:t0 + P], dstT8[:, :, t0:t0 + P], -1.0,
                        ht[:, u * EC:(u + 1) * EC, :], op0=ALU.mult, op1=ALU.add,
                    )

        # Residual stream: one [128, 4, 512] fp32 tile (qc-major subtiles).
        xq_pool = ctx.enter_context(tc.tile_pool(name="xq", bufs=1))
        xqb = xq_pool.tile([P, QC, E], F32, name="xqb", tag="xqb")
        nc.sync.dma_start(xqb[:], xq_d.rearrange("(a p) c -> p a c", p=P))
        xq = [xqb[:, qc, :] for qc in range(QC)]

        # hkv^T fp8 DR tile [128, 4, NKV]: (c, p) holds E-row c*128+p.
        hkvT_pool = ctx.enter_context(tc.tile_pool(name="hkvT", bufs=1))
        hkvT8 = hkvT_pool.tile([P, EC, NKV], FP8, name="hkvT8", tag="hkvT8")

        # PSUM pools (8 banks): shared 1-bank ring 6 + att 2 = 8.
        ss_pool = ctx.enter_context(tc.tile_pool(name="ss", bufs=6, space="PSUM"))
        att_pool = ctx.enter_context(tc.tile_pool(name="attp", bufs=2, space="PSUM"))

        wpool = ctx.enter_context(tc.tile_pool(name="w", bufs=1))

        def load_pack(l, which, sz):
            t = wpool.tile([P, sz], FP8, name=f"{which}_{l}")
            nc.sync.dma_start(t[:], wd[l][which])
            return t

        def load_rows(l):
            t = wpool.tile([1, 2, E], BF16, name=f"rows_{l}")
            nc.sync.dma_start(t[:], wd[l]["rows"])
            return t

        def dr(ap, S):
            return ap.rearrange("p (s j c) -> p s j c", s=S, j=2)

        work = ctx.enter_context(tc.tile_pool(name="work", bufs=1))
        big = ctx.enter_context(tc.tile_pool(name="big", bufs=1))
        ex_pool = ctx.enter_context(tc.tile_pool(name="ex", bufs=5))
        lnp = ctx.enter_context(tc.tile_pool(name="lnp", bufs=4))

        def ln_hn2():
            return lnp.tile([P, 2, E], BF16, name="hn2", tag="hn2", bufs=3)

        # q-side LN: per-tile stats+norm into hn2 slot qc%2; a following
        # ln_txn(nu=2) finishes each half.
        def ln_q_tile(qc, hn2, eng="D"):
            mv = stats_pool.tile([P, 2], F32, name="mv")
            ln_stats(xq[qc], mv[:])
            rstd = stats_pool.tile([P, 1], F32, name="rstd")
            rsqrt_chain(mv[:, 1:2], rstd[:], 1)
            ln_norm(xq[qc], mv[:, 0:1], rstd[:], hn2, qc % 2, eng=eng)

        crit = [None, None]
        with tc.tile_pool(name="kvln", bufs=4) as kvln_pool:
            # x_kv bf16, 4 chunks of [128, 4, 512] (token-block subtiles),
            # ring of 3: chunk a+3 reuses chunk a's space.
            xkvb = []

            def load_xkvb(a):
                t = kvln_pool.tile([P, 4, E], BF16, name=f"xkvb{a}",
                                   tag="xkvb", bufs=3)
                nc.sync.dma_start(
                    t[:], xkv_d[a * 512:(a + 1) * 512, :].rearrange(
                        "(u p) c -> p u c", p=P)
                )
                xkvb.append(t)

            load_xkvb(0)
            load_xkvb(1)
            crit[0] = load_pack(0, "crit", C_END)
            load_xkvb(2)
            load_xkvb(3)

            # ---- LN1(x_q, layer 0), two halves ----
            hqT8_l0 = work.tile([P, EC, NQ], FP8, name="hqT8_l0", tag="actT")
            for half in range(2):
                hn2 = ln_hn2()
                for qc in (2 * half, 2 * half + 1):
                    ln_q_tile(qc, hn2, eng="D")
                ln_txn(hn2, lnp, hqT8_l0, half * 256, eng="A", nu=2)

            # ---- startup: x_kv LN + XBAR transpose + fp8 quantize.
            #      Stats run one batch ahead of the norm/xbar chains so
            #      DVE's queue never blocks the next batch's stats. ----
            mv4s, rstd4s = {}, {}

            def kv_stats(b):
                mv4 = stats_pool.tile([P, 2, 4], F32, name="mv4", bufs=4)
                for u in range(4):
                    ln_stats(xkvb[b][:, u, :], mv4[:, :, u])
                rstd4 = stats_pool.tile([P, 4], F32, name="rstd4", bufs=4)
                rsqrt_chain(mv4[:, 1, :], rstd4[:], 4)
                mv4s[b], rstd4s[b] = mv4, rstd4

            def kv_finish(b):
                khn4 = kvln_pool.tile([P, 4, E], BF16, name="khn4",
                                      tag="khn4", bufs=2)
                for u in range(4):
                    ln_norm(xkvb[b][:, u, :], mv4s[b][:, 0, u:u + 1],
                            rstd4s[b][:, u:u + 1], khn4, u, eng="DDAD"[u])
                ln_txn(khn4, lnp, hkvT8, b * 512, eng="AADA"[b], nu=4)

            kv_stats(0)
            kv_stats(1)
            kv_finish(0)
            kv_stats(2)
            kv_finish(1)
            kv_stats(3)
            kv_finish(2)
            kv_finish(3)

        restA = [None, None]
        restB = [None, None]
        rows = [None, None]

        # va pair tiles persist across layers (v columns overwritten per
        # layer, the constant denominator columns are set once here).
        va = [
            big.tile([P, 2, H, 2 * DH], FP8, name=f"va{g}", tag=f"va{g}")
            for g in range(GK)
        ]
        for g in range(GK):
            nc.gpsimd.memset(va[g][:, :, :, DH:2 * DH], 1.0 / QKS)

        def proj_q(l, hqT8_ap):
            """Q projection + pair-rearrange for layer l."""
            wq_s = dr(crit[l][:, C_WQ:C_WK], SE)
            bq = crit[l][:, C_BQ:C_END].bitcast(F32)
            q8f = work.tile([P, EC, NQ], FP8, name="q8f", tag="q8f")
            q8p = work.tile([P, EC, 2, NQ], FP8, name="q8p", tag="q8p")
            for m in range(EC):
                ps = ss_pool.tile([P, E], F32, name="pp", tag="ss")
                for half in range(2):
                    c0, c1 = half * 256, half * 256 + 256
                    for s in range(SE):
                        nc.tensor.matmul(
                            ps[:, c0:c1], wq_s[:, s, :, m * P:(m + 1) * P],
                            hqT8_ap[:, 2 * s:2 * s + 2, c0:c1],
                            start=(s == 0), stop=(s == SE - 1), perf_mode=DRM,
                            skip_group_check=True,
                        )
                if m % 2:
                    nc.scalar.activation(
                        q8f[:, m, :], ps[:], AF.Identity,
                        bias=bq[:, m:m + 1], scale=1.0 / QKS,
                    )
                else:
                    nc.vector.tensor_scalar(
                        q8f[:, m, :], ps[:], 1.0 / QKS, bq[:, m:m + 1],
                        op0=ALU.mult, op1=ALU.add,
                    )
            # pair layout: head h=2m+half at partitions [64*half, 64*half+32),
            # (p, j) <-> d-row 32*j + p.  4 plain partition-slice DMAs.
            for half in range(2):
                for jj in range(2):
                    r0 = 64 * half + 32 * jj
                    nc.sync.dma_start(
                        q8p[64 * half:64 * half + 32, :, jj, :],
                        q8f[r0:r0 + 32, :, :],
                    )
            return q8p

        def proj_kv(l):
            """K (with pair-rearrange) and V, interleaved per E-chunk."""
            wk_s = dr(crit[l][:, C_WK:C_WV], SE)
            wv_s = dr(crit[l][:, C_WV:C_BQ], SE)
            k8f = big.tile([P, EC, NKV], FP8, name="k8f", tag="k8f")
            k8p = big.tile([P, EC, 2, NKV], FP8, name="k8p", tag="k8p")
            # n-major: K over tokens [0, 1024) first so the first-half
            # k8p DMAs (and attention groups g<4) unblock after only two
            # kv batches; V for the same token range interleaved.
            def k_cols(n):
                for m in range(EC):
                    ps = ss_pool.tile([P, E], F32, name="pp", tag="ss")
                    for s in range(SE):
                        nc.tensor.matmul(
                            ps[:], wk_s[:, s, :, m * P:(m + 1) * P],
                            hkvT8[:, 2 * s:2 * s + 2, n * 512:(n + 1) * 512],
                            start=(s == 0), stop=(s == SE - 1), perf_mode=DRM,
                        )
                    if (m + n) % 2:
                        nc.scalar.mul(k8f[:, m, n * 512:(n + 1) * 512], ps[:], 1.0 / QKS)
                    else:
                        nc.vector.tensor_scalar_mul(
                            k8f[:, m, n * 512:(n + 1) * 512], ps[:], 1.0 / QKS
                        )

            def k8p_dmas(t0, t1):
                for half in range(2):
                    for jj in range(2):
                        r0 = 64 * half + 32 * jj
                        nc.sync.dma_start(
                            k8p[64 * half:64 * half + 32, :, jj, t0:t1],
                            k8f[r0:r0 + 32, :, t0:t1],
                        )

            def v_cols(mv):
                ps = ss_pool.tile([P, E], F32, name="pp", tag="ss")
                for s in range(SE):
                    nc.tensor.matmul(
                        ps[:], hkvT8[:, 2 * s:2 * s + 2, mv * P:(mv + 1) * P],
                        wv_s[:, s, :, :],
                        start=(s == 0), stop=(s == SE - 1), perf_mode=DRM,
                    )
                dst = va[mv // 2][:, mv % 2, :, 0:DH]
                src = ps[:].rearrange("p (h d) -> p h d", h=H)
                if mv % 4 == 3:
                    nc.vector.tensor_scalar_mul(dst, src, 1.0 / QKS)
                else:
                    nc.scalar.mul(dst, src, 1.0 / QKS)

            k_cols(0)
            k_cols(1)
            k8p_dmas(0, 1024)
            for mv in range(8):
                v_cols(mv)
            k_cols(2)
            k_cols(3)
            k8p_dmas(1024, NKV)
            for mv in range(8, 16):
                v_cols(mv)
            return k8p

        # ---- layer 0 Q/K/V ----
        q8p = proj_q(0, hqT8_l0[:])
        k8p = proj_kv(0)

        for l in range(L):
            # ---- attention, head-pairs (fp8 DR scores) ----
            aoT = work.tile([P, EC, NQ], FP8, name="aoT", tag="aoT")
            for m in range(EC):          # pair (h0, h1) = (2m, 2m+1)
                # bulk weight loads, issued mid-attention so their
                # transfers never block latency-critical DMAs.
                if l == 0 and m == 1:
                    restA[0] = load_pack(0, "restA", A_END)
                    rows[0] = load_rows(0)
                if l == 0 and m == 2:
                    crit[1] = load_pack(1, "crit", C_END)
                if l == 0 and m == 3:
                    restB[0] = load_pack(0, "restB", B_END)
                kst = [k8p[0:32, m, :, :], k8p[64:96, m, :, :]]
                qmv = [q8p[0:32, m, :, :], q8p[64:96, m, :, :]]
                pso = [
                    att_pool.tile([P, E], F32, name="ps_oT", tag="att")
                    for _ in range(2)
                ]
                exs = [[], []]
                for g in range(GK):
                    for j in range(2):
                        ex = ex_pool.tile([P, 2, NQ], FP8, name="ex", tag="ex")
                        for sub in range(2):
                            ps_s = ss_pool.tile([P, NQ], F32, name="ps_s", tag="ss")
                            c0 = (2 * g + sub) * P
                            nc.tensor.matmul(
                                ps_s[:], kst[j][:, :, c0:c0 + P], qmv[j],
                                start=True, stop=True, perf_mode=DRM,
                            )
                            pat = EXP_PAT if m % 2 == 0 else EXP_PAT_ODD
                            if pat[j][g] == "A":
                                nc.scalar.activation(
                                    ex[:, sub, :].bitcast(I8), ps_s[:],
                                    AF.Identity, bias=schb_col[:], scale=SCH_A,
                                )
                            else:
                                nc.vector.tensor_scalar(
                                    ex[:, sub, :].bitcast(I8), ps_s[:],
                                    SCH_A, SCH_B, op0=ALU.mult, op1=ALU.add,
                                )
                        exs[j].append(ex)
                    if g >= 1:
                        for j in range(2):
                            nc.tensor.matmul(
                                pso[j][:], va[g - 1][:, :, 2 * m + j, :],
                                exs[j][g - 1][:],
                                start=(g == 1), stop=False, perf_mode=DRM,
                            )
                for j in range(2):
                    nc.tensor.matmul(
                        pso[j][:], va[GK - 1][:, :, 2 * m + j, :],
                        exs[j][GK - 1][:],
                        start=False, stop=True, perf_mode=DRM,
                    )
                    # normalize: aoT = (x8 unnorm) * rcp(sum/8) = x64 attnout
                    rcp = stats_pool.tile([DH, NQ], BF16, name="rcp", bufs=2)
                    with nc.allow_low_precision(reason="bf16 softmax denom"):
                        nc.vector.reciprocal(rcp[:], pso[j][DH:P, :])
                    nc.vector.tensor_tensor(
                        aoT[64 * j:64 * j + 64, m, :], pso[j][0:DH, :],
                        rcp[:], op=ALU.mult,
                    )

            # ---- out-proj (DR fp8) + bo row + residual; LN2 interleaved ----
            wo_s = dr(restA[l][:, A_WO:A_W1], SE)
            h2T8 = work.tile([P, EC, NQ], FP8, name="h2T8", tag="actT")
            h2r8 = (work.tile([P, EC, NQ], FP8, name="h2r8", tag="h2r8")
                    if H2R else None)
            hn2 = ln_hn2()
            for qc in range(QC):
                ps = ss_pool.tile([P, E], F32, name="pp", tag="ss")
                nc.tensor.matmul(
                    ps[:], ones1[:], rows[l][:, 0, :], start=True, stop=False,
                    skip_group_check=True,
                )
                for s in range(SE):
                    nc.tensor.matmul(
                        ps[:], aoT[:, 2 * s:2 * s + 2, qc * P:(qc + 1) * P],
                        wo_s[:, s, :, :],
                        start=False, stop=(s == SE - 1), perf_mode=DRM,
                        skip_group_check=True,
                    )
                nc.vector.scalar_tensor_tensor(
                    xq[qc], ps[:], 1.0 / (AOS * WS), xq[qc],
                    op0=ALU.mult, op1=ALU.add,
                )
                ln_q_tile(qc, hn2, eng="D")
                if qc % 2 == 1:
                    ln_txn(hn2, lnp, h2T8, (qc - 1) * P, resT8=h2r8,
                           eng="D", nu=2)
                    if qc == 1:
                        hn2 = ln_hn2()
            if l + 1 < L:
                restA[1] = load_pack(1, "restA", A_END)
                rows[1] = load_rows(1)

            # ---- hoisted K/V of layer l+1 (depend only on hkvT): emitted
            #      here so the PE fills the LN2-chain gap before FFN1. ----
            if l + 1 < L:
                k8p = proj_kv(l + 1)

            # ---- FFN1 (DR fp8): g^T = gelu(w1^T @ h2^T / 64 + b1) ----
            w1_s = dr(restA[l][:, A_W1:A_W1R], SE)
            w1r_s = dr(restA[l][:, A_W1R:A_END], SE)
            b1 = restB[l][:, B_B1:B_END].bitcast(F32)
            gT8 = big.tile([P, MC, NQ], FP8, name="gT8", tag="gT8")
            for m in range(MC):
                ps = ss_pool.tile([P, E], F32, name="pp", tag="ss")
                for half in range(2):
                    c0, c1 = half * 256, half * 256 + 256
                    for s in range(SE):
                        nc.tensor.matmul(
                            ps[:, c0:c1], w1_s[:, s, :, m * P:(m + 1) * P],
                            h2T8[:, 2 * s:2 * s + 2, c0:c1],
                            start=(s == 0), stop=False, perf_mode=DRM,
                            skip_group_check=True,
                        )
                    for s in range(SE):
                        nc.tensor.matmul(
                            ps[:, c0:c1], w1r_s[:, s, :, m * P:(m + 1) * P],
                            h2T8[:, 2 * s:2 * s + 2, c0:c1],
                            start=False, stop=(not H2R and s == SE - 1),
                            perf_mode=DRM, skip_group_check=True,
                        )
                    if H2R:
                        for s in range(SE):
                            nc.tensor.matmul(
                                ps[:, c0:c1], w1_s[:, s, :, m * P:(m + 1) * P],
                                h2r8[:, 2 * s:2 * s + 2, c0:c1],
                                start=False, stop=(s == SE - 1), perf_mode=DRM,
                                skip_group_check=True,
                            )
                nc.scalar.activation(
                    gT8[:, m, :], ps[:], AF.Gelu,
                    bias=b1[:, m:m + 1], scale=1.0 / WS,
                )

            # ---- FFN2 (DR fp8) + b2 row + residual; next LN1 interleaved ----
            w2_s = dr(restB[l][:, B_W2:B_W2R], SM)
            w2r_s = dr(restB[l][:, B_W2R:B_B1], SM)
            if l + 1 < L:
                hqT8 = work.tile([P, EC, NQ], FP8, name="hqT8", tag="actT2")
                hn2 = ln_hn2()
            for qc in range(QC):
                ps = ss_pool.tile([P, E], F32, name="pp", tag="ss")
                nc.tensor.matmul(
                    ps[:], ones1[:], rows[l][:, 1, :], start=True, stop=False,
                    skip_group_check=True,
                )
                for s in range(SM):
                    nc.tensor.matmul(
                        ps[:], gT8[:, 2 * s:2 * s + 2, qc * P:(qc + 1) * P],
                        w2_s[:, s, :, :],
                        start=False, stop=(not W2R and s == SM - 1),
                        perf_mode=DRM, skip_group_check=True,
                    )
                if W2R:
                    for s in range(SM):
                        nc.tensor.matmul(
                            ps[:], gT8[:, 2 * s:2 * s + 2, qc * P:(qc + 1) * P],
                            w2r_s[:, s, :, :],
                            start=False, stop=(s == SM - 1), perf_mode=DRM,
                            skip_group_check=True,
                        )
                nc.vector.scalar_tensor_tensor(
                    xq[qc], ps[:], 1.0 / WS, xq[qc], op0=ALU.mult, op1=ALU.add,
                )
                if l + 1 < L:
                    ln_q_tile(qc, hn2, eng="D")
                    if qc % 2 == 1:
                        ln_txn(hn2, lnp, hqT8, (qc - 1) * P, eng="D", nu=2)
                        if qc == 1:
                            hn2 = ln_hn2()
                else:
                    nc.sync.dma_start(y_d[qc * P:(qc + 1) * P, :], xq[qc])
            if l + 1 < L:
                restB[1] = load_pack(1, "restB", B_END)
                q8p = proj_q(l + 1, hqT8[:])

    nc.compile()
    return nc


def get_nc():
    if "nc" not in _CACHE:
        _CACHE["nc"] = _build()
    return _CACHE["nc"]


def _fp8(a):
    return np.clip(np.asarray(a, np.float32), -240.0, 240.0).astype(
        ml_dtypes.float8_e4m3
    )


def _bf16(a):
    return np.asarray(a, np.float32).astype(ml_dtypes.bfloat16)


def _rearr_dr(w8, S):
    """[S*2*128, C] (row-major contraction) -> [128, S*2*C] DR pair layout."""
    C = w8.shape[1]
    return np.ascontiguousarray(
        w8.reshape(S, 2, P, C).transpose(2, 0, 1, 3).reshape(P, S * 2 * C)
    )


def _cols(v):
    """[k*128] -> [128, k]: column m holds v[m*128:(m+1)*128]."""
    k = v.shape[0] // P
    return np.ascontiguousarray(np.asarray(v, np.float32).reshape(k, P).T)


def _u8(a):
    return np.ascontiguousarray(a).view(np.uint8)


def kernel(**inputs) -> np.ndarray:
    x_q = np.asarray(inputs["x_q"], np.float32)
    x_kv = np.asarray(inputs["x_kv"], np.float32)
    wq = np.asarray(inputs["wq"], np.float32)
    wkv = np.asarray(inputs["wkv"], np.float32)
    wo = np.asarray(inputs["wo"], np.float32)
    bo = np.asarray(inputs["bo"], np.float32)
    w1 = np.asarray(inputs["w1"], np.float32)
    b1 = np.asarray(inputs["b1"], np.float32)
    w2 = np.asarray(inputs["w2"], np.float32)
    b2 = np.asarray(inputs["b2"], np.float32)
    ln1_g = np.asarray(inputs["ln1_g"], np.float32)
    ln1_b = np.asarray(inputs["ln1_b"], np.float32)
    ln2_g = np.asarray(inputs["ln2_g"], np.float32)
    ln2_b = np.asarray(inputs["ln2_b"], np.float32)

    shared = {}
    for l in range(L):
        wk_f = wkv[l][:, :E]
        wv_f = wkv[l][:, E:]
        wq_eff = ln1_g[l][:, None] * wq[l]
        wk_eff = ln1_g[l][:, None] * wk_f
        wv_eff = ln1_g[l][:, None] * wv_f
        bq_eff = ln1_b[l] @ wq[l]
        bv_eff = ln1_b[l] @ wv_f
        bo_eff = bo[l] + bv_eff @ wo[l]
        w1_eff = ln2_g[l][:, None] * w1[l]
        b1_eff = ln2_b[l] @ w1[l] + b1[l]

        wq8 = _rearr_dr(_fp8(WS * wq_eff), SE)
        wk8 = _rearr_dr(_fp8(WS * wk_eff), SE)
        wv8 = _rearr_dr(_fp8(WS * wv_eff), SE)
        wo8 = _rearr_dr(_fp8(WS * wo[l]), SE)
        w18 = _rearr_dr(_fp8(WS * w1_eff), SE)
        w28 = _rearr_dr(_fp8(WS * w2[l]), SM)
        w1r8 = _rearr_dr(_fp8(WS * w1_eff - _fp8(WS * w1_eff).astype(np.float32)), SE)
        w2r8 = _rearr_dr(_fp8(WS * w2[l] - _fp8(WS * w2[l]).astype(np.float32)), SM)
        bq_c = _cols(QKS * bq_eff)      # [128, 4] f32
        b1_c = _cols(b1_eff)            # [128, 16] f32

        crit = np.concatenate(
            [_u8(wq8), _u8(wk8), _u8(wv8), _u8(bq_c)], axis=1)
        restA = np.concatenate([_u8(wo8), _u8(w18), _u8(w1r8)], axis=1)
        restB = np.concatenate([_u8(w28), _u8(w2r8), _u8(b1_c)], axis=1)
        rows2 = np.stack(
            [_bf16(AOS * WS * bo_eff), _bf16(WS * b2[l])], axis=0)[None]
        shared.update({
            f"crit_{l}": crit.view(ml_dtypes.float8_e4m3),
            f"restA_{l}": restA.view(ml_dtypes.float8_e4m3),
            f"restB_{l}": restB.view(ml_dtypes.float8_e4m3),
            f"rows_{l}": rows2,
        })

    in_maps = []
    for c in range(8):
        b, qc = c // 4, c % 4
        m = dict(shared)
        m["xq"] = np.ascontiguousarray(x_q[b, qc * NQ:(qc + 1) * NQ, :])
        m["xkv"] = np.ascontiguousarray(_bf16(x_kv[b]))
        in_maps.append(m)

    nc = get_nc()
    res = bass_utils.run_bass_kernel_spmd(nc, in_maps, core_ids=list(range(8)))

    out = np.empty((2, 2048, E), np.float32)
    for c in range(8):
        b, qc = c // 4, c % 4
        out[b, qc * NQ:(qc + 1) * NQ, :] = res.results[c]["y"]
    return out


# revision 81
# speedup vs baseline: 1.2515x; 1.0007x over previous
"""Trainium2 Bass kernel for a 2-layer cross-encoder (CrossEncoder).

Model: B=2, NQ=NKV=2048, E=512, H=8 (d_head=64), MLP=2048, depth=2, fp32 I/O.

Sharding (8 cores, no collectives): core c handles batch b=c//4 and query
rows [qc*512, (qc+1)*512) with qc=c%4.  Each core computes the full KV
projections for its batch so every core produces its output slice
independently.

Key structure (v4):
 - All heavy matmuls fp8e4m3 + DoubleRow (0.5 cyc/row), including the
   scores matmul (q/k stored as fp8 DR pair tiles, produced by a cheap
   SBUF->SBUF DMA partition-rearrange; head h sits at base partition
   0/64 of its E-chunk pair tile so the PE base-partition rule holds).
 - Attention runs head-PAIRS interleaved: the two heads' exp chains use
   opposite engines per group, so Act and DVE both stay fed and the
   scores->exp->attnV sem latency is hidden.
 - LayerNorm activations transposed by the DMA XBAR (dma_start_transpose)
   instead of PE identity matmuls + PSUM round trips; fp8 quantize runs
   from SBUF (2x/4x DVE modes).  rstd = 1/sqrt(var+eps) via bit trick +
   1 Newton step on Pool (batched for the kv setup).
 - exp() is Schraudolph-only (int8 round writes fp8e4m3 bits directly),
   split Act (Identity activation) / DVE; Act's only table is Gelu.
 - Softmax denominator comes free from 64 constant columns per head in V.
 - x_kv is uploaded bf16; all weights of a layer arrive as 3 packed DMAs
   staged so bulk transfers never sit in front of latency-critical XBAR
   or pair-rearrange DMAs in the shared DMA queue.
 - K/V of layer l+1 (which depend only on hkvT) are hoisted between
   FFN1(l) and FFN2(l) to fill idle engines there; LN2 interleaves with
   the O-proj residuals, next-layer LN1 with the FFN2 residuals.
"""

import numpy as np
import ml_dtypes

import concourse.bass as bass
import concourse.bacc as bacc
import concourse.mybir as mybir
import concourse.tile as tile
from concourse import bass_utils
from contextlib import ExitStack

P = 128
E = 512
EC = E // P        # 4 chunks of the embedding dim
SE = EC // 2       # 2 DoubleRow super-chunks
NQ = 512           # query rows per core
QC = NQ // P       # 4 query chunks
NKV = 2048
KC = NKV // P      # 16 key chunks of 128
KN = NKV // 512    # 4 key chunks of 512
GK = KC // 2       # 8 key pair-groups
H = 8
DH = 64
MLP = 2048
MC = MLP // P      # 16 mlp chunks of 128
SM = MC // 2       # 8 DoubleRow super-chunks
L = 2
LN_EPS = 1e-5
F32 = mybir.dt.float32
I32 = mybir.dt.int32
BF16 = mybir.dt.bfloat16
FP8 = mybir.dt.float8e4
I8 = mybir.dt.int8
AF = mybir.ActivationFunctionType
ALU = mybir.AluOpType
DRM = mybir.MatmulPerfMode.DoubleRow

WS = 64.0                       # fp8 weight pre-scale (host side)
QKS = 8.0                       # q/k storage scale
SCALE = DH ** -0.5
EXPS = SCALE / (QKS * QKS)      # exp scale applied to scores psum (=1/512)
AOS = 64.0                      # attnout storage scale (fp8 subnormal guard)
SCH_A = (8.0 / np.log(2.0)) * EXPS   # Schraudolph slope for fp8e4 bits
SCH_B = 56.0 - 0.47                  # fp8e4 exponent bias term - rms shift
RSQRT_MAGIC = 0x5F3759DF
# per-(head-in-pair, group) exp engine: A=Act(Identity act) D=DVE.
# Anti-aligned so the two heads of a pair use opposite engines; 9A/7D
# because DVE also owns the softmax divide.
EXP_PAT = ["AADADADA", "ADADADAA"]
EXP_PAT_ODD = ["AADADADA", "DDADADAA"]   # 9A/7D for odd pairs (balance)
H2R = False                              # FFN1 activation-residual pass
W2R = False                              # FFN2 weight-residual pass

# packed-weight byte offsets (per partition)
C_WQ, C_WK, C_WV, C_BQ, C_END = 0, 2048, 4096, 6144, 6160
A_WO, A_W1, A_W1R, A_END = 0, 2048, 10240, 18432
B_W2, B_W2R, B_B1, B_END = 0, 8192, 16384, 16448

_CACHE = {}


def _build():
    """Build the per-core Bass program (identical on all 8 cores)."""
    nc = bacc.Bacc("TRN2", target_bir_lowering=False, debug=False, num_devices=8)

    xq_d = nc.dram_tensor("xq", [NQ, E], F32, kind="ExternalInput").ap()
    xkv_d = nc.dram_tensor("xkv", [NKV, E], BF16, kind="ExternalInput").ap()
    wd = []
    for l in range(L):
        wd.append({
            "crit": nc.dram_tensor(f"crit_{l}", [P, C_END], FP8, kind="ExternalInput").ap(),
            "restA": nc.dram_tensor(f"restA_{l}", [P, A_END], FP8, kind="ExternalInput").ap(),
            "restB": nc.dram_tensor(f"restB_{l}", [P, B_END], FP8, kind="ExternalInput").ap(),
            "rows": nc.dram_tensor(f"rows_{l}", [1, 2, E], BF16, kind="ExternalInput").ap(),
        })
    y_d = nc.dram_tensor("y", [NQ, E], F32, kind="ExternalOutput").ap()

    with tile.TileContext(nc) as tc, ExitStack() as ctx:
        const_pool = ctx.enter_context(tc.tile_pool(name="const", bufs=1))
        ones1 = const_pool.tile([1, P], BF16)
        nc.gpsimd.memset(ones1[:], 1.0)
        schb_col = const_pool.tile([P, 1], F32)
        nc.gpsimd.memset(schb_col[:], SCH_B)

        stats_pool = ctx.enter_context(tc.tile_pool(name="stats", bufs=12))

        def rsqrt_chain(var_ap, rstd_ap, w):
            """rstd = 1/sqrt(var+eps) on Pool via bit trick + 1 Newton step."""
            ve = stats_pool.tile([P, 4], F32, name="ve")[:, :w]
            nc.gpsimd.tensor_scalar_add(ve, var_ap, LN_EPS)
            y0 = stats_pool.tile([P, 4], F32, name="y0")[:, :w]
            # int bit-trick ops run on DVE (Pool's Q7 rejects int shifts)
            nc.vector.tensor_scalar(
                y0.bitcast(I32), ve.bitcast(I32), 1, 0,
                op0=ALU.logical_shift_right, op1=ALU.bypass,
            )
            nc.vector.tensor_scalar(
                y0.bitcast(I32), y0.bitcast(I32), -1, RSQRT_MAGIC,
                op0=ALU.mult, op1=ALU.add,
            )
            t = stats_pool.tile([P, 4], F32, name="t")[:, :w]
            nc.gpsimd.tensor_tensor(t, y0, y0, op=ALU.mult)
            nc.gpsimd.tensor_tensor(t, t, ve, op=ALU.mult)
            nc.gpsimd.tensor_scalar(t, t, -0.5, 1.5, op0=ALU.mult, op1=ALU.add)
            nc.gpsimd.tensor_tensor(rstd_ap, y0, t, op=ALU.mult)

        def ln_stats(x_ap, mv_ap):
            bnst = stats_pool.tile([P, 6], F32, name="bnst")
            nc.vector.bn_stats(bnst[:], x_ap)
            nc.vector.bn_aggr(mv_ap, bnst[:])

        # norm for one [128, E] tile into slot u of a 4-wide hn4 buffer.
        def ln_norm(src_ap, mean_ap, rstd_ap, hn4, u, eng="D"):
            if eng == "P":
                nc.gpsimd.tensor_scalar(
                    hn4[:, u, :], src_ap, mean_ap, rstd_ap,
                    op0=ALU.subtract, op1=ALU.mult,
                )
            else:
                nc.vector.tensor_scalar(
                    hn4[:, u, :], src_ap, mean_ap, rstd_ap,
                    op0=ALU.subtract, op1=ALU.mult,
                )

        # one XBAR transpose + one quantize for an nu-tile hn batch
        # (nu*128 tokens).  dst slice [:, :, tok0:tok0+nu*128].
        def ln_txn(hn, t_pool, dstT8, tok0, resT8=None, eng="D", nu=4):
            ht = t_pool.tile([P, 4 * EC, P], BF16, name="ht", tag="ht",
                             bufs=2)[:, :nu * EC, :]
            nc.sync.dma_start_transpose(
                ht, hn[:].rearrange("p u c -> p (u c)")
            )
            dst = dstT8[:, :, tok0:tok0 + nu * P].rearrange(
                "p c (u t) -> p u c t", u=nu
            )
            src = ht.rearrange("p (u c) t -> p u c t", u=nu)
            if eng == "P":
                nc.gpsimd.tensor_copy(dst, src)
            elif eng == "D":
                nc.vector.tensor_copy(dst, src)
            else:
                nc.scalar.copy(dst, src)
            if resT8 is not None:
                # STT requires <=3D APs: one op per 128-token sub-block.
                for u in range(nu):
                    t0 = tok0 + u * P
                    nc.vector.scalar_tensor_tensor(
                        resT8[:, :, t0:t0 + P], dstT8[:, :, t0:t0 + P], -1.0,
                        ht[:, u * EC:(u + 1) * EC, :], op0=ALU.mult, op1=ALU.add,
                    )

        # Residual stream: one [128, 4, 512] fp32 tile (qc-major subtiles).
        xq_pool = ctx.enter_context(tc.tile_pool(name="xq", bufs=1))
        xqb = xq_pool.tile([P, QC, E], F32, name="xqb", tag="xqb")
        nc.sync.dma_start(xqb[:], xq_d.rearrange("(a p) c -> p a c", p=P))
        xq = [xqb[:, qc, :] for qc in range(QC)]

        # hkv^T fp8 DR tile [128, 4, NKV]: (c, p) holds E-row c*128+p.
        hkvT_pool = ctx.enter_context(tc.tile_pool(name="hkvT", bufs=1))
        hkvT8 = hkvT_pool.tile([P, EC, NKV], FP8, name="hkvT8", tag="hkvT8")

        # PSUM pools (8 banks): shared 1-bank ring 6 + att 2 = 8.
        ss_pool = ctx.enter_context(tc.tile_pool(name="ss", bufs=6, space="PSUM"))
        att_pool = ctx.enter_context(tc.tile_pool(name="attp", bufs=2, space="PSUM"))

        wpool = ctx.enter_context(tc.tile_pool(name="w", bufs=1))

        def load_pack(l, which, sz):
            t = wpool.tile([P, sz], FP8, name=f"{which}_{l}")
            nc.sync.dma_start(t[:], wd[l][which])
            return t

        def load_rows(l):
            t = wpool.tile([1, 2, E], BF16, name=f"rows_{l}")
            nc.sync.dma_start(t[:], wd[l]["rows"])
            return t

        def dr(ap, S):
            return ap.rearrange("p (s j c) -> p s j c", s=S, j=2)

        work = ctx.enter_context(tc.tile_pool(name="work", bufs=1))
        big = ctx.enter_context(tc.tile_pool(name="big", bufs=1))
        ex_pool = ctx.enter_context(tc.tile_pool(name="ex", bufs=6))
        lnp = ctx.enter_context(tc.tile_pool(name="lnp", bufs=4))

        def ln_hn2():
            return lnp.tile([P, 2, E], BF16, name="hn2", tag="hn2", bufs=4)

        # q-side LN: per-tile stats+norm into hn2 slot qc%2; a following
        # ln_txn(nu=2) finishes each half.
        def ln_q_tile(qc, hn2, eng="D"):
            mv = stats_pool.tile([P, 2], F32, name="mv")
            ln_stats(xq[qc], mv[:])
            rstd = stats_pool.tile([P, 1], F32, name="rstd")
            rsqrt_chain(mv[:, 1:2], rstd[:], 1)
            ln_norm(xq[qc], mv[:, 0:1], rstd[:], hn2, qc % 2, eng=eng)

        crit = [None, None]
        with tc.tile_pool(name="kvln", bufs=4) as kvln_pool:
            # x_kv bf16, 4 chunks of [128, 4, 512] (token-block subtiles),
            # ring of 3: chunk a+3 reuses chunk a's space.
            xkvb = []

            def load_xkvb(a):
                t = kvln_pool.tile([P, 4, E], BF16, name=f"xkvb{a}",
                                   tag="xkvb", bufs=3)
                nc.sync.dma_start(
                    t[:], xkv_d[a * 512:(a + 1) * 512, :].rearrange(
                        "(u p) c -> p u c", p=P)
                )
                xkvb.append(t)

            load_xkvb(0)
            load_xkvb(1)
            crit[0] = load_pack(0, "crit", C_END)
            load_xkvb(2)
            load_xkvb(3)

            # ---- LN1(x_q, layer 0), two halves ----
            hqT8_l0 = work.tile([P, EC, NQ], FP8, name="hqT8_l0", tag="actT")
            for half in range(2):
                hn2 = ln_hn2()
                for qc in (2 * half, 2 * half + 1):
                    ln_q_tile(qc, hn2, eng="D")
                ln_txn(hn2, lnp, hqT8_l0, half * 256, eng="A", nu=2)

            # ---- startup: x_kv LN + XBAR transpose + fp8 quantize.
            #      Stats run one batch ahead of the norm/xbar chains so
            #      DVE's queue never blocks the next batch's stats. ----
            mv4s, rstd4s = {}, {}

            def kv_stats(b):
                mv4 = stats_pool.tile([P, 2, 4], F32, name="mv4", bufs=4)
                for u in range(4):
                    ln_stats(xkvb[b][:, u, :], mv4[:, :, u])
                rstd4 = stats_pool.tile([P, 4], F32, name="rstd4", bufs=4)
                rsqrt_chain(mv4[:, 1, :], rstd4[:], 4)
                mv4s[b], rstd4s[b] = mv4, rstd4

            def kv_finish(b):
                khn4 = kvln_pool.tile([P, 4, E], BF16, name="khn4",
                                      tag="khn4", bufs=2)
                for u in range(4):
                    ln_norm(xkvb[b][:, u, :], mv4s[b][:, 0, u:u + 1],
                            rstd4s[b][:, u:u + 1], khn4, u, eng="DDAD"[u])
                ln_txn(khn4, lnp, hkvT8, b * 512, eng="AADA"[b], nu=4)

            kv_stats(0)
            kv_stats(1)
            kv_finish(0)
            kv_stats(2)
            kv_finish(1)
            kv_stats(3)
            kv_finish(2)
            kv_finish(3)

        restA = [None, None]
        restB = [None, None]
        rows = [None, None]

        # va pair tiles persist across layers (v columns overwritten per
        # layer, the constant denominator columns are set once here).
        va = [
            big.tile([P, 2, H, 2 * DH], FP8, name=f"va{g}", tag=f"va{g}")
            for g in range(GK)
        ]
        for g in range(GK):
            nc.gpsimd.memset(va[g][:, :, :, DH:2 * DH], 1.0 / QKS)

        def proj_q(l, hqT8_ap):
            """Q projection + pair-rearrange for layer l."""
            wq_s = dr(crit[l][:, C_WQ:C_WK], SE)
            bq = crit[l][:, C_BQ:C_END].bitcast(F32)
            q8f = work.tile([P, EC, NQ], FP8, name="q8f", tag="q8f")
            q8p = work.tile([P, EC, 2, NQ], FP8, name="q8p", tag="q8p")
            for m in range(EC):
                ps = ss_pool.tile([P, E], F32, name="pp", tag="ss")
                for half in range(2):
                    c0, c1 = half * 256, half * 256 + 256
                    for s in range(SE):
                        nc.tensor.matmul(
                            ps[:, c0:c1], wq_s[:, s, :, m * P:(m + 1) * P],
                            hqT8_ap[:, 2 * s:2 * s + 2, c0:c1],
                            start=(s == 0), stop=(s == SE - 1), perf_mode=DRM,
                            skip_group_check=True,
                        )
                if m % 2:
                    nc.scalar.activation(
                        q8f[:, m, :], ps[:], AF.Identity,
                        bias=bq[:, m:m + 1], scale=1.0 / QKS,
                    )
                else:
                    nc.vector.tensor_scalar(
                        q8f[:, m, :], ps[:], 1.0 / QKS, bq[:, m:m + 1],
                        op0=ALU.mult, op1=ALU.add,
                    )
            # pair layout: head h=2m+half at partitions [64*half, 64*half+32),
            # (p, j) <-> d-row 32*j + p.  4 plain partition-slice DMAs.
            for half in range(2):
                for jj in range(2):
                    r0 = 64 * half + 32 * jj
                    nc.sync.dma_start(
                        q8p[64 * half:64 * half + 32, :, jj, :],
                        q8f[r0:r0 + 32, :, :],
                    )
            return q8p

        def proj_kv(l):
            """K (with pair-rearrange) and V, interleaved per E-chunk."""
            wk_s = dr(crit[l][:, C_WK:C_WV], SE)
            wv_s = dr(crit[l][:, C_WV:C_BQ], SE)
            k8f = big.tile([P, EC, NKV], FP8, name="k8f", tag="k8f")
            k8p = big.tile([P, EC, 2, NKV], FP8, name="k8p", tag="k8p")
            # n-major: K over tokens [0, 1024) first so the first-half
            # k8p DMAs (and attention groups g<4) unblock after only two
            # kv batches; V for the same token range interleaved.
            def k_cols(n):
                for m in range(EC):
                    ps = ss_pool.tile([P, E], F32, name="pp", tag="ss")
                    for s in range(SE):
                        nc.tensor.matmul(
                            ps[:], wk_s[:, s, :, m * P:(m + 1) * P],
                            hkvT8[:, 2 * s:2 * s + 2, n * 512:(n + 1) * 512],
                            start=(s == 0), stop=(s == SE - 1), perf_mode=DRM,
                        )
                    if (m + n) % 2:
                        nc.scalar.mul(k8f[:, m, n * 512:(n + 1) * 512], ps[:], 1.0 / QKS)
                    else:
                        nc.vector.tensor_scalar_mul(
                            k8f[:, m, n * 512:(n + 1) * 512], ps[:], 1.0 / QKS
                        )

            def k8p_dmas(t0, t1):
                for half in range(2):
                    for jj in range(2):
                        r0 = 64 * half + 32 * jj
                        nc.sync.dma_start(
                            k8p[64 * half:64 * half + 32, :, jj, t0:t1],
                            k8f[r0:r0 + 32, :, t0:t1],
                        )

            def v_cols(mv):
                ps = ss_pool.tile([P, E], F32, name="pp", tag="ss")
                for s in range(SE):
                    nc.tensor.matmul(
                        ps[:], hkvT8[:, 2 * s:2 * s + 2, mv * P:(mv + 1) * P],
                        wv_s[:, s, :, :],
                        start=(s == 0), stop=(s == SE - 1), perf_mode=DRM,
                    )
                dst = va[mv // 2][:, mv % 2, :, 0:DH]
                src = ps[:].rearrange("p (h d) -> p h d", h=H)
                if mv % 4 == 3:
                    nc.vector.tensor_scalar_mul(dst, src, 1.0 / QKS)
                else:
                    nc.scalar.mul(dst, src, 1.0 / QKS)

            k_cols(0)
            k_cols(1)
            k8p_dmas(0, 1024)
            for mv in range(8):
                v_cols(mv)
            k_cols(2)
            k_cols(3)
            k8p_dmas(1024, NKV)
            for mv in range(8, 16):
                v_cols(mv)
            return k8p

        # ---- layer 0 Q/K/V ----
        q8p = proj_q(0, hqT8_l0[:])
        k8p = proj_kv(0)

        for l in range(L):
            # ---- attention, head-pairs (fp8 DR scores) ----
            aoT = work.tile([P, EC, NQ], FP8, name="aoT", tag="aoT")
            for m in range(EC):          # pair (h0, h1) = (2m, 2m+1)
                # bulk weight loads, issued mid-attention so their
                # transfers never block latency-critical DMAs.
                if l == 0 and m == 1:
                    restA[0] = load_pack(0, "restA", A_END)
                    rows[0] = load_rows(0)
                if l == 0 and m == 2:
                    crit[1] = load_pack(1, "crit", C_END)
                if l == 0 and m == 3:
                    restB[0] = load_pack(0, "restB", B_END)
                kst = [k8p[0:32, m, :, :], k8p[64:96, m, :, :]]
                qmv = [q8p[0:32, m, :, :], q8p[64:96, m, :, :]]
                pso = [
                    att_pool.tile([P, E], F32, name="ps_oT", tag="att")
                    for _ in range(2)
                ]
                exs = [[], []]
                for g in range(GK):
                    for j in range(2):
                        ex = ex_pool.tile([P, 2, NQ], FP8, name="ex", tag="ex")
                        for sub in range(2):
                            ps_s = ss_pool.tile([P, NQ], F32, name="ps_s", tag="ss")
                            c0 = (2 * g + sub) * P
                            nc.tensor.matmul(
                                ps_s[:], kst[j][:, :, c0:c0 + P], qmv[j],
                                start=True, stop=True, perf_mode=DRM,
                            )
                            pat = EXP_PAT if m % 2 == 0 else EXP_PAT_ODD
                            if pat[j][g] == "A":
                                nc.scalar.activation(
                                    ex[:, sub, :].bitcast(I8), ps_s[:],
                                    AF.Identity, bias=schb_col[:], scale=SCH_A,
                                )
                            else:
                                nc.vector.tensor_scalar(
                                    ex[:, sub, :].bitcast(I8), ps_s[:],
                                    SCH_A, SCH_B, op0=ALU.mult, op1=ALU.add,
                                )
                        exs[j].append(ex)
                    if g >= 1:
                        for j in range(2):
                            nc.tensor.matmul(
                                pso[j][:], va[g - 1][:, :, 2 * m + j, :],
                                exs[j][g - 1][:],
                                start=(g == 1), stop=False, perf_mode=DRM,
                            )
                for j in range(2):
                    nc.tensor.matmul(
                        pso[j][:], va[GK - 1][:, :, 2 * m + j, :],
                        exs[j][GK - 1][:],
                        start=False, stop=True, perf_mode=DRM,
                    )
                    # normalize: aoT = (x8 unnorm) * rcp(sum/8) = x64 attnout
                    rcp = stats_pool.tile([DH, NQ], BF16, name="rcp", bufs=2)
                    with nc.allow_low_precision(reason="bf16 softmax denom"):
                        nc.vector.reciprocal(rcp[:], pso[j][DH:P, :])
                    nc.vector.tensor_tensor(
                        aoT[64 * j:64 * j + 64, m, :], pso[j][0:DH, :],
                        rcp[:], op=ALU.mult,
                    )

            # ---- out-proj (DR fp8) + bo row + residual; LN2 interleaved ----
            wo_s = dr(restA[l][:, A_WO:A_W1], SE)
            h2T8 = work.tile([P, EC, NQ], FP8, name="h2T8", tag="actT")
            h2r8 = (work.tile([P, EC, NQ], FP8, name="h2r8", tag="h2r8")
                    if H2R else None)
            hn2 = ln_hn2()
            for qc in range(QC):
                ps = ss_pool.tile([P, E], F32, name="pp", tag="ss")
                nc.tensor.matmul(
                    ps[:], ones1[:], rows[l][:, 0, :], start=True, stop=False,
                    skip_group_check=True,
                )
                for s in range(SE):
                    nc.tensor.matmul(
                        ps[:], aoT[:, 2 * s:2 * s + 2, qc * P:(qc + 1) * P],
                        wo_s[:, s, :, :],
                        start=False, stop=(s == SE - 1), perf_mode=DRM,
                        skip_group_check=True,
                    )
                nc.vector.scalar_tensor_tensor(
                    xq[qc], ps[:], 1.0 / (AOS * WS), xq[qc],
                    op0=ALU.mult, op1=ALU.add,
                )
                ln_q_tile(qc, hn2, eng="D")
                if qc % 2 == 1:
                    ln_txn(hn2, lnp, h2T8, (qc - 1) * P, resT8=h2r8,
                           eng="D", nu=2)
                    if qc == 1:
                        hn2 = ln_hn2()
            if l + 1 < L:
                restA[1] = load_pack(1, "restA", A_END)
                rows[1] = load_rows(1)

            # ---- hoisted K/V of layer l+1 (depend only on hkvT): emitted
            #      here so the PE fills the LN2-chain gap before FFN1. ----
            if l + 1 < L:
                k8p = proj_kv(l + 1)

            # ---- FFN1 (DR fp8): g^T = gelu(w1^T @ h2^T / 64 + b1) ----
            w1_s = dr(restA[l][:, A_W1:A_W1R], SE)
            w1r_s = dr(restA[l][:, A_W1R:A_END], SE)
            b1 = restB[l][:, B_B1:B_END].bitcast(F32)
            gT8 = big.tile([P, MC, NQ], FP8, name="gT8", tag="gT8")
            for m in range(MC):
                ps = ss_pool.tile([P, E], F32, name="pp", tag="ss")
                for half in range(2):
                    c0, c1 = half * 256, half * 256 + 256
                    for s in range(SE):
                        nc.tensor.matmul(
                            ps[:, c0:c1], w1_s[:, s, :, m * P:(m + 1) * P],
                            h2T8[:, 2 * s:2 * s + 2, c0:c1],
                            start=(s == 0), stop=False, perf_mode=DRM,
                            skip_group_check=True,
                        )
                    for s in range(SE):
                        nc.tensor.matmul(
                            ps[:, c0:c1], w1r_s[:, s, :, m * P:(m + 1) * P],
                            h2T8[:, 2 * s:2 * s + 2, c0:c1],
                            start=False, stop=(not H2R and s == SE - 1),
                            perf_mode=DRM, skip_group_check=True,
                        )
                    if H2R:
                        for s in range(SE):
                            nc.tensor.matmul(
                                ps[:, c0:c1], w1_s[:, s, :, m * P:(m + 1) * P],
                                h2r8[:, 2 * s:2 * s + 2, c0:c1],
                                start=False, stop=(s == SE - 1), perf_mode=DRM,
                                skip_group_check=True,
                            )
                nc.scalar.activation(
                    gT8[:, m, :], ps[:], AF.Gelu,
                    bias=b1[:, m:m + 1], scale=1.0 / WS,
                )

            # ---- FFN2 (DR fp8) + b2 row + residual; next LN1 interleaved ----
            w2_s = dr(restB[l][:, B_W2:B_W2R], SM)
            w2r_s = dr(restB[l][:, B_W2R:B_B1], SM)
            if l + 1 < L:
                hqT8 = work.tile([P, EC, NQ], FP8, name="hqT8", tag="actT2")
                hn2 = ln_hn2()
            for qc in range(QC):
                ps = ss_pool.tile([P, E], F32, name="pp", tag="ss")
                nc.tensor.matmul(
                    ps[:], ones1[:], rows[l][:, 1, :], start=True, stop=False,
                    skip_group_check=True,
                )
                for s in range(SM):
                    nc.tensor.matmul(
                        ps[:], gT8[:, 2 * s:2 * s + 2, qc * P:(qc + 1) * P],
                        w2_s[:, s, :, :],
                        start=False, stop=(not W2R and s == SM - 1),
                        perf_mode=DRM, skip_group_check=True,
                    )
                if W2R:
                    for s in range(SM):
                        nc.tensor.matmul(
                            ps[:], gT8[:, 2 * s:2 * s + 2, qc * P:(qc + 1) * P],
                            w2r_s[:, s, :, :],
                            start=False, stop=(s == SM - 1), perf_mode=DRM,
                            skip_group_check=True,
                        )
                nc.vector.scalar_tensor_tensor(
                    xq[qc], ps[:], 1.0 / WS, xq[qc], op0=ALU.mult, op1=ALU.add,
                )
                if l + 1 < L:
                    ln_q_tile(qc, hn2, eng="D")
                    if qc % 2 == 1:
                        ln_txn(hn2, lnp, hqT8, (qc - 1) * P, eng="D", nu=2)
                        if qc == 1:
                            hn2 = ln_hn2()
                else:
                    nc.sync.dma_start(y_d[qc * P:(qc + 1) * P, :], xq[qc])
            if l + 1 < L:
                restB[1] = load_pack(1, "restB", B_END)
                q8p = proj_q(l + 1, hqT8[:])

    nc.compile()
    return nc


def get_nc():
    if "nc" not in _CACHE:
        _CACHE["nc"] = _build()
    return _CACHE["nc"]


def _fp8(a):
    return np.clip(np.asarray(a, np.float32), -240.0, 240.0).astype(
        ml_dtypes.float8_e4m3
    )


def _bf16(a):
    return np.asarray(a, np.float32).astype(ml_dtypes.bfloat16)


def _rearr_dr(w8, S):
    """[S*2*128, C] (row-major contraction) -> [128, S*2*C] DR pair layout."""
    C = w8.shape[1]
    return np.ascontiguousarray(
        w8.reshape(S, 2, P, C).transpose(2, 0, 1, 3).reshape(P, S * 2 * C)
    )


def _cols(v):
    """[k*128] -> [128, k]: column m holds v[m*128:(m+1)*128]."""
    k = v.shape[0] // P
    return np.ascontiguousarray(np.asarray(v, np.float32).reshape(k, P).T)


def _u8(a):
    return np.ascontiguousarray(a).view(np.uint8)


def kernel(**inputs) -> np.ndarray:
    x_q = np.asarray(inputs["x_q"], np.float32)
    x_kv = np.asarray(inputs["x_kv"], np.float32)
    wq = np.asarray(inputs["wq"], np.float32)
    wkv = np.asarray(inputs["wkv"], np.float32)
    wo = np.asarray(inputs["wo"], np.float32)
    bo = np.asarray(inputs["bo"], np.float32)
    w1 = np.asarray(inputs["w1"], np.float32)
    b1 = np.asarray(inputs["b1"], np.float32)
    w2 = np.asarray(inputs["w2"], np.float32)
    b2 = np.asarray(inputs["b2"], np.float32)
    ln1_g = np.asarray(inputs["ln1_g"], np.float32)
    ln1_b = np.asarray(inputs["ln1_b"], np.float32)
    ln2_g = np.asarray(inputs["ln2_g"], np.float32)
    ln2_b = np.asarray(inputs["ln2_b"], np.float32)

    shared = {}
    for l in range(L):
        wk_f = wkv[l][:, :E]
        wv_f = wkv[l][:, E:]
        wq_eff = ln1_g[l][:, None] * wq[l]
        wk_eff = ln1_g[l][:, None] * wk_f
        wv_eff = ln1_g[l][:, None] * wv_f
        bq_eff = ln1_b[l] @ wq[l]
        bv_eff = ln1_b[l] @ wv_f
        bo_eff = bo[l] + bv_eff @ wo[l]
        w1_eff = ln2_g[l][:, None] * w1[l]
        b1_eff = ln2_b[l] @ w1[l] + b1[l]

        wq8 = _rearr_dr(_fp8(WS * wq_eff), SE)
        wk8 = _rearr_dr(_fp8(WS * wk_eff), SE)
        wv8 = _rearr_dr(_fp8(WS * wv_eff), SE)
        wo8 = _rearr_dr(_fp8(WS * wo[l]), SE)
        w18 = _rearr_dr(_fp8(WS * w1_eff), SE)
        w28 = _rearr_dr(_fp8(WS * w2[l]), SM)
        w1r8 = _rearr_dr(_fp8(WS * w1_eff - _fp8(WS * w1_eff).astype(np.float32)), SE)
        w2r8 = _rearr_dr(_fp8(WS * w2[l] - _fp8(WS * w2[l]).astype(np.float32)), SM)
        bq_c = _cols(QKS * bq_eff)      # [128, 4] f32
        b1_c = _cols(b1_eff)            # [128, 16] f32

        crit = np.concatenate(
            [_u8(wq8), _u8(wk8), _u8(wv8), _u8(bq_c)], axis=1)
        restA = np.concatenate([_u8(wo8), _u8(w18), _u8(w1r8)], axis=1)
        restB = np.concatenate([_u8(w28), _u8(w2r8), _u8(b1_c)], axis=1)
        rows2 = np.stack(
            [_bf16(AOS * WS * bo_eff), _bf16(WS * b2[l])], axis=0)[None]
        shared.update({
            f"crit_{l}": crit.view(ml_dtypes.float8_e4m3),
            f"restA_{l}": restA.view(ml_dtypes.float8_e4m3),
            f"restB_{l}": restB.view(ml_dtypes.float8_e4m3),
            f"rows_{l}": rows2,
        })

    in_maps = []
    for c in range(8):
        b, qc = c // 4, c % 4
        m = dict(shared)
        m["xq"] = np.ascontiguousarray(x_q[b, qc * NQ:(qc + 1) * NQ, :])
        m["xkv"] = np.ascontiguousarray(_bf16(x_kv[b]))
        in_maps.append(m)

    nc = get_nc()
    res = bass_utils.run_bass_kernel_spmd(nc, in_maps, core_ids=list(range(8)))

    out = np.empty((2, 2048, E), np.float32)
    for c in range(8):
        b, qc = c // 4, c % 4
        out[b, qc * NQ:(qc + 1) * NQ, :] = res.results[c]["y"]
    return out
